# revision 1
# baseline (speedup 1.0000x reference)
"""Causal multi-head attention block (B=2, S=2048, D=768, H=12) on 8 trn2 cores.

Sharding: core c -> batch b = c//4 (data parallel), head group g = c%4
(tensor parallel, 3 heads per group). Each core computes its group's QKV
projection, causal attention, and a partial O-projection over its 192
z-columns. Host sums the 4 partials per batch and adds the biases that
commute through the math (v-bias and b_o).

On-core layout (everything "transposed", d on partitions, seq on free):
  xT   [768, 2048]   q/kT  [64*, 2048]      scores^T [keys, q]
so the softmax denominator comes free from a ones-column appended to V in
the PV matmul, and no on-chip transposes of activations are needed except
V (built via PE transpose from V^T).

The QKV projection uses a host-repacked weight matrix so every 128-wide
M-group is fully used:
  m0=[q_h0 q_h1] m1=[q_h2 v_h0] m2=[k_h0 k_h1] m3=[k_h2 v_h1] m4=[v_h2]
(q rows pre-scaled by 1/8; v bias folded into the host-side epilogue).

Matmul operands are float32r (full-rate fp32 on the PE). Scheduling
interleaves projection/transpose/O-proj work into the attention loop so
the scalar engine (exp) is never starved by a long PE FIFO stretch.
"""

import os
from collections import deque
from contextlib import ExitStack

import numpy as np

import concourse.tile as tile
from concourse import bacc, mybir
from concourse.bass_utils import run_bass_kernel_spmd
from concourse.masks import make_identity

F32 = mybir.dt.float32
F32R = mybir.dt.float32r
AF = mybir.ActivationFunctionType

B, S, D = 2, 2048, 768
NH, DH = 12, 64
HPC = 3            # heads per core
GD = HPC * DH      # 192 z-cols per core
KT, QT = 128, 512  # key tile (partitions), q tile (psum free)
NKT, NQT = S // KT, S // QT   # 16, 4
NTOK = S // 128    # 16 token tiles
NKD = D // 128     # 6 contraction tiles for the projections
WPK = 2 * GD + GD  # 576 packed projection rows


def build_bass():
    nc = bacc.Bacc(None)
    xT = nc.dram_tensor("xT", [D, S], F32, kind="ExternalInput")
    wpk = nc.dram_tensor("wpk", [D, WPK], F32, kind="ExternalInput")
    woT = nc.dram_tensor("woT", [GD, D], F32, kind="ExternalInput")
    bqk = nc.dram_tensor("bqk", [128, 4], F32, kind="ExternalInput")
    vones = nc.dram_tensor("vones", [128, 64], F32, kind="ExternalInput")
    out_p = nc.dram_tensor("out_p", [S, D], F32, kind="ExternalOutput")

    with tile.TileContext(nc) as tc, ExitStack() as ctx:
        const = ctx.enter_context(tc.tile_pool(name="const", bufs=1))
        ps = ctx.enter_context(tc.tile_pool(name="ps", bufs=6, space="PSUM"))
        psz = ctx.enter_context(tc.tile_pool(name="psz", bufs=2, space="PSUM"))
        expp = ctx.enter_context(tc.tile_pool(name="expp", bufs=9))
        small = ctx.enter_context(tc.tile_pool(name="small", bufs=4))

        xT_sb = const.tile([128, NKD, S], F32R)
        wpk_sb = const.tile([128, NKD, WPK], F32R)
        wo_a = const.tile([128, D], F32R)
        wo_b = const.tile([64, D], F32R)
        bqk_sb = const.tile([128, 4], F32)
        qT_sb = const.tile([128, 2, S], F32R)
        kT_sb = const.tile([128, 2, S], F32R)
        vvT = const.tile([128, 2, S], F32)
        v_aug = const.tile([128, HPC, NKT, DH + 1], F32R)
        zT01 = const.tile([128, S], F32R)
        zT2 = const.tile([64, S], F32R)
        ident = const.tile([128, 128], F32)
        ones64 = const.tile([1, 64], F32R)

        ones_stage = const.tile([128, 64], F32)
        make_identity(nc, ident[:])

        # ---- loads: k-interleaved so the first projection k-pairs unblock
        # early; everything not needed for (h0, qt0) comes after.
        xT_t = xT.rearrange("(t p) s -> t p s", p=128)
        wpk_t = wpk.rearrange("(t p) m -> t p m", p=128)
        for t in range(NKD):
            nc.sync.dma_start(
                out=wpk_sb[:, t, 0:384], in_=wpk_t[t][:, 0:384].bitcast(F32R)
            )
            nc.sync.dma_start(
                out=xT_sb[:, t, 0:QT], in_=xT_t[t][:, 0:QT].bitcast(F32R)
            )
        nc.sync.dma_start(out=bqk_sb[:], in_=bqk[:, :])
        for t in range(NKD):
            nc.sync.dma_start(
                out=wpk_sb[:, t, 384:WPK], in_=wpk_t[t][:, 384:WPK].bitcast(F32R)
            )
        nc.sync.dma_start(out=ones_stage[:], in_=vones[:, :])
        nc.sync.dma_start(out=ones64[:], in_=vones[0:1, 0:64].bitcast(F32R))
        nc.vector.tensor_copy(
            out=v_aug[:, :, :, DH],
            in_=ones_stage[:, 0 : HPC * NKT]
            .rearrange("p (h t) -> p h t", h=HPC)
            .bitcast(F32R),
        )
        for t in range(NKD):
            nc.sync.dma_start(
                out=xT_sb[:, t, QT : 2 * QT], in_=xT_t[t][:, QT : 2 * QT].bitcast(F32R)
            )
        nc.sync.dma_start(out=wo_a[:], in_=woT[0:128, :].bitcast(F32R))
        nc.sync.dma_start(out=wo_b[:], in_=woT[128:GD, :].bitcast(F32R))
        for t in range(NKD):
            nc.sync.dma_start(
                out=xT_sb[:, t, 2 * QT : S], in_=xT_t[t][:, 2 * QT : S].bitcast(F32R)
            )

        # packed projection m-groups: (col0, rows, evict spec)
        # evict spec: list of (psum row range, dst ap fn, bias col or None)
        def ev_q(col):
            return lambda n, r0, r1: qT_sb[r0:r1, col, n * QT : (n + 1) * QT]

        def ev_k(col):
            return lambda n, r0, r1: kT_sb[r0:r1, col, n * QT : (n + 1) * QT]

        def ev_v(col):
            return lambda n, r0, r1: vvT[r0:r1, col, n * QT : (n + 1) * QT]

        mgroups = [
            (0, 128, [((0, 128), ev_q(0), 0)]),
            (128, 128, [((0, 64), ev_q(1), 1), ((64, 128), ev_v(0), None)]),
            (256, 128, [((0, 128), ev_k(0), 2)]),
            (384, 128, [((0, 64), ev_k(1), 3), ((64, 128), ev_v(1), None)]),
            (512, 64, [((0, 64), ev_v(0), None)]),
        ]
        # v pieces: v_h0 -> vvT[64:128, 0], v_h1 -> vvT[64:128, 1],
        # v_h2 -> vvT[0:64, 0] (from the m4 group, psum rows 0:64)

        proj_psums = {}

        def proj_unit(mi, n, kpair):
            """Two K-step matmuls of group (mi, n); evictions after the last."""
            c0, msz, evicts = mgroups[mi]
            key = (mi, n)
            if key not in proj_psums:
                proj_psums[key] = ps.tile([128, QT], F32, tag="ps", name="projp")
            p = proj_psums[key]
            for k in (2 * kpair, 2 * kpair + 1):
                nc.tensor.matmul(
                    p[:msz, :],
                    lhsT=wpk_sb[:, k, c0 : c0 + msz],
                    rhs=xT_sb[:, k, n * QT : (n + 1) * QT],
                    start=(k == 0),
                    stop=(k == NKD - 1),
                )
            if kpair == 2:
                del proj_psums[key]
                for (r0, r1), dst, bcol in evicts:
                    if mi == 4:
                        dst_ap = dst(n, 0, 64)  # v_h2 rows live at psum 0:64
                    else:
                        dst_ap = dst(n, r0, r1)
                    if bcol is None:
                        nc.vector.tensor_copy(out=dst_ap, in_=p[r0:r1, :])
                    else:
                        nc.vector.tensor_scalar_add(
                            out=dst_ap,
                            in0=p[r0:r1, :],
                            scalar1=bqk_sb[r0:r1, bcol : bcol + 1],
                        )

        def transpose_unit(t, piece):
            """piece 0/1/2 = head 0/1/2; v_h0/v_h1 at vvT[64:128,0/1], v_h2 at vvT[0:64,0]."""
            if piece == 2:
                src = vvT[0:64, 0, t * 128 : (t + 1) * 128]
                idn = ident[0:64, 0:64]
            else:
                src = vvT[64:128, piece, t * 128 : (t + 1) * 128]
                idn = ident[64:128, 64:128]
            pt = ps.tile([128, QT], F32, tag="ps")
            nc.tensor.transpose(pt[:, 0:64], src, idn)
            nc.vector.tensor_copy(v_aug[:, piece, t, 0:64], pt[:, 0:64])

        out_pair = out_p.rearrange("(tp a p) d -> tp p a d", a=2, p=128)
        o_pairs = {}

        def o_proj_unit(t, n2):
            key = t // 2
            if key not in o_pairs:
                o_pairs[key] = expp.tile([128, 2, D], F32, tag="osb", name="osb", bufs=2)
            ob = o_pairs[key]
            po = ps.tile([128, QT], F32, tag="ps")
            nc.tensor.matmul(
                po[:, 0:384],
                lhsT=zT01[:, t * 128 : (t + 1) * 128],
                rhs=wo_a[:, n2 * 384 : (n2 + 1) * 384],
                start=True,
                stop=False,
            )
            nc.tensor.matmul(
                po[:, 0:384],
                lhsT=zT2[:, t * 128 : (t + 1) * 128],
                rhs=wo_b[:, n2 * 384 : (n2 + 1) * 384],
                start=False,
                stop=True,
            )
            if t >= 12 and (t + n2) % 2 == 0:
                nc.scalar.activation(
                    out=ob[:, t % 2, n2 * 384 : (n2 + 1) * 384],
                    in_=po[:, 0:384],
                    func=AF.Copy,
                )
            else:
                nc.vector.tensor_copy(
                    out=ob[:, t % 2, n2 * 384 : (n2 + 1) * 384], in_=po[:, 0:384]
                )
            if t % 2 == 1 and n2 == 1:
                del o_pairs[key]
                nc.sync.dma_start(out=out_pair[key], in_=ob[:, :, :])

        # background work queue of (key, fn), drained between attention
        # iterations. Queue order is topological (a group's transposes come
        # after its evictions), so force-draining "through the last needed
        # unit" preserves all producer->consumer program ordering.
        work = deque()

        def q_proj(n, mis=range(5)):
            for mi in mis:
                for kpair in range(3):
                    work.append(
                        (("proj", n, mi), lambda mi=mi, n=n, kp=kpair: proj_unit(mi, n, kp))
                    )

        def q_tr(ts, pieces=range(HPC)):
            for t in ts:
                for piece in pieces:
                    work.append(
                        (("tr", t, piece), lambda t=t, p=piece: transpose_unit(t, p))
                    )

        def drain(k=1):
            for _ in range(k):
                if work:
                    work.popleft()[1]()

        def drain_all():
            while work:
                work.popleft()[1]()

        PROJ_GROUPS_FOR_HEAD = {0: (0, 1, 2), 1: (0, 2, 3), 2: (1, 3, 4)}

        def force_drain_for(h, qt):
            """Emit queued units up to the last one attention(h, qt) depends on."""
            needed = set()
            for n in range(qt + 1):
                for mi in PROJ_GROUPS_FOR_HEAD[h]:
                    needed.add(("proj", n, mi))
            for t in range(4 * qt + 4):
                needed.add(("tr", t, h))
            last = -1
            for i, (key, _) in enumerate(work):
                if key in needed:
                    last = i
            for _ in range(last + 1):
                work.popleft()[1]()

        def qh(h):
            m, off = divmod(h * 64, 128)
            return qT_sb[off : off + 64, m, :]

        def kh(h):
            m, off = divmod(h * 64, 128)
            return kT_sb[off : off + 64, m, :]

        zdst = [zT01[0:64, :], zT01[64:128, :], zT2[0:64, :]]

        # PV matmuls are pipelined ~4 iterations behind their exp across
        # block boundaries, so the in-order PE FIFO never waits on the
        # exp/mask chain, not even at the end of a block.
        pvq = deque()  # (block_serial, pv_closure)
        blk_serial = [0]

        def pv_drain(depth):
            while len(pvq) > depth:
                pvq.popleft()[1]()

        def pv_flush(upto_serial):
            while pvq and pvq[0][0] <= upto_serial:
                pvq.popleft()[1]()

        def attention(h, qt, per_kt):
            """scores^T -> exp -> causal mask -> PV into zp; diagonal blocks
            narrowed to q columns >= 128*r."""
            zp = psz.tile([DH + 1, QT], F32)
            nkt = 4 * qt + 4
            blk = blk_serial[0]
            blk_serial[0] += 1

            def pv(kt, es, lo):
                nc.tensor.matmul(
                    zp[:, lo:QT],
                    lhsT=v_aug[:, h, kt, :],
                    rhs=es[:, lo:QT],
                    start=(kt == 0),
                    stop=(kt == nkt - 1),
                )

            for kt in range(nkt):
                rr = kt - 4 * qt
                lo = 128 * rr if rr > 0 else 0
                sp = ps.tile([128, QT], F32, tag="ps")
                nc.tensor.matmul(
                    sp[:, lo:QT],
                    lhsT=kh(h)[:, kt * 128 : (kt + 1) * 128],
                    rhs=qh(h)[:, qt * QT + lo : (qt + 1) * QT],
                    start=True,
                    stop=True,
                )
                es = expp.tile([128, QT], F32R, tag="expp")
                nc.scalar.activation(out=es[:, lo:QT], in_=sp[:, lo:QT], func=AF.Exp)
                if rr >= 0:  # diagonal block: zero where key > query
                    nc.gpsimd.affine_select(
                        out=es[:, lo:QT],
                        in_=es[:, lo:QT],
                        compare_op=mybir.AluOpType.is_ge,
                        fill=0.0,
                        base=0,
                        channel_multiplier=-1,
                        pattern=[[1, QT - lo]],
                    )
                pvq.append((blk, lambda kt=kt, es=es, lo=lo: pv(kt, es, lo)))
                if per_kt == 2:
                    drain(2)
                elif per_kt == 9:
                    drain(1)
                elif kt % 2 == 0:
                    drain(1)
                pv_drain(7)
            return zp, blk

        def normalize(zp, h, qt, cols=slice(0, QT)):
            rec = small.tile([1, QT], F32R, tag="rec")
            with nc.allow_low_precision(reason="f32r is fp32-precision"):
                nc.vector.reciprocal(rec[:, cols], zp[DH : DH + 1, cols])
            bc = ps.tile([128, QT], F32, tag="ps")
            nc.tensor.matmul(
                bc[0:64, cols], lhsT=ones64[:], rhs=rec[:, cols], start=True, stop=True
            )
            bc_sb = small.tile([64, QT], F32, tag="bcsb")
            if qt == NQT - 1:
                nc.vector.tensor_copy(out=bc_sb[:, cols], in_=bc[0:64, cols])
            else:
                nc.scalar.activation(out=bc_sb[:, cols], in_=bc[0:64, cols], func=AF.Copy)
            nc.vector.tensor_mul(
                zdst[h][:, qt * QT : (qt + 1) * QT][:, cols],
                zp[0:DH, cols],
                bc_sb[:, cols],
            )

        # ---- schedule ----
        # prologue: only what attention(h0, qt0) needs; the rest queues up.
        for mi in (0, 2, 1):
            for kpair in range(3):
                proj_unit(mi, 0, kpair)
        for t in range(4):
            transpose_unit(t, 0)
        q_proj(0, mis=(3,))
        q_tr(range(4), pieces=(1,))
        q_proj(0, mis=(4,))
        q_tr(range(4), pieces=(2,))
        for n in range(1, NQT):
            q_proj(n, mis=(0, 2, 1))
            q_tr(range(4 * n, 4 * n + 4), pieces=(0,))
            q_proj(n, mis=(3,))
            q_tr(range(4 * n, 4 * n + 4), pieces=(1,))
            q_proj(n, mis=(4,))
            q_tr(range(4 * n, 4 * n + 4), pieces=(2,))

        pending = None
        for qt in range(NQT):
            per_kt = [2, 1, 1, 9][qt]
            for h in range(HPC):
                force_drain_for(h, qt)
                zp, blk = attention(h, qt, per_kt)
                if pending is not None:
                    pv_flush(pending[3])  # pending block's PV accumulation done
                    normalize(*pending[:3])
                    ph, pqt = pending[1], pending[2]
                    if ph == HPC - 1:  # whole q-tile normalized -> O-proj ready
                        for t in range(4 * pqt, 4 * pqt + 4):
                            for n2 in range(2):
                                work.append(
                                    (("o", pqt), lambda t=t, n2=n2: o_proj_unit(t, n2))
                                )
                pending = (zp, h, qt, blk)
        # final block: normalize in column halves so the last O-proj pairs
        # start while the second half's recip/broadcast chain is still running
        pv_flush(pending[3])
        drain_all()
        normalize(*pending[:3], cols=slice(0, QT // 2))
        for t in (12, 13):
            for n2 in range(2):
                o_proj_unit(t, n2)
        normalize(*pending[:3], cols=slice(QT // 2, QT))
        for t in (14, 15):
            for n2 in range(2):
                o_proj_unit(t, n2)
    nc.finalize()
    return nc


_NC_CACHE = {}


def make_in_maps(x, W_qkv, b_qkv, W_o):
    in_maps = []
    for c in range(8):
        b, g = divmod(c, 4)
        hs = [HPC * g + i for i in range(HPC)]
        qr = [np.arange(64 * h, 64 * h + 64) for h in hs]
        w_q = [W_qkv[i] * 0.125 for i in qr]
        w_k = [W_qkv[768 + i] for i in qr]
        w_v = [W_qkv[1536 + i] for i in qr]
        b_q = [b_qkv[i] * 0.125 for i in qr]
        b_k = [b_qkv[768 + i] for i in qr]
        # packed rows: m0=[q0 q1] m1=[q2 v0] m2=[k0 k1] m3=[k2 v1] m4=[v2]
        wpk = np.concatenate(
            [w_q[0], w_q[1], w_q[2], w_v[0], w_k[0], w_k[1], w_k[2], w_v[1], w_v[2]],
            axis=0,
        )
        bqk_col = np.zeros((128, 4), np.float32)
        bqk_col[:, 0] = np.concatenate([b_q[0], b_q[1]])
        bqk_col[0:64, 1] = b_q[2]
        bqk_col[:, 2] = np.concatenate([b_k[0], b_k[1]])
        bqk_col[0:64, 3] = b_k[2]
        in_maps.append(
            {
                "xT": np.ascontiguousarray(x[b].T),
                "wpk": np.ascontiguousarray(wpk.T),
                "woT": np.ascontiguousarray(W_o[:, GD * g : GD * (g + 1)].T),
                "bqk": bqk_col,
                "vones": np.ones((128, 64), np.float32),
            }
        )
    return in_maps


def make_in_maps_for_test(inputs):
    return make_in_maps(
        np.asarray(inputs["x"], np.float32),
        np.asarray(inputs["W_qkv"], np.float32),
        np.asarray(inputs["b_qkv"], np.float32),
        np.asarray(inputs["W_o"], np.float32),
    )


def kernel(x, W_qkv, b_qkv, W_o, b_o):
    x = np.asarray(x, np.float32)
    W_qkv = np.asarray(W_qkv, np.float32)
    b_qkv = np.asarray(b_qkv, np.float32)
    W_o = np.asarray(W_o, np.float32)
    b_o = np.asarray(b_o, np.float32)

    if "nc" not in _NC_CACHE:
        _NC_CACHE["nc"] = build_bass()
    nc = _NC_CACHE["nc"]

    in_maps = make_in_maps(x, W_qkv, b_qkv, W_o)

    res = run_bass_kernel_spmd(
        nc,
        in_maps,
        list(range(8)),
        trace=bool(int(os.environ.get("KERNEL_TRACE", "0"))),
    )
    _NC_CACHE["last_results"] = res

    out = np.zeros((B, S, D), np.float32)
    for c in range(8):
        out[c // 4] += res.results[c]["out_p"]
    out += b_qkv[1536:] @ W_o.T + b_o
    return out



# revision 2
# speedup vs baseline: 1.0078x; 1.0078x over previous
"""Causal multi-head attention block (B=2, S=2048, D=768, H=12) on 8 trn2 cores.

Sharding: core c -> batch b = c//4 (data parallel), head group g = c%4
(tensor parallel, 3 heads per group). Each core computes its group's QKV
projection, causal attention, and a partial O-projection over its 192
z-columns. Host sums the 4 partials per batch and adds the biases that
commute through the math (v-bias and b_o).

On-core layout (everything "transposed", d on partitions, seq on free):
  xT   [768, 2048]   q/kT  [64*, 2048]      scores^T [keys, q]
so the softmax denominator comes free from a ones-column appended to V in
the PV matmul, and no on-chip transposes of activations are needed except
V (built via PE transpose from V^T).

The QKV projection uses a host-repacked weight matrix so every 128-wide
M-group is fully used:
  m0=[q_h0 q_h1] m1=[q_h2 v_h0] m2=[k_h0 k_h1] m3=[k_h2 v_h1] m4=[v_h2]
(q rows pre-scaled by 1/8; v bias folded into the host-side epilogue).

Matmul operands are float32r (full-rate fp32 on the PE). Scheduling
interleaves projection/transpose/O-proj work into the attention loop so
the scalar engine (exp) is never starved by a long PE FIFO stretch.
"""

import os
from collections import deque
from contextlib import ExitStack

import numpy as np

import concourse.tile as tile
from concourse import bacc, mybir
from concourse.bass_utils import run_bass_kernel_spmd
from concourse.masks import make_identity

F32 = mybir.dt.float32
F32R = mybir.dt.float32r
F16 = mybir.dt.float16
AF = mybir.ActivationFunctionType

B, S, D = 2, 2048, 768
NH, DH = 12, 64
HPC = 3            # heads per core
GD = HPC * DH      # 192 z-cols per core
KT, QT = 128, 512  # key tile (partitions), q tile (psum free)
NKT, NQT = S // KT, S // QT   # 16, 4
NTOK = S // 128    # 16 token tiles
NKD = D // 128     # 6 contraction tiles for the projections
WPK = 2 * GD + GD  # 576 packed projection rows


def build_bass():
    nc = bacc.Bacc(None)
    xT = nc.dram_tensor("xT", [D, S], F16, kind="ExternalInput")
    wpk = nc.dram_tensor("wpk", [D, WPK], F16, kind="ExternalInput")
    woT = nc.dram_tensor("woT", [GD, D], F16, kind="ExternalInput")
    bqk = nc.dram_tensor("bqk", [128, 4], F32, kind="ExternalInput")
    vones = nc.dram_tensor("vones", [128, 64], F16, kind="ExternalInput")
    out_p = nc.dram_tensor("out_p", [S, D], F16, kind="ExternalOutput")

    with tile.TileContext(nc) as tc, ExitStack() as ctx:
        const = ctx.enter_context(tc.tile_pool(name="const", bufs=1))
        ps = ctx.enter_context(tc.tile_pool(name="ps", bufs=6, space="PSUM"))
        psz = ctx.enter_context(tc.tile_pool(name="psz", bufs=2, space="PSUM"))
        expp = ctx.enter_context(tc.tile_pool(name="expp", bufs=9))
        small = ctx.enter_context(tc.tile_pool(name="small", bufs=4))

        xT_sb = const.tile([128, NKD, S], F16)
        wpk_sb = const.tile([128, NKD, WPK], F16)
        wo_a = const.tile([128, D], F16)
        wo_b = const.tile([64, D], F16)
        bqk_sb = const.tile([128, 4], F32)
        qT_sb = const.tile([128, 2, S], F16)
        kT_sb = const.tile([128, 2, S], F16)
        vvT = const.tile([128, 2, S], F16)
        v_aug = const.tile([128, HPC, NKT, DH + 1], F16)
        zT01 = const.tile([128, S], F16)
        zT2 = const.tile([64, S], F16)
        ident = const.tile([128, 128], F16)
        ones64 = const.tile([1, 64], F16)

        ones_stage = const.tile([128, 64], F16)
        make_identity(nc, ident[:])

        # ---- loads: k-interleaved so the first projection k-pairs unblock
        # early; everything not needed for (h0, qt0) comes after.
        xT_t = xT.rearrange("(t p) s -> t p s", p=128)
        wpk_t = wpk.rearrange("(t p) m -> t p m", p=128)
        for t in range(NKD):
            nc.sync.dma_start(
                out=wpk_sb[:, t, 0:384], in_=wpk_t[t][:, 0:384]
            )
            nc.sync.dma_start(
                out=xT_sb[:, t, 0:QT], in_=xT_t[t][:, 0:QT]
            )
        nc.sync.dma_start(out=bqk_sb[:], in_=bqk[:, :])
        for t in range(NKD):
            nc.sync.dma_start(
                out=wpk_sb[:, t, 384:WPK], in_=wpk_t[t][:, 384:WPK]
            )
        nc.sync.dma_start(out=ones_stage[:], in_=vones[:, :])
        nc.sync.dma_start(out=ones64[:], in_=vones[0:1, 0:64])
        nc.vector.tensor_copy(
            out=v_aug[:, :, :, DH],
            in_=ones_stage[:, 0 : HPC * NKT]
            .rearrange("p (h t) -> p h t", h=HPC),
        )
        for t in range(NKD):
            nc.sync.dma_start(
                out=xT_sb[:, t, QT : 2 * QT], in_=xT_t[t][:, QT : 2 * QT]
            )
        nc.sync.dma_start(out=wo_a[:], in_=woT[0:128, :])
        nc.sync.dma_start(out=wo_b[:], in_=woT[128:GD, :])
        for t in range(NKD):
            nc.sync.dma_start(
                out=xT_sb[:, t, 2 * QT : S], in_=xT_t[t][:, 2 * QT : S]
            )

        # packed projection m-groups: (col0, rows, evict spec)
        # evict spec: list of (psum row range, dst ap fn, bias col or None)
        def ev_q(col):
            return lambda n, r0, r1: qT_sb[r0:r1, col, n * QT : (n + 1) * QT]

        def ev_k(col):
            return lambda n, r0, r1: kT_sb[r0:r1, col, n * QT : (n + 1) * QT]

        def ev_v(col):
            return lambda n, r0, r1: vvT[r0:r1, col, n * QT : (n + 1) * QT]

        mgroups = [
            (0, 128, [((0, 128), ev_q(0), 0)]),
            (128, 128, [((0, 64), ev_q(1), 1), ((64, 128), ev_v(0), None)]),
            (256, 128, [((0, 128), ev_k(0), 2)]),
            (384, 128, [((0, 64), ev_k(1), 3), ((64, 128), ev_v(1), None)]),
            (512, 64, [((0, 64), ev_v(0), None)]),
        ]
        # v pieces: v_h0 -> vvT[64:128, 0], v_h1 -> vvT[64:128, 1],
        # v_h2 -> vvT[0:64, 0] (from the m4 group, psum rows 0:64)

        proj_psums = {}

        def proj_unit(mi, n, kpair):
            """Two K-step matmuls of group (mi, n); evictions after the last."""
            c0, msz, evicts = mgroups[mi]
            key = (mi, n)
            if key not in proj_psums:
                proj_psums[key] = ps.tile([128, QT], F32, tag="ps", name="projp")
            p = proj_psums[key]
            for k in (2 * kpair, 2 * kpair + 1):
                nc.tensor.matmul(
                    p[:msz, :],
                    lhsT=wpk_sb[:, k, c0 : c0 + msz],
                    rhs=xT_sb[:, k, n * QT : (n + 1) * QT],
                    start=(k == 0),
                    stop=(k == NKD - 1),
                )
            if kpair == 2:
                del proj_psums[key]
                for (r0, r1), dst, bcol in evicts:
                    if mi == 4:
                        dst_ap = dst(n, 0, 64)  # v_h2 rows live at psum 0:64
                    else:
                        dst_ap = dst(n, r0, r1)
                    if bcol is None:
                        nc.vector.tensor_copy(out=dst_ap, in_=p[r0:r1, :])
                    else:
                        nc.vector.tensor_scalar_add(
                            out=dst_ap,
                            in0=p[r0:r1, :],
                            scalar1=bqk_sb[r0:r1, bcol : bcol + 1],
                        )

        def transpose_unit(t, piece):
            """piece 0/1/2 = head 0/1/2; v_h0/v_h1 at vvT[64:128,0/1], v_h2 at vvT[0:64,0]."""
            if piece == 2:
                src = vvT[0:64, 0, t * 128 : (t + 1) * 128]
                idn = ident[0:64, 0:64]
            else:
                src = vvT[64:128, piece, t * 128 : (t + 1) * 128]
                idn = ident[64:128, 64:128]
            pt = ps.tile([128, QT], F16, tag="ps")
            nc.tensor.transpose(pt[:, 0:64], src, idn)
            nc.vector.tensor_copy(v_aug[:, piece, t, 0:64], pt[:, 0:64])

        out_pair = out_p.rearrange("(tp a p) d -> tp p a d", a=2, p=128)
        o_pairs = {}

        def o_proj_unit(t, n2):
            key = t // 2
            if key not in o_pairs:
                o_pairs[key] = expp.tile([128, 2, D], F16, tag="osb", name="osb", bufs=2)
            ob = o_pairs[key]
            po = ps.tile([128, QT], F32, tag="ps")
            nc.tensor.matmul(
                po[:, 0:384],
                lhsT=zT01[:, t * 128 : (t + 1) * 128],
                rhs=wo_a[:, n2 * 384 : (n2 + 1) * 384],
                start=True,
                stop=False,
            )
            nc.tensor.matmul(
                po[:, 0:384],
                lhsT=zT2[:, t * 128 : (t + 1) * 128],
                rhs=wo_b[:, n2 * 384 : (n2 + 1) * 384],
                start=False,
                stop=True,
            )
            if t >= 12 and (t + n2) % 2 == 0:
                nc.scalar.activation(
                    out=ob[:, t % 2, n2 * 384 : (n2 + 1) * 384],
                    in_=po[:, 0:384],
                    func=AF.Copy,
                )
            else:
                nc.vector.tensor_copy(
                    out=ob[:, t % 2, n2 * 384 : (n2 + 1) * 384], in_=po[:, 0:384]
                )
            if t % 2 == 1 and n2 == 1:
                del o_pairs[key]
                nc.sync.dma_start(out=out_pair[key], in_=ob[:, :, :])

        # background work queue of (key, fn), drained between attention
        # iterations. Queue order is topological (a group's transposes come
        # after its evictions), so force-draining "through the last needed
        # unit" preserves all producer->consumer program ordering.
        work = deque()

        def q_proj(n, mis=range(5)):
            for mi in mis:
                for kpair in range(3):
                    work.append(
                        (("proj", n, mi), lambda mi=mi, n=n, kp=kpair: proj_unit(mi, n, kp))
                    )

        def q_tr(ts, pieces=range(HPC)):
            for t in ts:
                for piece in pieces:
                    work.append(
                        (("tr", t, piece), lambda t=t, p=piece: transpose_unit(t, p))
                    )

        def drain(k=1):
            for _ in range(k):
                if work:
                    work.popleft()[1]()

        def drain_all():
            while work:
                work.popleft()[1]()

        PROJ_GROUPS_FOR_HEAD = {0: (0, 1, 2), 1: (0, 2, 3), 2: (1, 3, 4)}

        def force_drain_for(h, qt):
            """Emit queued units up to the last one attention(h, qt) depends on."""
            needed = set()
            for n in range(qt + 1):
                for mi in PROJ_GROUPS_FOR_HEAD[h]:
                    needed.add(("proj", n, mi))
            for t in range(4 * qt + 4):
                needed.add(("tr", t, h))
            last = -1
            for i, (key, _) in enumerate(work):
                if key in needed:
                    last = i
            for _ in range(last + 1):
                work.popleft()[1]()

        def qh(h):
            m, off = divmod(h * 64, 128)
            return qT_sb[off : off + 64, m, :]

        def kh(h):
            m, off = divmod(h * 64, 128)
            return kT_sb[off : off + 64, m, :]

        zdst = [zT01[0:64, :], zT01[64:128, :], zT2[0:64, :]]

        # PV matmuls are pipelined ~4 iterations behind their exp across
        # block boundaries, so the in-order PE FIFO never waits on the
        # exp/mask chain, not even at the end of a block.
        pvq = deque()  # (block_serial, pv_closure)
        blk_serial = [0]

        def pv_drain(depth):
            while len(pvq) > depth:
                pvq.popleft()[1]()

        def pv_flush(upto_serial):
            while pvq and pvq[0][0] <= upto_serial:
                pvq.popleft()[1]()

        def attention(h, qt, per_kt):
            """scores^T -> exp -> causal mask -> PV into zp; diagonal blocks
            narrowed to q columns >= 128*r."""
            zp = psz.tile([DH + 1, QT], F32)
            nkt = 4 * qt + 4
            blk = blk_serial[0]
            blk_serial[0] += 1

            def pv(kt, es, lo):
                nc.tensor.matmul(
                    zp[:, lo:QT],
                    lhsT=v_aug[:, h, kt, :],
                    rhs=es[:, lo:QT],
                    start=(kt == 0),
                    stop=(kt == nkt - 1),
                )

            for kt in range(nkt):
                rr = kt - 4 * qt
                lo = 128 * rr if rr > 0 else 0
                sp = ps.tile([128, QT], F32, tag="ps")
                nc.tensor.matmul(
                    sp[:, lo:QT],
                    lhsT=kh(h)[:, kt * 128 : (kt + 1) * 128],
                    rhs=qh(h)[:, qt * QT + lo : (qt + 1) * QT],
                    start=True,
                    stop=True,
                )
                es = expp.tile([128, QT], F16, tag="expp")
                nc.scalar.activation(out=es[:, lo:QT], in_=sp[:, lo:QT], func=AF.Exp)
                if rr >= 0:  # diagonal block: zero where key > query
                    nc.gpsimd.affine_select(
                        out=es[:, lo:QT],
                        in_=es[:, lo:QT],
                        compare_op=mybir.AluOpType.is_ge,
                        fill=0.0,
                        base=0,
                        channel_multiplier=-1,
                        pattern=[[1, QT - lo]],
                    )
                pvq.append((blk, lambda kt=kt, es=es, lo=lo: pv(kt, es, lo)))
                if per_kt == 2:
                    drain(2)
                elif per_kt == 9:
                    drain(1)
                elif kt % 2 == 0:
                    drain(1)
                pv_drain(7)
            return zp, blk

        def normalize(zp, h, qt, cols=slice(0, QT)):
            rec = small.tile([1, QT], F16, tag="rec")
            with nc.allow_low_precision(reason="f32r is fp32-precision"):
                nc.vector.reciprocal(rec[:, cols], zp[DH : DH + 1, cols])
            bc = ps.tile([128, QT], F32, tag="ps")
            nc.tensor.matmul(
                bc[0:64, cols], lhsT=ones64[:], rhs=rec[:, cols], start=True, stop=True
            )
            bc_sb = small.tile([64, QT], F16, tag="bcsb")
            if qt == NQT - 1:
                nc.vector.tensor_copy(out=bc_sb[:, cols], in_=bc[0:64, cols])
            else:
                nc.scalar.activation(out=bc_sb[:, cols], in_=bc[0:64, cols], func=AF.Copy)
            nc.vector.tensor_mul(
                zdst[h][:, qt * QT : (qt + 1) * QT][:, cols],
                zp[0:DH, cols],
                bc_sb[:, cols],
            )

        # ---- schedule ----
        # prologue: only what attention(h0, qt0) needs; the rest queues up.
        for mi in (0, 2, 1):
            for kpair in range(3):
                proj_unit(mi, 0, kpair)
        for t in range(4):
            transpose_unit(t, 0)
        q_proj(0, mis=(3,))
        q_tr(range(4), pieces=(1,))
        q_proj(0, mis=(4,))
        q_tr(range(4), pieces=(2,))
        for n in range(1, NQT):
            q_proj(n, mis=(0, 2, 1))
            q_tr(range(4 * n, 4 * n + 4), pieces=(0,))
            q_proj(n, mis=(3,))
            q_tr(range(4 * n, 4 * n + 4), pieces=(1,))
            q_proj(n, mis=(4,))
            q_tr(range(4 * n, 4 * n + 4), pieces=(2,))

        pending = None
        for qt in range(NQT):
            per_kt = [2, 1, 1, 9][qt]
            for h in range(HPC):
                force_drain_for(h, qt)
                zp, blk = attention(h, qt, per_kt)
                if pending is not None:
                    pv_flush(pending[3])  # pending block's PV accumulation done
                    normalize(*pending[:3])
                    ph, pqt = pending[1], pending[2]
                    if ph == HPC - 1:  # whole q-tile normalized -> O-proj ready
                        for t in range(4 * pqt, 4 * pqt + 4):
                            for n2 in range(2):
                                work.append(
                                    (("o", pqt), lambda t=t, n2=n2: o_proj_unit(t, n2))
                                )
                pending = (zp, h, qt, blk)
        # final block: normalize in column halves so the last O-proj pairs
        # start while the second half's recip/broadcast chain is still running
        pv_flush(pending[3])
        drain_all()
        normalize(*pending[:3], cols=slice(0, QT // 2))
        for t in (12, 13):
            for n2 in range(2):
                o_proj_unit(t, n2)
        normalize(*pending[:3], cols=slice(QT // 2, QT))
        for t in (14, 15):
            for n2 in range(2):
                o_proj_unit(t, n2)
    nc.finalize()
    return nc


_NC_CACHE = {}


def make_in_maps(x, W_qkv, b_qkv, W_o):
    in_maps = []
    for c in range(8):
        b, g = divmod(c, 4)
        hs = [HPC * g + i for i in range(HPC)]
        qr = [np.arange(64 * h, 64 * h + 64) for h in hs]
        w_q = [W_qkv[i] * 0.125 for i in qr]
        w_k = [W_qkv[768 + i] for i in qr]
        w_v = [W_qkv[1536 + i] for i in qr]
        b_q = [b_qkv[i] * 0.125 for i in qr]
        b_k = [b_qkv[768 + i] for i in qr]
        # packed rows: m0=[q0 q1] m1=[q2 v0] m2=[k0 k1] m3=[k2 v1] m4=[v2]
        wpk = np.concatenate(
            [w_q[0], w_q[1], w_q[2], w_v[0], w_k[0], w_k[1], w_k[2], w_v[1], w_v[2]],
            axis=0,
        )
        bqk_col = np.zeros((128, 4), np.float32)
        bqk_col[:, 0] = np.concatenate([b_q[0], b_q[1]])
        bqk_col[0:64, 1] = b_q[2]
        bqk_col[:, 2] = np.concatenate([b_k[0], b_k[1]])
        bqk_col[0:64, 3] = b_k[2]
        in_maps.append(
            {
                "xT": np.ascontiguousarray(x[b].T.astype(np.float16)),
                "wpk": np.ascontiguousarray(wpk.T.astype(np.float16)),
                "woT": np.ascontiguousarray(W_o[:, GD * g : GD * (g + 1)].T.astype(np.float16)),
                "bqk": bqk_col,
                "vones": np.ones((128, 64), np.float16),
            }
        )
    return in_maps


def make_in_maps_for_test(inputs):
    return make_in_maps(
        np.asarray(inputs["x"], np.float32),
        np.asarray(inputs["W_qkv"], np.float32),
        np.asarray(inputs["b_qkv"], np.float32),
        np.asarray(inputs["W_o"], np.float32),
    )


def kernel(x, W_qkv, b_qkv, W_o, b_o):
    x = np.asarray(x, np.float32)
    W_qkv = np.asarray(W_qkv, np.float32)
    b_qkv = np.asarray(b_qkv, np.float32)
    W_o = np.asarray(W_o, np.float32)
    b_o = np.asarray(b_o, np.float32)

    if "nc" not in _NC_CACHE:
        _NC_CACHE["nc"] = build_bass()
    nc = _NC_CACHE["nc"]

    in_maps = make_in_maps(x, W_qkv, b_qkv, W_o)

    res = run_bass_kernel_spmd(
        nc,
        in_maps,
        list(range(8)),
        trace=bool(int(os.environ.get("KERNEL_TRACE", "0"))),
    )
    _NC_CACHE["last_results"] = res

    out = np.zeros((B, S, D), np.float32)
    for c in range(8):
        out[c // 4] += res.results[c]["out_p"].astype(np.float32)
    out += b_qkv[1536:] @ W_o.T + b_o
    return out



# revision 3
# speedup vs baseline: 1.0333x; 1.0252x over previous
"""Causal multi-head attention block (B=2, S=2048, D=768, H=12) on 8 trn2 cores.

Sharding: core c -> batch b = c//4 (data parallel), head group g = c%4
(tensor parallel, 3 heads per group). Each core computes its group's QKV
projection, causal attention, and a partial O-projection over its 192
z-columns. Host sums the 4 partials per batch and adds the biases that
commute through the math (v-bias and b_o).

On-core layout (everything "transposed", d on partitions, seq on free):
  xT   [768, 2048]   q/kT  [64*, 2048]      scores^T [keys, q]
so the softmax denominator comes free from a ones-column appended to V in
the PV matmul, and no on-chip transposes of activations are needed except
V (built via PE transpose from V^T).

The QKV projection uses a host-repacked weight matrix so every 128-wide
M-group is fully used:
  m0=[q_h0 q_h1] m1=[q_h2 v_h0] m2=[k_h0 k_h1] m3=[k_h2 v_h1] m4=[v_h2]
(q rows pre-scaled by 1/8; v bias folded into the host-side epilogue).

Matmul operands are float32r (full-rate fp32 on the PE). Scheduling
interleaves projection/transpose/O-proj work into the attention loop so
the scalar engine (exp) is never starved by a long PE FIFO stretch.
"""

import os
from collections import deque
from contextlib import ExitStack

import numpy as np

import concourse.tile as tile
from concourse import bacc, mybir
from concourse.bass_utils import run_bass_kernel_spmd
from concourse.masks import make_identity

F32 = mybir.dt.float32
F32R = mybir.dt.float32r
F16 = mybir.dt.float16
AF = mybir.ActivationFunctionType

B, S, D = 2, 2048, 768
NH, DH = 12, 64
HPC = 3            # heads per core
GD = HPC * DH      # 192 z-cols per core
KT, QT = 128, 512  # key tile (partitions), q tile (psum free)
NKT, NQT = S // KT, S // QT   # 16, 4
NTOK = S // 128    # 16 token tiles
NKD = D // 128     # 6 contraction tiles for the projections
WPK = 2 * GD + GD  # 576 packed projection rows


def build_bass():
    nc = bacc.Bacc(None)
    xT = nc.dram_tensor("xT", [D, S], F16, kind="ExternalInput")
    wpk = nc.dram_tensor("wpk", [D, WPK], F16, kind="ExternalInput")
    woT = nc.dram_tensor("woT", [GD, D], F16, kind="ExternalInput")
    bqk = nc.dram_tensor("bqk", [128, 4], F32, kind="ExternalInput")
    vones = nc.dram_tensor("vones", [128, 64], F16, kind="ExternalInput")
    out_p = nc.dram_tensor("out_p", [S, D], F16, kind="ExternalOutput")

    with tile.TileContext(nc) as tc, ExitStack() as ctx:
        const = ctx.enter_context(tc.tile_pool(name="const", bufs=1))
        ps = ctx.enter_context(tc.tile_pool(name="ps", bufs=2, space="PSUM"))
        ps2 = ctx.enter_context(tc.tile_pool(name="ps2", bufs=2, space="PSUM"))
        psz = ctx.enter_context(tc.tile_pool(name="psz", bufs=2, space="PSUM"))
        expp = ctx.enter_context(tc.tile_pool(name="expp", bufs=9))
        small = ctx.enter_context(tc.tile_pool(name="small", bufs=4))

        xT_sb = const.tile([128, NKD, S], F16)
        wpk_sb = const.tile([128, NKD, WPK], F16)
        wo_a = const.tile([128, D], F16)
        wo_b = const.tile([64, D], F16)
        bqk_sb = const.tile([128, 4], F32)
        qT_sb = const.tile([128, 2, S], F16)
        kT_sb = const.tile([128, 2, S], F16)
        vvT = const.tile([128, 2, S], F16)
        v_aug = const.tile([128, HPC, NKT, DH + 1], F16)
        zT01 = const.tile([128, S], F16)
        zT2 = const.tile([64, S], F16)
        ident = const.tile([128, 128], F16)
        ones64 = const.tile([1, 64], F16)

        ones_stage = const.tile([128, 64], F16)
        make_identity(nc, ident[:])

        # ---- loads: k-interleaved so the first projection k-pairs unblock
        # early; everything not needed for (h0, qt0) comes after.
        xT_t = xT.rearrange("(t p) s -> t p s", p=128)
        wpk_t = wpk.rearrange("(t p) m -> t p m", p=128)
        for t in range(NKD):
            nc.sync.dma_start(
                out=wpk_sb[:, t, 0:384], in_=wpk_t[t][:, 0:384]
            )
            nc.sync.dma_start(
                out=xT_sb[:, t, 0:QT], in_=xT_t[t][:, 0:QT]
            )
        nc.sync.dma_start(out=bqk_sb[:], in_=bqk[:, :])
        for t in range(NKD):
            nc.sync.dma_start(
                out=wpk_sb[:, t, 384:WPK], in_=wpk_t[t][:, 384:WPK]
            )
        nc.sync.dma_start(out=ones_stage[:], in_=vones[:, :])
        nc.sync.dma_start(out=ones64[:], in_=vones[0:1, 0:64])
        nc.vector.tensor_copy(
            out=v_aug[:, :, :, DH],
            in_=ones_stage[:, 0 : HPC * NKT]
            .rearrange("p (h t) -> p h t", h=HPC),
        )
        for t in range(NKD):
            nc.sync.dma_start(
                out=xT_sb[:, t, QT : 2 * QT], in_=xT_t[t][:, QT : 2 * QT]
            )
        nc.sync.dma_start(out=wo_a[:], in_=woT[0:128, :])
        nc.sync.dma_start(out=wo_b[:], in_=woT[128:GD, :])
        for t in range(NKD):
            nc.sync.dma_start(
                out=xT_sb[:, t, 2 * QT : S], in_=xT_t[t][:, 2 * QT : S]
            )

        # packed projection m-groups: (col0, rows, evict spec)
        # evict spec: list of (psum row range, dst ap fn, bias col or None)
        def ev_q(col):
            return lambda n, r0, r1: qT_sb[r0:r1, col, n * QT : (n + 1) * QT]

        def ev_k(col):
            return lambda n, r0, r1: kT_sb[r0:r1, col, n * QT : (n + 1) * QT]

        def ev_v(col):
            return lambda n, r0, r1: vvT[r0:r1, col, n * QT : (n + 1) * QT]

        mgroups = [
            (0, 128, [((0, 128), ev_q(0), 0)]),
            (128, 128, [((0, 64), ev_q(1), 1), ((64, 128), ev_v(0), None)]),
            (256, 128, [((0, 128), ev_k(0), 2)]),
            (384, 128, [((0, 64), ev_k(1), 3), ((64, 128), ev_v(1), None)]),
            (512, 64, [((0, 64), ev_v(0), None)]),
        ]
        # v pieces: v_h0 -> vvT[64:128, 0], v_h1 -> vvT[64:128, 1],
        # v_h2 -> vvT[0:64, 0] (from the m4 group, psum rows 0:64)

        proj_psums = {}

        def proj_unit(mi, n, kpair):
            """Two K-step matmuls of group (mi, n); evictions after the last."""
            c0, msz, evicts = mgroups[mi]
            key = (mi, n)
            if key not in proj_psums:
                proj_psums[key] = ps.tile([128, QT], F32, tag="ps", name="projp")
            p = proj_psums[key]
            for k in (2 * kpair, 2 * kpair + 1):
                nc.tensor.matmul(
                    p[:msz, :],
                    lhsT=wpk_sb[:, k, c0 : c0 + msz],
                    rhs=xT_sb[:, k, n * QT : (n + 1) * QT],
                    start=(k == 0),
                    stop=(k == NKD - 1),
                )
            if kpair == 2:
                del proj_psums[key]
                for (r0, r1), dst, bcol in evicts:
                    if mi == 4:
                        dst_ap = dst(n, 0, 64)  # v_h2 rows live at psum 0:64
                    else:
                        dst_ap = dst(n, r0, r1)
                    if bcol is None:
                        nc.vector.tensor_copy(out=dst_ap, in_=p[r0:r1, :])
                    else:
                        nc.vector.tensor_scalar_add(
                            out=dst_ap,
                            in0=p[r0:r1, :],
                            scalar1=bqk_sb[r0:r1, bcol : bcol + 1],
                        )

        def transpose_unit(t, piece):
            """piece 0/1/2 = head 0/1/2; v_h0/v_h1 at vvT[64:128,0/1], v_h2 at vvT[0:64,0]."""
            if piece == 2:
                src = vvT[0:64, 0, t * 128 : (t + 1) * 128]
                idn = ident[0:64, 0:64]
            else:
                src = vvT[64:128, piece, t * 128 : (t + 1) * 128]
                idn = ident[64:128, 64:128]
            pt = ps.tile([128, QT], F16, tag="ps")
            nc.tensor.transpose(pt[:, 0:64], src, idn)
            nc.vector.tensor_copy(v_aug[:, piece, t, 0:64], pt[:, 0:64])

        out_pair = out_p.rearrange("(tp a p) d -> tp p a d", a=2, p=128)
        o_pairs = {}

        def o_proj_unit(t, n2):
            key = t // 2
            if key not in o_pairs:
                o_pairs[key] = expp.tile([128, 2, D], F16, tag="osb", name="osb", bufs=2)
            ob = o_pairs[key]
            po = ps.tile([128, QT], F32, tag="ps")
            nc.tensor.matmul(
                po[:, 0:384],
                lhsT=zT01[:, t * 128 : (t + 1) * 128],
                rhs=wo_a[:, n2 * 384 : (n2 + 1) * 384],
                start=True,
                stop=False,
            )
            nc.tensor.matmul(
                po[:, 0:384],
                lhsT=zT2[:, t * 128 : (t + 1) * 128],
                rhs=wo_b[:, n2 * 384 : (n2 + 1) * 384],
                start=False,
                stop=True,
            )
            if t >= 12 and (t + n2) % 2 == 0:
                nc.scalar.activation(
                    out=ob[:, t % 2, n2 * 384 : (n2 + 1) * 384],
                    in_=po[:, 0:384],
                    func=AF.Copy,
                )
            else:
                nc.vector.tensor_copy(
                    out=ob[:, t % 2, n2 * 384 : (n2 + 1) * 384], in_=po[:, 0:384]
                )
            if t % 2 == 1 and n2 == 1:
                del o_pairs[key]
                nc.sync.dma_start(out=out_pair[key], in_=ob[:, :, :])

        # background work queue of (key, fn), drained between attention
        # iterations. Queue order is topological (a group's transposes come
        # after its evictions), so force-draining "through the last needed
        # unit" preserves all producer->consumer program ordering.
        work = deque()

        def q_proj(n, mis=range(5)):
            for mi in mis:
                for kpair in range(3):
                    work.append(
                        (("proj", n, mi), lambda mi=mi, n=n, kp=kpair: proj_unit(mi, n, kp))
                    )

        def q_tr(ts, pieces=range(HPC)):
            for t in ts:
                for piece in pieces:
                    work.append(
                        (("tr", t, piece), lambda t=t, p=piece: transpose_unit(t, p))
                    )

        def drain(k=1):
            for _ in range(k):
                if work:
                    work.popleft()[1]()

        def drain_all():
            while work:
                work.popleft()[1]()

        PROJ_GROUPS_FOR_HEAD = {0: (0, 1, 2), 1: (0, 2, 3), 2: (1, 3, 4)}

        def force_drain_for(h, qt):
            """Emit queued units up to the last one attention(h, qt) depends on."""
            needed = set()
            for n in range(qt + 1):
                for mi in PROJ_GROUPS_FOR_HEAD[h]:
                    needed.add(("proj", n, mi))
            for t in range(4 * qt + 4):
                needed.add(("tr", t, h))
            last = -1
            for i, (key, _) in enumerate(work):
                if key in needed:
                    last = i
            for _ in range(last + 1):
                work.popleft()[1]()

        def qh(h):
            m, off = divmod(h * 64, 128)
            return qT_sb[off : off + 64, m, :]

        def kh(h):
            m, off = divmod(h * 64, 128)
            return kT_sb[off : off + 64, m, :]

        zdst = [zT01[0:64, :], zT01[64:128, :], zT2[0:64, :]]

        # PV matmuls are pipelined ~4 iterations behind their exp across
        # block boundaries, so the in-order PE FIFO never waits on the
        # exp/mask chain, not even at the end of a block.
        pvq = deque()  # (block_serial, pv_closure)
        blk_serial = [0]

        def pv_drain(depth):
            while len(pvq) > depth:
                pvq.popleft()[1]()

        def pv_flush(upto_serial):
            while pvq and pvq[0][0] <= upto_serial:
                pvq.popleft()[1]()

        def attention(h, qt, per_pair):
            """scores^T -> exp -> causal mask -> PV into zp. Score matmuls for
            kt pairs land in one 2-bank psum tile so a single exp covers both;
            start=True zeroes the whole bank, so the unwritten low columns of
            narrowed diagonal halves exp to 1.0 and are never consumed. The
            causal mask only touches the 128-wide diagonal square."""
            zp = psz.tile([DH + 1, QT], F32)
            nkt = 4 * qt + 4
            blk = blk_serial[0]
            blk_serial[0] += 1

            def pv(kt, es_ap, lo):
                nc.tensor.matmul(
                    zp[:, lo:QT],
                    lhsT=v_aug[:, h, kt, :],
                    rhs=es_ap[:, lo:QT],
                    start=(kt == 0),
                    stop=(kt == nkt - 1),
                )

            for kp in range(nkt // 2):
                kt0 = 2 * kp
                rr0 = kt0 - 4 * qt
                lo_pair = 128 * rr0 if rr0 > 0 else 0
                sp = ps2.tile([128, 2, QT], F32, tag="ps2")
                los = []
                for i in (0, 1):
                    kt = kt0 + i
                    rr = kt - 4 * qt
                    lo = 128 * rr if rr > 0 else 0
                    los.append(lo)
                    nc.tensor.matmul(
                        sp[:, i, lo:QT],
                        lhsT=kh(h)[:, kt * 128 : (kt + 1) * 128],
                        rhs=qh(h)[:, qt * QT + lo : (qt + 1) * QT],
                        start=True,
                        stop=True,
                    )
                es = expp.tile([128, 2, QT], F16, tag="expp")
                nc.scalar.activation(
                    out=es[:, :, lo_pair:QT], in_=sp[:, :, lo_pair:QT], func=AF.Exp
                )
                for i in (0, 1):
                    kt = kt0 + i
                    rr = kt - 4 * qt
                    lo = los[i]
                    if rr >= 0:  # diagonal square: zero where key > query
                        hi = min(lo + 128, QT)
                        nc.gpsimd.affine_select(
                            out=es[:, i, lo:hi],
                            in_=es[:, i, lo:hi],
                            compare_op=mybir.AluOpType.is_ge,
                            fill=0.0,
                            base=0,
                            channel_multiplier=-1,
                            pattern=[[1, hi - lo]],
                        )
                    pvq.append(
                        (blk, lambda kt=kt, es=es, i=i, lo=lo: pv(kt, es[:, i], lo))
                    )
                drain(per_pair)
                pv_drain(7)
            return zp, blk

        def normalize(zp, h, qt, cols=slice(0, QT)):
            rec = small.tile([1, QT], F16, tag="rec")
            with nc.allow_low_precision(reason="f32r is fp32-precision"):
                nc.vector.reciprocal(rec[:, cols], zp[DH : DH + 1, cols])
            bc = ps.tile([128, QT], F32, tag="ps")
            nc.tensor.matmul(
                bc[0:64, cols], lhsT=ones64[:], rhs=rec[:, cols], start=True, stop=True
            )
            bc_sb = small.tile([64, QT], F16, tag="bcsb")
            if qt == NQT - 1:
                nc.vector.tensor_copy(out=bc_sb[:, cols], in_=bc[0:64, cols])
            else:
                nc.scalar.activation(out=bc_sb[:, cols], in_=bc[0:64, cols], func=AF.Copy)
            nc.vector.tensor_mul(
                zdst[h][:, qt * QT : (qt + 1) * QT][:, cols],
                zp[0:DH, cols],
                bc_sb[:, cols],
            )

        # ---- schedule ----
        # prologue: only what attention(h0, qt0) needs; the rest queues up.
        for mi in (0, 2, 1):
            for kpair in range(3):
                proj_unit(mi, 0, kpair)
        for t in range(4):
            transpose_unit(t, 0)
        q_proj(0, mis=(3,))
        q_tr(range(4), pieces=(1,))
        q_proj(0, mis=(4,))
        q_tr(range(4), pieces=(2,))
        for n in range(1, NQT):
            q_proj(n, mis=(0, 2, 1))
            q_tr(range(4 * n, 4 * n + 4), pieces=(0,))
            q_proj(n, mis=(3,))
            q_tr(range(4 * n, 4 * n + 4), pieces=(1,))
            q_proj(n, mis=(4,))
            q_tr(range(4 * n, 4 * n + 4), pieces=(2,))

        pending = None
        for qt in range(NQT):
            per_pair = [4, 1, 1, 2][qt]
            for h in range(HPC):
                force_drain_for(h, qt)
                zp, blk = attention(h, qt, per_pair)
                if pending is not None:
                    pv_flush(pending[3])  # pending block's PV accumulation done
                    normalize(*pending[:3])
                    ph, pqt = pending[1], pending[2]
                    if ph == HPC - 1:  # whole q-tile normalized -> O-proj ready
                        for t in range(4 * pqt, 4 * pqt + 4):
                            for n2 in range(2):
                                work.append(
                                    (("o", pqt), lambda t=t, n2=n2: o_proj_unit(t, n2))
                                )
                pending = (zp, h, qt, blk)
        # final block: normalize in column halves so the last O-proj pairs
        # start while the second half's recip/broadcast chain is still running
        pv_flush(pending[3])
        drain_all()
        normalize(*pending[:3], cols=slice(0, QT // 2))
        for t in (12, 13):
            for n2 in range(2):
                o_proj_unit(t, n2)
        normalize(*pending[:3], cols=slice(QT // 2, QT))
        for t in (14, 15):
            for n2 in range(2):
                o_proj_unit(t, n2)
    nc.finalize()
    return nc


_NC_CACHE = {}


def make_in_maps(x, W_qkv, b_qkv, W_o):
    in_maps = []
    for c in range(8):
        b, g = divmod(c, 4)
        hs = [HPC * g + i for i in range(HPC)]
        qr = [np.arange(64 * h, 64 * h + 64) for h in hs]
        w_q = [W_qkv[i] * 0.125 for i in qr]
        w_k = [W_qkv[768 + i] for i in qr]
        w_v = [W_qkv[1536 + i] for i in qr]
        b_q = [b_qkv[i] * 0.125 for i in qr]
        b_k = [b_qkv[768 + i] for i in qr]
        # packed rows: m0=[q0 q1] m1=[q2 v0] m2=[k0 k1] m3=[k2 v1] m4=[v2]
        wpk = np.concatenate(
            [w_q[0], w_q[1], w_q[2], w_v[0], w_k[0], w_k[1], w_k[2], w_v[1], w_v[2]],
            axis=0,
        )
        bqk_col = np.zeros((128, 4), np.float32)
        bqk_col[:, 0] = np.concatenate([b_q[0], b_q[1]])
        bqk_col[0:64, 1] = b_q[2]
        bqk_col[:, 2] = np.concatenate([b_k[0], b_k[1]])
        bqk_col[0:64, 3] = b_k[2]
        in_maps.append(
            {
                "xT": np.ascontiguousarray(x[b].T.astype(np.float16)),
                "wpk": np.ascontiguousarray(wpk.T.astype(np.float16)),
                "woT": np.ascontiguousarray(W_o[:, GD * g : GD * (g + 1)].T.astype(np.float16)),
                "bqk": bqk_col,
                "vones": np.ones((128, 64), np.float16),
            }
        )
    return in_maps


def make_in_maps_for_test(inputs):
    return make_in_maps(
        np.asarray(inputs["x"], np.float32),
        np.asarray(inputs["W_qkv"], np.float32),
        np.asarray(inputs["b_qkv"], np.float32),
        np.asarray(inputs["W_o"], np.float32),
    )


def kernel(x, W_qkv, b_qkv, W_o, b_o):
    x = np.asarray(x, np.float32)
    W_qkv = np.asarray(W_qkv, np.float32)
    b_qkv = np.asarray(b_qkv, np.float32)
    W_o = np.asarray(W_o, np.float32)
    b_o = np.asarray(b_o, np.float32)

    if "nc" not in _NC_CACHE:
        _NC_CACHE["nc"] = build_bass()
    nc = _NC_CACHE["nc"]

    in_maps = make_in_maps(x, W_qkv, b_qkv, W_o)

    res = run_bass_kernel_spmd(
        nc,
        in_maps,
        list(range(8)),
        trace=bool(int(os.environ.get("KERNEL_TRACE", "0"))),
    )
    _NC_CACHE["last_results"] = res

    out = np.zeros((B, S, D), np.float32)
    for c in range(8):
        out[c // 4] += res.results[c]["out_p"].astype(np.float32)
    out += b_qkv[1536:] @ W_o.T + b_o
    return out



# revision 7
# speedup vs baseline: 1.1250x; 1.0887x over previous
"""Causal multi-head attention block (B=2, S=2048, D=768, H=12) on 8 trn2 cores.

Sharding: core c -> batch b = c//4 (data parallel), head group g = c%4
(tensor parallel, 3 heads per group). Each core computes its group's QKV
projection, causal attention, and a partial O-projection over its 192
z-columns. Host sums the 4 partials per batch and adds the biases that
commute through the math (v-bias and b_o).

On-core layout: xT [768, 2048] (d on partitions) so q^T/k^T come straight
out of the projection with head dims on partitions; V is projected
separately in [keys, dh] layout (lhsT = xT key-block, rhs = W_v^T columns)
so the PV matmul needs no transposes at all. A ones-column appended to V
yields the softmax denominator for free.

All matmul operands are fp16 (1 col/cycle on the PE, 10-bit mantissa keeps
rel-err ~5e-4); psum stays f32. Score matmuls for kt pairs land in one
2-bank psum tile so a single exp instruction covers both; the causal mask
only touches the 128-wide diagonal square.

The packed q/k projection weight is host-repacked so all three 128-wide
M-groups are full:  m0=[q_h0 q_h1]  m1=[q_h2 k_h0]  m2=[k_h1 k_h2]
(q rows pre-scaled by 1/8; v bias folded into the host-side epilogue).
"""

import os
from collections import deque
from contextlib import ExitStack

import numpy as np

import concourse.tile as tile
from concourse import bacc, mybir
from concourse.bass_utils import run_bass_kernel_spmd

F32 = mybir.dt.float32
F16 = mybir.dt.float16
AF = mybir.ActivationFunctionType

B, S, D = 2, 2048, 768
NH, DH = 12, 64
HPC = 3            # heads per core
GD = HPC * DH      # 192 z-cols per core
KT, QT = 128, 512  # key tile (partitions), q tile (psum free)
NKT, NQT = S // KT, S // QT   # 16, 4
NKD = D // 128     # 6 contraction tiles for the projections
WPK = 2 * GD       # 384 packed q/k projection rows


def build_bass():
    nc = bacc.Bacc(None)
    xT = nc.dram_tensor("xT", [D, S], F16, kind="ExternalInput")
    wpk = nc.dram_tensor("wpk", [D, WPK], F16, kind="ExternalInput")
    wv = nc.dram_tensor("wv", [D, GD], F16, kind="ExternalInput")
    woT = nc.dram_tensor("woT", [GD, D], F16, kind="ExternalInput")
    bqk = nc.dram_tensor("bqk", [128, 3], F32, kind="ExternalInput")
    vones = nc.dram_tensor("vones", [128, 64], F16, kind="ExternalInput")
    out_p = nc.dram_tensor("out_p", [S, D], F16, kind="ExternalOutput")

    with tile.TileContext(nc) as tc, ExitStack() as ctx:
        const = ctx.enter_context(tc.tile_pool(name="const", bufs=1))
        ps = ctx.enter_context(tc.tile_pool(name="ps", bufs=2, space="PSUM"))
        ps2 = ctx.enter_context(tc.tile_pool(name="ps2", bufs=2, space="PSUM"))
        psz = ctx.enter_context(tc.tile_pool(name="psz", bufs=2, space="PSUM"))
        expp = ctx.enter_context(tc.tile_pool(name="expp", bufs=9))
        small = ctx.enter_context(tc.tile_pool(name="small", bufs=4))

        xT_sb = const.tile([128, NKD, S], F16)
        wpk_sb = const.tile([128, NKD, WPK], F16)
        wv_sb = const.tile([128, NKD, GD], F16)
        wo_a = const.tile([128, D], F16)
        wo_b = const.tile([64, D], F16)
        bqk_sb = const.tile([128, 3], F32)
        qT_sb = const.tile([128, 2, S], F16)
        kT_sb = const.tile([128, 2, S], F16)
        v_aug = const.tile([128, NKT, HPC, DH + 1], F16)
        zT01 = const.tile([128, S], F16)
        zT2 = const.tile([64, S], F16)
        ones64 = const.tile([1, 64], F16)
        ones_stage = const.tile([128, 64], F16)

        # ---- loads: k-interleaved so the first projection k-pairs unblock
        # early. Early DMAs fan out over four queues (SP/Act/DVE/Pool) to
        # dodge the ~650ns per-DMA dispatch serialization on a single queue.
        xT_t = xT.rearrange("(t p) s -> t p s", p=128)
        wpk_t = wpk.rearrange("(t p) m -> t p m", p=128)
        wv_t = wv.rearrange("(t p) m -> t p m", p=128)
        qdma = [nc.sync, nc.scalar, nc.gpsimd]
        for t in range(NKD):
            qdma[t % 3].dma_start(out=wpk_sb[:, t, :], in_=wpk_t[t][:, :])
            qdma[(t + 1) % 3].dma_start(out=xT_sb[:, t, 0:QT], in_=xT_t[t][:, 0:QT])
        nc.sync.dma_start(out=bqk_sb[:], in_=bqk[:, :])
        for t in range(NKD):
            qdma[t % 3].dma_start(out=wv_sb[:, t, :], in_=wv_t[t][:, :])
        nc.gpsimd.dma_start(out=ones_stage[:], in_=vones[:, :])
        nc.gpsimd.dma_start(out=ones64[:], in_=vones[0:1, 0:64])
        nc.vector.tensor_copy(
            out=v_aug[:, :, :, DH],
            in_=ones_stage[:, 0 : NKT * HPC].rearrange("p (t h) -> p t h", t=NKT),
        )
        for t in range(NKD):
            qdma[t % 3].dma_start(
                out=xT_sb[:, t, QT : 2 * QT], in_=xT_t[t][:, QT : 2 * QT]
            )
        nc.scalar.dma_start(out=wo_a[:], in_=woT[0:128, :])
        nc.gpsimd.dma_start(out=wo_b[:], in_=woT[128:GD, :])
        for t in range(NKD):
            qdma[t % 3].dma_start(
                out=xT_sb[:, t, 2 * QT : S], in_=xT_t[t][:, 2 * QT : S]
            )

        # packed q/k projection m-groups: (col0, evict spec). m2 holds
        # [q2 k2]; its k2 rows evict to kT partitions 0:64 (cross-base copy)
        # so every head's q and k share a partition base for the PE.
        mgroups = [
            (0, [((0, 128), lambda n: qT_sb[0:128, 0, n * QT : (n + 1) * QT], 0)]),
            (128, [((0, 128), lambda n: kT_sb[0:128, 0, n * QT : (n + 1) * QT], 1)]),
            (256, [
                ((0, 64), lambda n: qT_sb[0:64, 1, n * QT : (n + 1) * QT], 2),
                ((64, 128), lambda n: kT_sb[0:64, 1, n * QT : (n + 1) * QT], 2),
            ]),
        ]

        proj_psums = {}

        def proj_unit(mi, n, kpair):
            """Two K-step matmuls of group (mi, n); evictions after the last."""
            c0, evicts = mgroups[mi]
            key = (mi, n)
            if key not in proj_psums:
                proj_psums[key] = ps.tile([128, QT], F32, tag="ps", name="projp")
            p = proj_psums[key]
            for k in (2 * kpair, 2 * kpair + 1):
                nc.tensor.matmul(
                    p[:, :],
                    lhsT=wpk_sb[:, k, c0 : c0 + 128],
                    rhs=xT_sb[:, k, n * QT : (n + 1) * QT],
                    start=(k == 0),
                    stop=(k == NKD - 1),
                )
            if kpair == 2:
                del proj_psums[key]
                for (r0, r1), dst, bcol in evicts:
                    nc.vector.tensor_scalar_add(
                        out=dst(n),
                        in0=p[r0:r1, :],
                        scalar1=bqk_sb[r0:r1, bcol : bcol + 1],
                    )

        vp_psums = {}

        def v_unit(t, kpair):
            """V projection for keys-tile t: [128 keys, 192] psum; evict into
            v_aug[:, t, :, 0:64] (all 3 heads at once)."""
            if t not in vp_psums:
                vp_psums[t] = ps.tile([128, QT], F32, tag="ps", name="vp")
            p = vp_psums[t]
            for k in (2 * kpair, 2 * kpair + 1):
                nc.tensor.matmul(
                    p[:, 0:GD],
                    lhsT=xT_sb[:, k, t * 128 : (t + 1) * 128],
                    rhs=wv_sb[:, k, :],
                    start=(k == 0),
                    stop=(k == NKD - 1),
                )
            if kpair == 2:
                del vp_psums[t]
                nc.vector.tensor_copy(
                    out=v_aug[:, t, :, 0:DH],
                    in_=p[:, 0:GD].rearrange("p (h d) -> p h d", h=HPC),
                )

        out_t = out_p.rearrange("(tp p) d -> tp p d", p=128)
        o_done = {}

        def o_proj_unit(t, n2, evict_eng=None):
            key = t
            if key not in o_done:
                o_done[key] = expp.tile([128, D], F16, tag="osb", name="osb", bufs=3)
            ob = o_done[key]
            po = ps.tile([128, QT], F32, tag="ps")
            nc.tensor.matmul(
                po[:, 0:384],
                lhsT=zT01[:, t * 128 : (t + 1) * 128],
                rhs=wo_a[:, n2 * 384 : (n2 + 1) * 384],
                start=True,
                stop=False,
            )
            nc.tensor.matmul(
                po[:, 0:384],
                lhsT=zT2[:, t * 128 : (t + 1) * 128],
                rhs=wo_b[:, n2 * 384 : (n2 + 1) * 384],
                start=False,
                stop=True,
            )
            eng = evict_eng
            if eng is None:
                eng = "act" if (t + n2) % 2 == 0 else "dve"
            if eng == "act":
                nc.scalar.activation(
                    out=ob[:, n2 * 384 : (n2 + 1) * 384], in_=po[:, 0:384], func=AF.Copy
                )
            else:
                nc.vector.tensor_copy(
                    out=ob[:, n2 * 384 : (n2 + 1) * 384], in_=po[:, 0:384]
                )
            if n2 == 1:
                del o_done[key]
                qdma[2 - (t % 2)].dma_start(out=out_t[t], in_=ob[:, :])

        # background work queue of (key, fn), drained between attention
        # iterations. Queue order is topological, so force-draining "through
        # the last needed unit" preserves all producer->consumer ordering.
        work = deque()

        def q_proj(n, mis):
            for mi in mis:
                for kpair in range(3):
                    work.append(
                        (("proj", n, mi), lambda mi=mi, n=n, kp=kpair: proj_unit(mi, n, kp))
                    )

        def q_v(ts):
            for t in ts:
                for kpair in range(3):
                    work.append((("v", t), lambda t=t, kp=kpair: v_unit(t, kp)))

        def drain(k=1):
            for _ in range(k):
                if work:
                    work.popleft()[1]()

        def drain_all():
            while work:
                work.popleft()[1]()

        # head h's scores need these packed q/k groups
        PROJ_GROUPS_FOR_HEAD = {0: (0, 1), 1: (0, 1), 2: (2,)}

        def force_drain_for(h, qt):
            """Emit queued units up to the last one attention(h, qt) depends on."""
            needed = set()
            for n in range(qt + 1):
                for mi in PROJ_GROUPS_FOR_HEAD[h]:
                    needed.add(("proj", n, mi))
            for t in range(4 * qt + 4):
                needed.add(("v", t))
            last = -1
            for i, (key, _) in enumerate(work):
                if key in needed:
                    last = i
            for _ in range(last + 1):
                work.popleft()[1]()

        def qh(h):
            col, off = [(0, 0), (0, 64), (1, 0)][h]
            return qT_sb[off : off + 64, col, :]

        def kh(h):
            col, off = [(0, 0), (0, 64), (1, 0)][h]
            return kT_sb[off : off + 64, col, :]

        zdst = [zT01[0:64, :], zT01[64:128, :], zT2[0:64, :]]

        # PV matmuls are pipelined a few pairs behind their exp across block
        # boundaries, so the in-order PE FIFO never waits on the exp/mask
        # chain, not even at the end of a block.
        pvq = deque()  # (block_serial, pv_closure)
        blk_serial = [0]

        def pv_drain(depth):
            while len(pvq) > depth:
                pvq.popleft()[1]()

        def pv_flush(upto_serial):
            while pvq and pvq[0][0] <= upto_serial:
                pvq.popleft()[1]()

        def attention(h, qt, per_pair):
            """scores^T -> exp -> causal mask -> PV into zp. Score matmuls for
            kt pairs land in one 2-bank psum tile so a single exp covers both;
            start=True zeroes the whole bank, so the unwritten low columns of
            narrowed diagonal halves exp to 1.0 and are never consumed."""
            zp = psz.tile([DH + 1, QT], F32)
            nkt = 4 * qt + 4
            blk = blk_serial[0]
            blk_serial[0] += 1

            def pv(kt, es_ap, lo):
                nc.tensor.matmul(
                    zp[:, lo:QT],
                    lhsT=v_aug[:, kt, h, :],
                    rhs=es_ap[:, lo:QT],
                    start=(kt == 0),
                    stop=(kt == nkt - 1),
                )

            for kp in range(nkt // 2):
                kt0 = 2 * kp
                rr0 = kt0 - 4 * qt
                lo_pair = 128 * rr0 if rr0 > 0 else 0
                sp = ps2.tile([128, 2, QT], F32, tag="ps2")
                los = []
                for i in (0, 1):
                    kt = kt0 + i
                    rr = kt - 4 * qt
                    lo = 128 * rr if rr > 0 else 0
                    los.append(lo)
                    nc.tensor.matmul(
                        sp[:, i, lo:QT],
                        lhsT=kh(h)[:, kt * 128 : (kt + 1) * 128],
                        rhs=qh(h)[:, qt * QT + lo : (qt + 1) * QT],
                        start=True,
                        stop=True,
                    )
                es = expp.tile([128, 2, QT], F16, tag="expp")
                nc.scalar.activation(
                    out=es[:, :, lo_pair:QT], in_=sp[:, :, lo_pair:QT], func=AF.Exp
                )
                for i in (0, 1):
                    kt = kt0 + i
                    rr = kt - 4 * qt
                    lo = los[i]
                    if rr >= 0:  # diagonal square: zero where key > query
                        hi = min(lo + 128, QT)
                        nc.gpsimd.affine_select(
                            out=es[:, i, lo:hi],
                            in_=es[:, i, lo:hi],
                            compare_op=mybir.AluOpType.is_ge,
                            fill=0.0,
                            base=0,
                            channel_multiplier=-1,
                            pattern=[[1, hi - lo]],
                        )
                    pvq.append(
                        (blk, lambda kt=kt, es=es, i=i, lo=lo: pv(kt, es[:, i], lo))
                    )
                drain(per_pair)
                pv_drain(7)
            return zp, blk

        def normalize(zp, h, qt, cols=slice(0, QT)):
            rec = small.tile([1, QT], F16, tag="rec")
            with nc.allow_low_precision(reason="fp16 normalize"):
                nc.vector.reciprocal(rec[:, cols], zp[DH : DH + 1, cols])
            bc = ps.tile([128, QT], F32, tag="ps")
            nc.tensor.matmul(
                bc[0:64, cols], lhsT=ones64[:], rhs=rec[:, cols], start=True, stop=True
            )
            bc_sb = small.tile([64, QT], F16, tag="bcsb")
            if qt == NQT - 1:
                nc.vector.tensor_copy(out=bc_sb[:, cols], in_=bc[0:64, cols])
            else:
                nc.scalar.activation(out=bc_sb[:, cols], in_=bc[0:64, cols], func=AF.Copy)
            nc.vector.tensor_mul(
                zdst[h][:, qt * QT : (qt + 1) * QT][:, cols],
                zp[0:DH, cols],
                bc_sb[:, cols],
            )

        # ---- schedule ----
        # prologue: only what attention(h0, qt0) needs; the rest queues up.
        for mi in (0, 1):
            for kpair in range(3):
                proj_unit(mi, 0, kpair)
        for t in range(4):
            for kpair in range(3):
                v_unit(t, kpair)
        q_proj(0, mis=(2,))
        for n in range(1, NQT):
            q_proj(n, mis=(0, 1))
            q_v(range(4 * n, 4 * n + 2))
            q_proj(n, mis=(2,))
            q_v(range(4 * n + 2, 4 * n + 4))

        pending = None
        for qt in range(NQT):
            per_pair = [4, 1, 1, 2][qt]
            for h in range(HPC):
                force_drain_for(h, qt)
                zp, blk = attention(h, qt, per_pair)
                if pending is not None:
                    pv_flush(pending[3])  # pending block's PV accumulation done
                    normalize(*pending[:3])
                    ph, pqt = pending[1], pending[2]
                    if ph == HPC - 1:  # whole q-tile normalized -> O-proj ready
                        for t in range(4 * pqt, 4 * pqt + 4):
                            for n2 in range(2):
                                work.append(
                                    (("o", pqt), lambda t=t, n2=n2: o_proj_unit(t, n2))
                                )
                pending = (zp, h, qt, blk)
        # final block: normalize in column halves so the last O-proj pairs
        # start while the second half's recip/broadcast chain is still running
        pv_flush(pending[3])
        drain_all()
        normalize(*pending[:3], cols=slice(0, QT // 2))
        engs = ["dve", "act", "dve", "act", "dve", "act", "dve", "act"]
        ei = 0
        for t in (12, 13):
            for n2 in range(2):
                o_proj_unit(t, n2, evict_eng=engs[ei])
                ei += 1
        normalize(*pending[:3], cols=slice(QT // 2, QT))
        for t in (14, 15):
            for n2 in range(2):
                o_proj_unit(t, n2, evict_eng=engs[ei % len(engs)])
                ei += 1
    nc.finalize()
    return nc


_NC_CACHE = {}


def make_in_maps(x, W_qkv, b_qkv, W_o):
    in_maps = []
    for c in range(8):
        b, g = divmod(c, 4)
        hs = [HPC * g + i for i in range(HPC)]
        qr = [np.arange(64 * h, 64 * h + 64) for h in hs]
        w_q = [W_qkv[i] * 0.125 for i in qr]
        w_k = [W_qkv[768 + i] for i in qr]
        w_v = [W_qkv[1536 + i] for i in qr]
        b_q = [b_qkv[i] * 0.125 for i in qr]
        b_k = [b_qkv[768 + i] for i in qr]
        # packed rows: m0=[q0 q1] m1=[k0 k1] m2=[q2 k2]
        wpk = np.concatenate(
            [w_q[0], w_q[1], w_k[0], w_k[1], w_q[2], w_k[2]], axis=0
        )
        wv = np.concatenate([w_v[0], w_v[1], w_v[2]], axis=0)
        bqk_col = np.zeros((128, 3), np.float32)
        bqk_col[:, 0] = np.concatenate([b_q[0], b_q[1]])
        bqk_col[:, 1] = np.concatenate([b_k[0], b_k[1]])
        bqk_col[:, 2] = np.concatenate([b_q[2], b_k[2]])
        in_maps.append(
            {
                "xT": np.ascontiguousarray(x[b].T.astype(np.float16)),
                "wpk": np.ascontiguousarray(wpk.T.astype(np.float16)),
                "wv": np.ascontiguousarray(wv.T.astype(np.float16)),
                "woT": np.ascontiguousarray(
                    W_o[:, GD * g : GD * (g + 1)].T.astype(np.float16)
                ),
                "bqk": bqk_col,
                "vones": np.ones((128, 64), np.float16),
            }
        )
    return in_maps


def make_in_maps_for_test(inputs):
    return make_in_maps(
        np.asarray(inputs["x"], np.float32),
        np.asarray(inputs["W_qkv"], np.float32),
        np.asarray(inputs["b_qkv"], np.float32),
        np.asarray(inputs["W_o"], np.float32),
    )


def kernel(x, W_qkv, b_qkv, W_o, b_o):
    x = np.asarray(x, np.float32)
    W_qkv = np.asarray(W_qkv, np.float32)
    b_qkv = np.asarray(b_qkv, np.float32)
    W_o = np.asarray(W_o, np.float32)
    b_o = np.asarray(b_o, np.float32)

    if "nc" not in _NC_CACHE:
        _NC_CACHE["nc"] = build_bass()
    nc = _NC_CACHE["nc"]

    in_maps = make_in_maps(x, W_qkv, b_qkv, W_o)

    res = run_bass_kernel_spmd(
        nc,
        in_maps,
        list(range(8)),
        trace=bool(int(os.environ.get("KERNEL_TRACE", "0"))),
    )
    _NC_CACHE["last_results"] = res

    out = np.zeros((B, S, D), np.float32)
    for c in range(8):
        out[c // 4] += res.results[c]["out_p"].astype(np.float32)
    out += b_qkv[1536:] @ W_o.T + b_o
    return out


# revision 13
# speedup vs baseline: 1.1313x; 1.0056x over previous
"""Causal multi-head attention block (B=2, S=2048, D=768, H=12) on 8 trn2 cores.

Sharding: core c -> batch b = c//4 (data parallel), head group g = c%4
(tensor parallel, 3 heads per group). Each core computes its group's QKV
projection, causal attention, and a partial O-projection over its 192
z-columns. Host sums the 4 partials per batch and adds the biases that
commute through the math (v-bias and b_o).

On-core layout: xT [768, 2048] (d on partitions) so q^T/k^T come straight
out of the projection with head dims on partitions; V is projected
separately in [keys, dh] layout (lhsT = xT key-block, rhs = W_v^T columns)
so the PV matmul needs no transposes at all. A ones-column appended to V
yields the softmax denominator for free.

All matmul operands are fp16 (1 col/cycle on the PE, 10-bit mantissa keeps
rel-err ~5e-4); psum stays f32. Score matmuls for kt pairs land in one
2-bank psum tile so a single exp instruction covers both; the causal mask
only touches the 128-wide diagonal square.

The packed q/k projection weight is host-repacked so all three 128-wide
M-groups are full:  m0=[q_h0 q_h1]  m1=[q_h2 k_h0]  m2=[k_h1 k_h2]
(q rows pre-scaled by 1/8; v bias folded into the host-side epilogue).
"""

import os
from collections import deque
from contextlib import ExitStack

import numpy as np

import concourse.tile as tile
from concourse import bacc, mybir
from concourse.bass_utils import run_bass_kernel_spmd

F32 = mybir.dt.float32
F16 = mybir.dt.float16
AF = mybir.ActivationFunctionType

B, S, D = 2, 2048, 768
NH, DH = 12, 64
HPC = 3            # heads per core
GD = HPC * DH      # 192 z-cols per core
KT, QT = 128, 512  # key tile (partitions), q tile (psum free)
NKT, NQT = S // KT, S // QT   # 16, 4
NKD = D // 128     # 6 contraction tiles for the projections
WPK = 2 * GD       # 384 packed q/k projection rows


def build_bass():
    nc = bacc.Bacc(None)
    xT = nc.dram_tensor("xT", [D, S], F16, kind="ExternalInput")
    wpk = nc.dram_tensor("wpk", [D, WPK], F16, kind="ExternalInput")
    wv = nc.dram_tensor("wv", [D, GD], F16, kind="ExternalInput")
    woT = nc.dram_tensor("woT", [GD, D], F16, kind="ExternalInput")
    bqk = nc.dram_tensor("bqk", [128, 3], F32, kind="ExternalInput")
    vones = nc.dram_tensor("vones", [128, 64], F16, kind="ExternalInput")
    out_p = nc.dram_tensor("out_p", [S, D], F16, kind="ExternalOutput")

    with tile.TileContext(nc) as tc, ExitStack() as ctx:
        const = ctx.enter_context(tc.tile_pool(name="const", bufs=1))
        ps = ctx.enter_context(tc.tile_pool(name="ps", bufs=2, space="PSUM"))
        ps2 = ctx.enter_context(tc.tile_pool(name="ps2", bufs=2, space="PSUM"))
        psz = ctx.enter_context(tc.tile_pool(name="psz", bufs=2, space="PSUM"))
        expp = ctx.enter_context(tc.tile_pool(name="expp", bufs=9))
        small = ctx.enter_context(tc.tile_pool(name="small", bufs=4))

        xT_sb = const.tile([128, NKD, S], F16)
        wpk_sb = const.tile([128, NKD, WPK], F16)
        wv_sb = const.tile([128, NKD, GD], F16)
        wo_a = const.tile([128, D], F16)
        wo_b = const.tile([64, D], F16)
        bqk_sb = const.tile([128, 3], F32)
        qT_sb = const.tile([128, 2, S], F16)
        kT_sb = const.tile([128, 2, S], F16)
        v_aug = const.tile([128, NKT, HPC, DH + 1], F16)
        zT01 = const.tile([128, S], F16)
        zT2 = const.tile([64, S], F16)
        ones64 = const.tile([1, 64], F16)
        ones_stage = const.tile([128, 64], F16)

        # ---- loads: k-interleaved so the first projection k-pairs unblock
        # early. Early DMAs fan out over four queues (SP/Act/DVE/Pool) to
        # dodge the ~650ns per-DMA dispatch serialization on a single queue.
        xT_t = xT.rearrange("(t p) s -> t p s", p=128)
        wpk_t = wpk.rearrange("(t p) m -> t p m", p=128)
        wv_t = wv.rearrange("(t p) m -> t p m", p=128)
        qdma = [nc.sync, nc.scalar, nc.gpsimd]
        # m0 weight slivers + first x chunks first, spread over all 3 queues
        for t in range(NKD):
            qdma[t % 3].dma_start(out=wpk_sb[:, t, 0:128], in_=wpk_t[t][:, 0:128])
            qdma[(t + 1) % 3].dma_start(out=xT_sb[:, t, 0:QT], in_=xT_t[t][:, 0:QT])
        for t in range(NKD):
            qdma[t % 3].dma_start(out=wpk_sb[:, t, 128:WPK], in_=wpk_t[t][:, 128:WPK])
        nc.sync.dma_start(out=bqk_sb[:], in_=bqk[:, :])
        for t in range(NKD):
            qdma[t % 3].dma_start(out=wv_sb[:, t, :], in_=wv_t[t][:, :])
        nc.gpsimd.dma_start(out=ones_stage[:], in_=vones[:, :])
        nc.gpsimd.dma_start(out=ones64[:], in_=vones[0:1, 0:64])
        nc.vector.tensor_copy(
            out=v_aug[:, :, :, DH],
            in_=ones_stage[:, 0 : NKT * HPC].rearrange("p (t h) -> p t h", t=NKT),
        )
        for t in range(NKD):
            qdma[t % 3].dma_start(
                out=xT_sb[:, t, QT : 2 * QT], in_=xT_t[t][:, QT : 2 * QT]
            )
        nc.scalar.dma_start(out=wo_a[:], in_=woT[0:128, :])
        nc.gpsimd.dma_start(out=wo_b[:], in_=woT[128:GD, :])
        for t in range(NKD):
            qdma[t % 3].dma_start(
                out=xT_sb[:, t, 2 * QT : S], in_=xT_t[t][:, 2 * QT : S]
            )

        # packed q/k projection m-groups: (col0, evict spec). m2 holds
        # [q2 k2]; its k2 rows evict to kT partitions 0:64 (cross-base copy)
        # so every head's q and k share a partition base for the PE.
        mgroups = [
            (0, [((0, 128), lambda n: qT_sb[0:128, 0, n * QT : (n + 1) * QT], 0)]),
            (128, [((0, 128), lambda n: kT_sb[0:128, 0, n * QT : (n + 1) * QT], 1)]),
            (256, [
                ((0, 64), lambda n: qT_sb[0:64, 1, n * QT : (n + 1) * QT], 2),
                ((64, 128), lambda n: kT_sb[0:64, 1, n * QT : (n + 1) * QT], 2),
            ]),
        ]

        proj_psums = {}

        def proj_unit(mi, n, kpair):
            """Two K-step matmuls of group (mi, n); evictions after the last."""
            c0, evicts = mgroups[mi]
            key = (mi, n)
            if key not in proj_psums:
                proj_psums[key] = ps.tile([128, QT], F32, tag="ps", name="projp")
            p = proj_psums[key]
            for k in (2 * kpair, 2 * kpair + 1):
                nc.tensor.matmul(
                    p[:, :],
                    lhsT=wpk_sb[:, k, c0 : c0 + 128],
                    rhs=xT_sb[:, k, n * QT : (n + 1) * QT],
                    start=(k == 0),
                    stop=(k == NKD - 1),
                )
            if kpair == 2:
                del proj_psums[key]
                for (r0, r1), dst, bcol in evicts:
                    nc.vector.tensor_scalar_add(
                        out=dst(n),
                        in0=p[r0:r1, :],
                        scalar1=bqk_sb[r0:r1, bcol : bcol + 1],
                    )

        vp_psums = {}

        def v_unit(t, kpair):
            """V projection for keys-tile t: [128 keys, 192] psum; evict into
            v_aug[:, t, :, 0:64] (all 3 heads at once)."""
            if t not in vp_psums:
                vp_psums[t] = ps.tile([128, QT], F32, tag="ps", name="vp")
            p = vp_psums[t]
            for k in (2 * kpair, 2 * kpair + 1):
                nc.tensor.matmul(
                    p[:, 0:GD],
                    lhsT=xT_sb[:, k, t * 128 : (t + 1) * 128],
                    rhs=wv_sb[:, k, :],
                    start=(k == 0),
                    stop=(k == NKD - 1),
                )
            if kpair == 2:
                del vp_psums[t]
                nc.vector.tensor_copy(
                    out=v_aug[:, t, :, 0:DH],
                    in_=p[:, 0:GD].rearrange("p (h d) -> p h d", h=HPC),
                )

        out_t = out_p.rearrange("(tp p) d -> tp p d", p=128)
        o_done = {}

        def o_proj_unit(t, n2, evict_eng=None, tail=False):
            key = t
            if key not in o_done:
                o_done[key] = expp.tile([128, D], F16, tag="osb", name="osb", bufs=3)
            ob = o_done[key]
            po = ps.tile([128, QT], F32, tag="ps")
            nc.tensor.matmul(
                po[:, 0:384],
                lhsT=zT01[:, t * 128 : (t + 1) * 128],
                rhs=wo_a[:, n2 * 384 : (n2 + 1) * 384],
                start=True,
                stop=False,
            )
            nc.tensor.matmul(
                po[:, 0:384],
                lhsT=zT2[:, t * 128 : (t + 1) * 128],
                rhs=wo_b[:, n2 * 384 : (n2 + 1) * 384],
                start=False,
                stop=True,
            )
            eng = evict_eng
            if eng is None:
                eng = "dve"
            if eng == "act":
                nc.scalar.activation(
                    out=ob[:, n2 * 384 : (n2 + 1) * 384], in_=po[:, 0:384], func=AF.Copy
                )
            else:
                nc.vector.tensor_copy(
                    out=ob[:, n2 * 384 : (n2 + 1) * 384], in_=po[:, 0:384]
                )
            if n2 == 1:
                del o_done[key]
                (nc.sync if (t % 2 == 0 or tail) else nc.gpsimd).dma_start(
                    out=out_t[t], in_=ob[:, :]
                )

        # background work queue of (key, fn), drained between attention
        # iterations. Queue order is topological, so force-draining "through
        # the last needed unit" preserves all producer->consumer ordering.
        work = deque()

        def q_proj(n, mis):
            for mi in mis:
                for kpair in range(3):
                    work.append(
                        (("proj", n, mi), lambda mi=mi, n=n, kp=kpair: proj_unit(mi, n, kp))
                    )

        def q_v(ts):
            for t in ts:
                for kpair in range(3):
                    work.append((("v", t), lambda t=t, kp=kpair: v_unit(t, kp)))

        def drain(k=1):
            for _ in range(k):
                if work:
                    work.popleft()[1]()

        def drain_all():
            while work:
                work.popleft()[1]()

        # head h's scores need these packed q/k groups
        PROJ_GROUPS_FOR_HEAD = {0: (0, 1), 1: (0, 1), 2: (2,)}

        def force_drain_for(h, qt):
            """Emit queued units up to the last one attention(h, qt) depends on."""
            needed = set()
            for n in range(qt + 1):
                for mi in PROJ_GROUPS_FOR_HEAD[h]:
                    needed.add(("proj", n, mi))
            for t in range(4 * qt + 4):
                needed.add(("v", t))
            last = -1
            for i, (key, _) in enumerate(work):
                if key in needed:
                    last = i
            for _ in range(last + 1):
                work.popleft()[1]()

        def qh(h):
            col, off = [(0, 0), (0, 64), (1, 0)][h]
            return qT_sb[off : off + 64, col, :]

        def kh(h):
            col, off = [(0, 0), (0, 64), (1, 0)][h]
            return kT_sb[off : off + 64, col, :]

        zdst = [zT01[0:64, :], zT01[64:128, :], zT2[0:64, :]]

        # PV matmuls are pipelined a few pairs behind their exp across block
        # boundaries, so the in-order PE FIFO never waits on the exp/mask
        # chain, not even at the end of a block.
        pvq = deque()  # (block_serial, pv_closure)
        blk_serial = [0]

        def pv_drain(depth):
            while len(pvq) > depth:
                pvq.popleft()[1]()

        def pv_flush(upto_serial):
            while pvq and pvq[0][0] <= upto_serial:
                pvq.popleft()[1]()

        def attention(h, qt, per_pair, last=False):
            """scores^T -> exp -> causal mask -> PV into zp. Score matmuls for
            kt pairs land in one 2-bank psum tile so a single exp covers both;
            start=True zeroes the whole bank, so the unwritten low columns of
            narrowed diagonal halves exp to 1.0 and are never consumed."""
            zp = psz.tile([DH + 1, QT], F32)
            nkt = 4 * qt + 4
            blk = blk_serial[0]
            blk_serial[0] += 1

            def pv(kt, es_ap, lo):
                nc.tensor.matmul(
                    zp[:, lo:QT],
                    lhsT=v_aug[:, kt, h, :],
                    rhs=es_ap[:, lo:QT],
                    start=(kt == 0),
                    stop=(kt == nkt - 1),
                )

            def pv_q(kt, es_ap, lo):
                # last block: per-128-col accumulation groups, so quarter q's
                # group closes at kt = nkt-4+q and can be read immediately.
                for q in range(lo // 128, 4):
                    c0 = 128 * q
                    nc.tensor.matmul(
                        zp[:, c0 : c0 + 128],
                        lhsT=v_aug[:, kt, h, :],
                        rhs=es_ap[:, c0 : c0 + 128],
                        start=(kt == 0),
                        stop=(kt == nkt - 4 + q),
                    )
                if kt >= nkt - 4:
                    stairs(kt - (nkt - 4))

            st_rec, st_bc = {}, {}

            def qcols(q):
                return slice(128 * q, 128 * (q + 1))

            def stairs(step):
                # stage skew keeps the in-order PE FIFO free of long waits:
                # each quarter's PE pieces (bc, o-proj) are emitted one or two
                # pv-steps after the pv that closed the quarter's psum group.
                if step >= 1:
                    st_bc[step - 1] = norm_bc(st_rec[step - 1], qcols(step - 1), bc_ps2=True)
                st_rec[step] = norm_recip(zp, qcols(step))
                if step >= 1:
                    norm_post(zp, h, qt, st_bc[step - 1], qcols(step - 1), eng="act")
                if step >= 2:
                    t = 4 * qt + step - 2
                    o_proj_unit(t, 0, evict_eng="dve", tail=True)
                    o_proj_unit(t, 1, evict_eng="act", tail=True)

            def stairs_final():
                st_bc[3] = norm_bc(st_rec[3], qcols(3), bc_ps2=True)
                norm_post(zp, h, qt, st_bc[3], qcols(3), eng="act")
                for t in (4 * qt + 2, 4 * qt + 3):
                    o_proj_unit(t, 0, evict_eng="dve", tail=True)
                    o_proj_unit(t, 1, evict_eng="act", tail=True)

            for kp in range(nkt // 2):
                kt0 = 2 * kp
                rr0 = kt0 - 4 * qt
                lo_pair = 128 * rr0 if rr0 > 0 else 0
                sp = ps2.tile([128, 2, QT], F32, tag="ps2")
                los = []
                for i in (0, 1):
                    kt = kt0 + i
                    rr = kt - 4 * qt
                    lo = 128 * rr if rr > 0 else 0
                    los.append(lo)
                    nc.tensor.matmul(
                        sp[:, i, lo:QT],
                        lhsT=kh(h)[:, kt * 128 : (kt + 1) * 128],
                        rhs=qh(h)[:, qt * QT + lo : (qt + 1) * QT],
                        start=True,
                        stop=True,
                    )
                es = expp.tile([128, 2, QT], F16, tag="expp")
                nc.scalar.activation(
                    out=es[:, :, lo_pair:QT], in_=sp[:, :, lo_pair:QT], func=AF.Exp
                )
                for i in (0, 1):
                    kt = kt0 + i
                    rr = kt - 4 * qt
                    lo = los[i]
                    if rr >= 0:  # diagonal square: zero where key > query
                        hi = min(lo + 128, QT)
                        nc.gpsimd.affine_select(
                            out=es[:, i, lo:hi],
                            in_=es[:, i, lo:hi],
                            compare_op=mybir.AluOpType.is_ge,
                            fill=0.0,
                            base=0,
                            channel_multiplier=-1,
                            pattern=[[1, hi - lo]],
                        )
                    fn = pv_q if last else pv
                    pvq.append(
                        (blk, lambda kt=kt, es=es, i=i, lo=lo, fn=fn: fn(kt, es[:, i], lo))
                    )
                drain(per_pair)
                pv_drain(2 if last else 7)
            if last:
                pv_flush(blk)
                stairs_final()
            return zp, blk

        def norm_recip(zp, cols):
            rec = small.tile([1, QT], F16, tag="rec")
            with nc.allow_low_precision(reason="fp16 normalize"):
                nc.vector.reciprocal(rec[:, cols], zp[DH : DH + 1, cols])
            return rec

        def norm_bc(rec, cols, bc_ps2=False):
            if bc_ps2:
                bc2 = ps2.tile([128, 2, QT], F32, tag="ps2", name="bc2")
                bc = bc2[:, 0]
            else:
                bc = ps.tile([128, QT], F32, tag="ps")
            nc.tensor.matmul(
                bc[0:64, cols], lhsT=ones64[:], rhs=rec[:, cols], start=True, stop=True
            )
            return bc

        def norm_post(zp, h, qt, bc, cols, eng="act"):
            bc_sb = small.tile([64, QT], F16, tag="bcsb")
            if eng == "dve":
                nc.vector.tensor_copy(out=bc_sb[:, cols], in_=bc[0:64, cols])
            else:
                nc.scalar.activation(out=bc_sb[:, cols], in_=bc[0:64, cols], func=AF.Copy)
            nc.vector.tensor_mul(
                zdst[h][:, qt * QT : (qt + 1) * QT][:, cols],
                zp[0:DH, cols],
                bc_sb[:, cols],
            )

        def normalize(zp, h, qt, cols=slice(0, QT)):
            rec = norm_recip(zp, cols)
            bc = norm_bc(rec, cols)
            norm_post(zp, h, qt, bc, cols, eng="act")

        # ---- schedule ----
        # prologue: only what attention(h0, qt0) needs; the rest queues up.
        for mi in (0, 1):
            for kpair in range(3):
                proj_unit(mi, 0, kpair)
        for t in range(4):
            for kpair in range(3):
                v_unit(t, kpair)
        q_proj(0, mis=(2,))
        for n in range(1, NQT):
            q_proj(n, mis=(0, 1))
            q_v(range(4 * n, 4 * n + 2))
            q_proj(n, mis=(2,))
            q_v(range(4 * n + 2, 4 * n + 4))

        pending = None
        for qt in range(NQT):
            per_pair = [4, 1, 1, 2][qt]
            for h in range(HPC):
                force_drain_for(h, qt)
                is_last = qt == NQT - 1 and h == HPC - 1
                if is_last:
                    # everything queued must land before the staircase epilogue
                    pv_flush(pending[3])
                    normalize(*pending[:3])
                    drain_all()
                    pending = None
                zp, blk = attention(h, qt, per_pair, last=is_last)
                if pending is not None:
                    pv_flush(pending[3])  # pending block's PV accumulation done
                    normalize(*pending[:3])
                    ph, pqt = pending[1], pending[2]
                    if ph == HPC - 1:  # whole q-tile normalized -> O-proj ready
                        for t in range(4 * pqt, 4 * pqt + 4):
                            for n2 in range(2):
                                work.append(
                                    (("o", pqt), lambda t=t, n2=n2: o_proj_unit(t, n2))
                                )
                if not is_last:
                    pending = (zp, h, qt, blk)
    nc.finalize()
    return nc


_NC_CACHE = {}


def make_in_maps(x, W_qkv, b_qkv, W_o):
    in_maps = []
    for c in range(8):
        b, g = divmod(c, 4)
        hs = [HPC * g + i for i in range(HPC)]
        qr = [np.arange(64 * h, 64 * h + 64) for h in hs]
        w_q = [W_qkv[i] * 0.125 for i in qr]
        w_k = [W_qkv[768 + i] for i in qr]
        w_v = [W_qkv[1536 + i] for i in qr]
        b_q = [b_qkv[i] * 0.125 for i in qr]
        b_k = [b_qkv[768 + i] for i in qr]
        # packed rows: m0=[q0 q1] m1=[k0 k1] m2=[q2 k2]
        wpk = np.concatenate(
            [w_q[0], w_q[1], w_k[0], w_k[1], w_q[2], w_k[2]], axis=0
        )
        wv = np.concatenate([w_v[0], w_v[1], w_v[2]], axis=0)
        bqk_col = np.zeros((128, 3), np.float32)
        bqk_col[:, 0] = np.concatenate([b_q[0], b_q[1]])
        bqk_col[:, 1] = np.concatenate([b_k[0], b_k[1]])
        bqk_col[:, 2] = np.concatenate([b_q[2], b_k[2]])
        in_maps.append(
            {
                "xT": np.ascontiguousarray(x[b].T.astype(np.float16)),
                "wpk": np.ascontiguousarray(wpk.T.astype(np.float16)),
                "wv": np.ascontiguousarray(wv.T.astype(np.float16)),
                "woT": np.ascontiguousarray(
                    W_o[:, GD * g : GD * (g + 1)].T.astype(np.float16)
                ),
                "bqk": bqk_col,
                "vones": np.ones((128, 64), np.float16),
            }
        )
    return in_maps


def make_in_maps_for_test(inputs):
    return make_in_maps(
        np.asarray(inputs["x"], np.float32),
        np.asarray(inputs["W_qkv"], np.float32),
        np.asarray(inputs["b_qkv"], np.float32),
        np.asarray(inputs["W_o"], np.float32),
    )


def kernel(x, W_qkv, b_qkv, W_o, b_o):
    x = np.asarray(x, np.float32)
    W_qkv = np.asarray(W_qkv, np.float32)
    b_qkv = np.asarray(b_qkv, np.float32)
    W_o = np.asarray(W_o, np.float32)
    b_o = np.asarray(b_o, np.float32)

    if "nc" not in _NC_CACHE:
        _NC_CACHE["nc"] = build_bass()
    nc = _NC_CACHE["nc"]

    in_maps = make_in_maps(x, W_qkv, b_qkv, W_o)

    res = run_bass_kernel_spmd(
        nc,
        in_maps,
        list(range(8)),
        trace=bool(int(os.environ.get("KERNEL_TRACE", "0"))),
    )
    _NC_CACHE["last_results"] = res

    out = np.zeros((B, S, D), np.float32)
    for c in range(8):
        out[c // 4] += res.results[c]["out_p"].astype(np.float32)
    out += b_qkv[1536:] @ W_o.T + b_o
    return out


# revision 14
# speedup vs baseline: 1.1324x; 1.0010x over previous
"""Causal multi-head attention block (B=2, S=2048, D=768, H=12) on 8 trn2 cores.

Sharding: core c -> batch b = c//4 (data parallel), head group g = c%4
(tensor parallel, 3 heads per group). Each core computes its group's QKV
projection, causal attention, and a partial O-projection over its 192
z-columns. Host sums the 4 partials per batch and adds the biases that
commute through the math (v-bias and b_o).

On-core layout: xT [768, 2048] (d on partitions) so q^T/k^T come straight
out of the projection with head dims on partitions; V is projected
separately in [keys, dh] layout (lhsT = xT key-block, rhs = W_v^T columns)
so the PV matmul needs no transposes at all. A ones-column appended to V
yields the softmax denominator for free.

All matmul operands are fp16 (1 col/cycle on the PE, 10-bit mantissa keeps
rel-err ~5e-4); psum stays f32. Score matmuls for kt pairs land in one
2-bank psum tile so a single exp instruction covers both; the causal mask
only touches the 128-wide diagonal square.

The packed q/k projection weight is host-repacked so all three 128-wide
M-groups are full:  m0=[q_h0 q_h1]  m1=[q_h2 k_h0]  m2=[k_h1 k_h2]
(q rows pre-scaled by 1/8; v bias folded into the host-side epilogue).
"""

import os
from collections import deque
from contextlib import ExitStack

import numpy as np

import concourse.tile as tile
from concourse import bacc, mybir
from concourse.bass_utils import run_bass_kernel_spmd

F32 = mybir.dt.float32
F16 = mybir.dt.float16
AF = mybir.ActivationFunctionType

B, S, D = 2, 2048, 768
NH, DH = 12, 64
HPC = 3            # heads per core
GD = HPC * DH      # 192 z-cols per core
KT, QT = 128, 512  # key tile (partitions), q tile (psum free)
NKT, NQT = S // KT, S // QT   # 16, 4
NKD = D // 128     # 6 contraction tiles for the projections
WPK = 2 * GD       # 384 packed q/k projection rows


def build_bass():
    nc = bacc.Bacc(None)
    xT = nc.dram_tensor("xT", [D, S], F16, kind="ExternalInput")
    wpk = nc.dram_tensor("wpk", [D, WPK], F16, kind="ExternalInput")
    wv = nc.dram_tensor("wv", [D, GD], F16, kind="ExternalInput")
    woT = nc.dram_tensor("woT", [GD, D], F16, kind="ExternalInput")
    bqk = nc.dram_tensor("bqk", [128, 3], F32, kind="ExternalInput")
    vones = nc.dram_tensor("vones", [128, 64], F16, kind="ExternalInput")
    out_p = nc.dram_tensor("out_p", [S, D], F16, kind="ExternalOutput")

    with tile.TileContext(nc) as tc, ExitStack() as ctx:
        const = ctx.enter_context(tc.tile_pool(name="const", bufs=1))
        ps = ctx.enter_context(tc.tile_pool(name="ps", bufs=2, space="PSUM"))
        ps2 = ctx.enter_context(tc.tile_pool(name="ps2", bufs=2, space="PSUM"))
        psz = ctx.enter_context(tc.tile_pool(name="psz", bufs=2, space="PSUM"))
        expp = ctx.enter_context(tc.tile_pool(name="expp", bufs=9))
        small = ctx.enter_context(tc.tile_pool(name="small", bufs=4))

        xT_sb = const.tile([128, NKD, S], F16)
        wpk_sb = const.tile([128, NKD, WPK], F16)
        wv_sb = const.tile([128, NKD, GD], F16)
        wo_a = const.tile([128, D], F16)
        wo_b = const.tile([64, D], F16)
        bqk_sb = const.tile([128, 3], F32)
        qT_sb = const.tile([128, 2, S], F16)
        kT_sb = const.tile([128, 2, S], F16)
        v_aug = const.tile([128, NKT, HPC, DH + 1], F16)
        zT01 = const.tile([128, S], F16)
        zT2 = const.tile([64, S], F16)
        ones64 = const.tile([1, 64], F16)
        ones_stage = const.tile([128, 64], F16)

        # ---- loads: k-interleaved so the first projection k-pairs unblock
        # early. Early DMAs fan out over four queues (SP/Act/DVE/Pool) to
        # dodge the ~650ns per-DMA dispatch serialization on a single queue.
        xT_t = xT.rearrange("(t p) s -> t p s", p=128)
        wpk_t = wpk.rearrange("(t p) m -> t p m", p=128)
        wv_t = wv.rearrange("(t p) m -> t p m", p=128)
        qdma = [nc.sync, nc.scalar, nc.gpsimd]
        # m0 weight slivers + first x chunks first, spread over all 3 queues
        for t in range(NKD):
            qdma[t % 3].dma_start(out=wpk_sb[:, t, 0:128], in_=wpk_t[t][:, 0:128])
            qdma[(t + 1) % 3].dma_start(out=xT_sb[:, t, 0:QT], in_=xT_t[t][:, 0:QT])
        for t in range(NKD):
            qdma[t % 3].dma_start(out=wpk_sb[:, t, 128:WPK], in_=wpk_t[t][:, 128:WPK])
        nc.sync.dma_start(out=bqk_sb[:], in_=bqk[:, :])
        for t in range(NKD):
            qdma[t % 3].dma_start(out=wv_sb[:, t, :], in_=wv_t[t][:, :])
        nc.gpsimd.dma_start(out=ones_stage[:], in_=vones[:, :])
        nc.gpsimd.dma_start(out=ones64[:], in_=vones[0:1, 0:64])
        nc.vector.tensor_copy(
            out=v_aug[:, :, :, DH],
            in_=ones_stage[:, 0 : NKT * HPC].rearrange("p (t h) -> p t h", t=NKT),
        )
        for t in range(NKD):
            qdma[t % 3].dma_start(
                out=xT_sb[:, t, QT : 2 * QT], in_=xT_t[t][:, QT : 2 * QT]
            )
        nc.scalar.dma_start(out=wo_a[:], in_=woT[0:128, :])
        nc.gpsimd.dma_start(out=wo_b[:], in_=woT[128:GD, :])
        for t in range(NKD):
            qdma[t % 3].dma_start(
                out=xT_sb[:, t, 2 * QT : S], in_=xT_t[t][:, 2 * QT : S]
            )

        # packed q/k projection m-groups: (col0, evict spec). m2 holds
        # [q2 k2]; its k2 rows evict to kT partitions 0:64 (cross-base copy)
        # so every head's q and k share a partition base for the PE.
        mgroups = [
            (0, [((0, 128), lambda n: qT_sb[0:128, 0, n * QT : (n + 1) * QT], 0)]),
            (128, [((0, 128), lambda n: kT_sb[0:128, 0, n * QT : (n + 1) * QT], 1)]),
            (256, [
                ((0, 64), lambda n: qT_sb[0:64, 1, n * QT : (n + 1) * QT], 2),
                ((64, 128), lambda n: kT_sb[0:64, 1, n * QT : (n + 1) * QT], 2),
            ]),
        ]

        proj_psums = {}

        def proj_unit(mi, n, kpair):
            """Two K-step matmuls of group (mi, n); evictions after the last."""
            c0, evicts = mgroups[mi]
            key = (mi, n)
            if key not in proj_psums:
                proj_psums[key] = ps.tile([128, QT], F32, tag="ps", name="projp")
            p = proj_psums[key]
            for k in (2 * kpair, 2 * kpair + 1):
                nc.tensor.matmul(
                    p[:, :],
                    lhsT=wpk_sb[:, k, c0 : c0 + 128],
                    rhs=xT_sb[:, k, n * QT : (n + 1) * QT],
                    start=(k == 0),
                    stop=(k == NKD - 1),
                )
            if kpair == 2:
                del proj_psums[key]
                for (r0, r1), dst, bcol in evicts:
                    nc.vector.tensor_scalar_add(
                        out=dst(n),
                        in0=p[r0:r1, :],
                        scalar1=bqk_sb[r0:r1, bcol : bcol + 1],
                    )

        vp_psums = {}

        def v_unit(t, kpair):
            """V projection for keys-tile t: [128 keys, 192] psum; evict into
            v_aug[:, t, :, 0:64] (all 3 heads at once)."""
            if t not in vp_psums:
                vp_psums[t] = ps.tile([128, QT], F32, tag="ps", name="vp")
            p = vp_psums[t]
            for k in (2 * kpair, 2 * kpair + 1):
                nc.tensor.matmul(
                    p[:, 0:GD],
                    lhsT=xT_sb[:, k, t * 128 : (t + 1) * 128],
                    rhs=wv_sb[:, k, :],
                    start=(k == 0),
                    stop=(k == NKD - 1),
                )
            if kpair == 2:
                del vp_psums[t]
                nc.vector.tensor_copy(
                    out=v_aug[:, t, :, 0:DH],
                    in_=p[:, 0:GD].rearrange("p (h d) -> p h d", h=HPC),
                )

        out_t = out_p.rearrange("(tp p) d -> tp p d", p=128)
        o_done = {}

        def o_proj_unit(t, n2, evict_eng=None, tail=False):
            key = t
            if key not in o_done:
                o_done[key] = expp.tile([128, D], F16, tag="osb", name="osb", bufs=3)
            ob = o_done[key]
            po = ps.tile([128, QT], F32, tag="ps")
            nc.tensor.matmul(
                po[:, 0:384],
                lhsT=zT01[:, t * 128 : (t + 1) * 128],
                rhs=wo_a[:, n2 * 384 : (n2 + 1) * 384],
                start=True,
                stop=False,
            )
            nc.tensor.matmul(
                po[:, 0:384],
                lhsT=zT2[:, t * 128 : (t + 1) * 128],
                rhs=wo_b[:, n2 * 384 : (n2 + 1) * 384],
                start=False,
                stop=True,
            )
            eng = evict_eng
            if eng is None:
                eng = "dve"
            if eng == "act":
                nc.scalar.activation(
                    out=ob[:, n2 * 384 : (n2 + 1) * 384], in_=po[:, 0:384], func=AF.Copy
                )
            else:
                nc.vector.tensor_copy(
                    out=ob[:, n2 * 384 : (n2 + 1) * 384], in_=po[:, 0:384]
                )
            if n2 == 1:
                del o_done[key]
                (nc.sync if (t % 2 == 0 or tail) else nc.gpsimd).dma_start(
                    out=out_t[t], in_=ob[:, :]
                )

        # background work queue of (key, fn), drained between attention
        # iterations. Queue order is topological, so force-draining "through
        # the last needed unit" preserves all producer->consumer ordering.
        work = deque()

        def q_proj(n, mis):
            for mi in mis:
                for kpair in range(3):
                    work.append(
                        (("proj", n, mi), lambda mi=mi, n=n, kp=kpair: proj_unit(mi, n, kp))
                    )

        def q_v(ts):
            for t in ts:
                for kpair in range(3):
                    work.append((("v", t), lambda t=t, kp=kpair: v_unit(t, kp)))

        def drain(k=1):
            for _ in range(k):
                if work:
                    work.popleft()[1]()

        def drain_all():
            while work:
                work.popleft()[1]()

        # head h's scores need these packed q/k groups
        PROJ_GROUPS_FOR_HEAD = {0: (0, 1), 1: (0, 1), 2: (2,)}

        def force_drain_for(h, qt):
            """Emit queued units up to the last one attention(h, qt) depends on."""
            needed = set()
            for n in range(qt + 1):
                for mi in PROJ_GROUPS_FOR_HEAD[h]:
                    needed.add(("proj", n, mi))
            for t in range(4 * qt + 4):
                needed.add(("v", t))
            last = -1
            for i, (key, _) in enumerate(work):
                if key in needed:
                    last = i
            for _ in range(last + 1):
                work.popleft()[1]()

        def qh(h):
            col, off = [(0, 0), (0, 64), (1, 0)][h]
            return qT_sb[off : off + 64, col, :]

        def kh(h):
            col, off = [(0, 0), (0, 64), (1, 0)][h]
            return kT_sb[off : off + 64, col, :]

        zdst = [zT01[0:64, :], zT01[64:128, :], zT2[0:64, :]]

        # PV matmuls are pipelined a few pairs behind their exp across block
        # boundaries, so the in-order PE FIFO never waits on the exp/mask
        # chain, not even at the end of a block.
        pvq = deque()  # (block_serial, pv_closure)
        blk_serial = [0]

        def pv_drain(depth):
            while len(pvq) > depth:
                pvq.popleft()[1]()

        def pv_flush(upto_serial):
            while pvq and pvq[0][0] <= upto_serial:
                pvq.popleft()[1]()

        def attention(h, qt, per_pair, last=False):
            """scores^T -> exp -> causal mask -> PV into zp. Score matmuls for
            kt pairs land in one 2-bank psum tile so a single exp covers both;
            start=True zeroes the whole bank, so the unwritten low columns of
            narrowed diagonal halves exp to 1.0 and are never consumed."""
            zp = psz.tile([DH + 1, QT], F32)
            nkt = 4 * qt + 4
            blk = blk_serial[0]
            blk_serial[0] += 1

            def pv(kt, es_ap, lo):
                nc.tensor.matmul(
                    zp[:, lo:QT],
                    lhsT=v_aug[:, kt, h, :],
                    rhs=es_ap[:, lo:QT],
                    start=(kt == 0),
                    stop=(kt == nkt - 1),
                )

            def pv_q(kt, es_ap, lo):
                # last block: quarter q of zp sees its final write at
                # kt = nkt-4+q, so it can be normalized immediately after.
                # Only that closing kt is split off (multiple start=True
                # writes into one bank would re-zero siblings' columns).
                r = kt - (nkt - 4)
                if r < 0:
                    nc.tensor.matmul(
                        zp[:, 0:QT],
                        lhsT=v_aug[:, kt, h, :],
                        rhs=es_ap[:, 0:QT],
                        start=(kt == 0),
                        stop=False,
                        skip_group_check=True,
                    )
                else:
                    c0 = 128 * r
                    nc.tensor.matmul(
                        zp[:, c0 : c0 + 128],
                        lhsT=v_aug[:, kt, h, :],
                        rhs=es_ap[:, c0 : c0 + 128],
                        start=False,
                        stop=True,
                        skip_group_check=True,
                    )
                    if c0 + 128 < QT:
                        nc.tensor.matmul(
                            zp[:, c0 + 128 : QT],
                            lhsT=v_aug[:, kt, h, :],
                            rhs=es_ap[:, c0 + 128 : QT],
                            start=False,
                            stop=(kt == nkt - 1),
                            skip_group_check=True,
                        )
                    stairs(r)

            st_rec, st_bc = {}, {}

            def qcols(q):
                return slice(128 * q, 128 * (q + 1))

            def stairs(step):
                # stage skew keeps the in-order PE FIFO free of long waits:
                # each quarter's PE pieces (bc, o-proj) are emitted one or two
                # pv-steps after the pv that closed the quarter's psum group.
                if step >= 1:
                    st_bc[step - 1] = norm_bc(st_rec[step - 1], qcols(step - 1), bc_ps2=True)
                st_rec[step] = norm_recip(zp, qcols(step))
                if step >= 1:
                    norm_post(zp, h, qt, st_bc[step - 1], qcols(step - 1), eng="act")
                if step >= 2:
                    t = 4 * qt + step - 2
                    o_proj_unit(t, 0, evict_eng="dve", tail=True)
                    o_proj_unit(t, 1, evict_eng="act", tail=True)

            def stairs_final():
                st_bc[3] = norm_bc(st_rec[3], qcols(3), bc_ps2=True)
                norm_post(zp, h, qt, st_bc[3], qcols(3), eng="act")
                for t in (4 * qt + 2, 4 * qt + 3):
                    o_proj_unit(t, 0, evict_eng="dve", tail=True)
                    o_proj_unit(t, 1, evict_eng="act", tail=True)

            for kp in range(nkt // 2):
                kt0 = 2 * kp
                rr0 = kt0 - 4 * qt
                lo_pair = 128 * rr0 if rr0 > 0 else 0
                sp = ps2.tile([128, 2, QT], F32, tag="ps2")
                los = []
                for i in (0, 1):
                    kt = kt0 + i
                    rr = kt - 4 * qt
                    lo = 128 * rr if rr > 0 else 0
                    los.append(lo)
                    nc.tensor.matmul(
                        sp[:, i, lo:QT],
                        lhsT=kh(h)[:, kt * 128 : (kt + 1) * 128],
                        rhs=qh(h)[:, qt * QT + lo : (qt + 1) * QT],
                        start=True,
                        stop=True,
                    )
                es = expp.tile([128, 2, QT], F16, tag="expp")
                nc.scalar.activation(
                    out=es[:, :, lo_pair:QT], in_=sp[:, :, lo_pair:QT], func=AF.Exp
                )
                for i in (0, 1):
                    kt = kt0 + i
                    rr = kt - 4 * qt
                    lo = los[i]
                    if rr >= 0:  # diagonal square: zero where key > query
                        hi = min(lo + 128, QT)
                        nc.gpsimd.affine_select(
                            out=es[:, i, lo:hi],
                            in_=es[:, i, lo:hi],
                            compare_op=mybir.AluOpType.is_ge,
                            fill=0.0,
                            base=0,
                            channel_multiplier=-1,
                            pattern=[[1, hi - lo]],
                        )
                    fn = pv_q if last else pv
                    pvq.append(
                        (blk, lambda kt=kt, es=es, i=i, lo=lo, fn=fn: fn(kt, es[:, i], lo))
                    )
                drain(per_pair)
                pv_drain(2 if last else 7)
            if last:
                pv_flush(blk)
                stairs_final()
            return zp, blk

        def norm_recip(zp, cols):
            rec = small.tile([1, QT], F16, tag="rec")
            with nc.allow_low_precision(reason="fp16 normalize"):
                nc.vector.reciprocal(rec[:, cols], zp[DH : DH + 1, cols])
            return rec

        def norm_bc(rec, cols, bc_ps2=False):
            if bc_ps2:
                bc2 = ps2.tile([128, 2, QT], F32, tag="ps2", name="bc2")
                bc = bc2[:, 0]
            else:
                bc = ps.tile([128, QT], F32, tag="ps")
            nc.tensor.matmul(
                bc[0:64, cols], lhsT=ones64[:], rhs=rec[:, cols], start=True, stop=True
            )
            return bc

        def norm_post(zp, h, qt, bc, cols, eng="act"):
            bc_sb = small.tile([64, QT], F16, tag="bcsb")
            if eng == "dve":
                nc.vector.tensor_copy(out=bc_sb[:, cols], in_=bc[0:64, cols])
            else:
                nc.scalar.activation(out=bc_sb[:, cols], in_=bc[0:64, cols], func=AF.Copy)
            nc.vector.tensor_mul(
                zdst[h][:, qt * QT : (qt + 1) * QT][:, cols],
                zp[0:DH, cols],
                bc_sb[:, cols],
            )

        def normalize(zp, h, qt, cols=slice(0, QT)):
            rec = norm_recip(zp, cols)
            bc = norm_bc(rec, cols)
            norm_post(zp, h, qt, bc, cols, eng="act")

        # ---- schedule ----
        # prologue: only what attention(h0, qt0) needs; the rest queues up.
        for mi in (0, 1):
            for kpair in range(3):
                proj_unit(mi, 0, kpair)
        for t in range(4):
            for kpair in range(3):
                v_unit(t, kpair)
        q_proj(0, mis=(2,))
        for n in range(1, NQT):
            q_proj(n, mis=(0, 1))
            q_v(range(4 * n, 4 * n + 2))
            q_proj(n, mis=(2,))
            q_v(range(4 * n + 2, 4 * n + 4))

        pending = None
        for qt in range(NQT):
            per_pair = [4, 1, 1, 2][qt]
            for h in range(HPC):
                force_drain_for(h, qt)
                is_last = qt == NQT - 1 and h == HPC - 1
                if is_last:
                    # everything queued must land before the staircase epilogue
                    pv_flush(pending[3])
                    normalize(*pending[:3])
                    drain_all()
                    pending = None
                zp, blk = attention(h, qt, per_pair, last=is_last)
                if pending is not None:
                    pv_flush(pending[3])  # pending block's PV accumulation done
                    normalize(*pending[:3])
                    ph, pqt = pending[1], pending[2]
                    if ph == HPC - 1:  # whole q-tile normalized -> O-proj ready
                        for t in range(4 * pqt, 4 * pqt + 4):
                            for n2 in range(2):
                                work.append(
                                    (("o", pqt), lambda t=t, n2=n2: o_proj_unit(t, n2))
                                )
                if not is_last:
                    pending = (zp, h, qt, blk)
    nc.finalize()
    return nc


_NC_CACHE = {}


def make_in_maps(x, W_qkv, b_qkv, W_o):
    in_maps = []
    for c in range(8):
        b, g = divmod(c, 4)
        hs = [HPC * g + i for i in range(HPC)]
        qr = [np.arange(64 * h, 64 * h + 64) for h in hs]
        w_q = [W_qkv[i] * 0.125 for i in qr]
        w_k = [W_qkv[768 + i] for i in qr]
        w_v = [W_qkv[1536 + i] for i in qr]
        b_q = [b_qkv[i] * 0.125 for i in qr]
        b_k = [b_qkv[768 + i] for i in qr]
        # packed rows: m0=[q0 q1] m1=[k0 k1] m2=[q2 k2]
        wpk = np.concatenate(
            [w_q[0], w_q[1], w_k[0], w_k[1], w_q[2], w_k[2]], axis=0
        )
        wv = np.concatenate([w_v[0], w_v[1], w_v[2]], axis=0)
        bqk_col = np.zeros((128, 3), np.float32)
        bqk_col[:, 0] = np.concatenate([b_q[0], b_q[1]])
        bqk_col[:, 1] = np.concatenate([b_k[0], b_k[1]])
        bqk_col[:, 2] = np.concatenate([b_q[2], b_k[2]])
        in_maps.append(
            {
                "xT": np.ascontiguousarray(x[b].T.astype(np.float16)),
                "wpk": np.ascontiguousarray(wpk.T.astype(np.float16)),
                "wv": np.ascontiguousarray(wv.T.astype(np.float16)),
                "woT": np.ascontiguousarray(
                    W_o[:, GD * g : GD * (g + 1)].T.astype(np.float16)
                ),
                "bqk": bqk_col,
                "vones": np.ones((128, 64), np.float16),
            }
        )
    return in_maps


def make_in_maps_for_test(inputs):
    return make_in_maps(
        np.asarray(inputs["x"], np.float32),
        np.asarray(inputs["W_qkv"], np.float32),
        np.asarray(inputs["b_qkv"], np.float32),
        np.asarray(inputs["W_o"], np.float32),
    )


def kernel(x, W_qkv, b_qkv, W_o, b_o):
    x = np.asarray(x, np.float32)
    W_qkv = np.asarray(W_qkv, np.float32)
    b_qkv = np.asarray(b_qkv, np.float32)
    W_o = np.asarray(W_o, np.float32)
    b_o = np.asarray(b_o, np.float32)

    if "nc" not in _NC_CACHE:
        _NC_CACHE["nc"] = build_bass()
    nc = _NC_CACHE["nc"]

    in_maps = make_in_maps(x, W_qkv, b_qkv, W_o)

    res = run_bass_kernel_spmd(
        nc,
        in_maps,
        list(range(8)),
        trace=bool(int(os.environ.get("KERNEL_TRACE", "0"))),
    )
    _NC_CACHE["last_results"] = res

    out = np.zeros((B, S, D), np.float32)
    for c in range(8):
        out[c // 4] += res.results[c]["out_p"].astype(np.float32)
    out += b_qkv[1536:] @ W_o.T + b_o
    return out


# revision 23
# speedup vs baseline: 1.1669x; 1.0304x over previous
"""Causal multi-head attention block (B=2, S=2048, D=768, H=12) on 8 trn2 cores.

Sharding: core c -> batch b = c//4 (data parallel), head group g = c%4
(tensor parallel, 3 heads per group). Each core computes its group's QKV
projection, causal attention, and a partial O-projection over its 192
z-columns. Host sums the 4 partials per batch and adds the biases that
commute through the math (v-bias and b_o).

On-core layout: xT [768, 2048] (d on partitions) so q^T/k^T come straight
out of the projection with head dims on partitions; V is projected
separately in [keys, dh] layout (lhsT = xT key-block, rhs = W_v^T columns)
so the PV matmul needs no transposes at all. A ones-column appended to V
yields the softmax denominator for free.

All matmul operands are fp16 (1 col/cycle on the PE, 10-bit mantissa keeps
rel-err ~5e-4); psum stays f32. Score matmuls for kt pairs land in one
2-bank psum tile so a single exp instruction covers both; the causal mask
only touches the 128-wide diagonal square.

The packed q/k projection weight is host-repacked so all three 128-wide
M-groups are full:  m0=[q_h0 q_h1]  m1=[q_h2 k_h0]  m2=[k_h1 k_h2]
(q rows pre-scaled by 1/8; v bias folded into the host-side epilogue).
"""

import os
from collections import deque
from contextlib import ExitStack

import numpy as np

import concourse.tile as tile
from concourse import bacc, mybir
from concourse.bass_utils import run_bass_kernel_spmd

F32 = mybir.dt.float32
F16 = mybir.dt.float16
F8 = mybir.dt.float8e4
DR = mybir.MatmulPerfMode.DoubleRow
AF = mybir.ActivationFunctionType

B, S, D = 2, 2048, 768
NH, DH = 12, 64
HPC = 3            # heads per core
GD = HPC * DH      # 192 z-cols per core
KT, QT = 128, 512  # key tile (partitions), q tile (psum free)
NKT, NQT = S // KT, S // QT   # 16, 4
NKD = D // 128     # 6 contraction tiles for the projections
WPK = 2 * GD       # 384 packed q/k projection rows


def build_bass():
    nc = bacc.Bacc(None)
    xT16 = nc.dram_tensor("xT16", [D, QT], F16, kind="ExternalInput")
    wpk16 = nc.dram_tensor("wpk16", [D, WPK], F16, kind="ExternalInput")
    wv16 = nc.dram_tensor("wv16", [D, GD], F16, kind="ExternalInput")
    x8 = nc.dram_tensor("x8", [D, S], F8, kind="ExternalInput")
    xr = nc.dram_tensor("xr", [D, S], F8, kind="ExternalInput")
    wpk8 = nc.dram_tensor("wpk8", [D, WPK], F8, kind="ExternalInput")
    wpkr = nc.dram_tensor("wpkr", [D, WPK], F8, kind="ExternalInput")
    wv8 = nc.dram_tensor("wv8", [D, GD], F8, kind="ExternalInput")
    wvr = nc.dram_tensor("wvr", [D, GD], F8, kind="ExternalInput")
    woT = nc.dram_tensor("woT", [GD, D], F16, kind="ExternalInput")
    bqk = nc.dram_tensor("bqk", [128, 3], F32, kind="ExternalInput")
    vones = nc.dram_tensor("vones", [128, 64], F16, kind="ExternalInput")
    out_p = nc.dram_tensor("out_p", [S, D], F16, kind="ExternalOutput")

    with tile.TileContext(nc) as tc, ExitStack() as ctx:
        const = ctx.enter_context(tc.tile_pool(name="const", bufs=1))
        ps = ctx.enter_context(tc.tile_pool(name="ps", bufs=2, space="PSUM"))
        ps2 = ctx.enter_context(tc.tile_pool(name="ps2", bufs=2, space="PSUM"))
        psz = ctx.enter_context(tc.tile_pool(name="psz", bufs=2, space="PSUM"))
        expp = ctx.enter_context(tc.tile_pool(name="expp", bufs=9))
        small = ctx.enter_context(tc.tile_pool(name="small", bufs=4))

        xT16_sb = const.tile([128, NKD, QT], F16)
        wpk16_sb = const.tile([128, NKD, WPK], F16)
        wv16_sb = const.tile([128, NKD, GD], F16)
        x8_sb = const.tile([128, 3, 2, S], F8)
        xr_sb = const.tile([128, 3, 2, S], F8)
        wpk8_sb = const.tile([128, 3, 2, WPK], F8)
        wpkr_sb = const.tile([128, 3, 2, WPK], F8)
        wv8_sb = const.tile([128, 3, 2, GD], F8)
        wvr_sb = const.tile([128, 3, 2, GD], F8)
        wo_a = const.tile([128, D], F16)
        wo_b = const.tile([64, D], F16)
        bqk_sb = const.tile([128, 3], F32)
        qT_sb = const.tile([128, 2, S], F16)
        kT_sb = const.tile([128, 2, S], F16)
        v_aug = const.tile([128, NKT, HPC, DH + 1], F16)
        zT01 = const.tile([128, S], F16)
        zT2 = const.tile([64, S], F16)
        ones64 = const.tile([1, 64], F16)
        ones_stage = const.tile([128, 64], F16)

        # ---- loads: k-interleaved so the first projection k-pairs unblock
        # early. Early DMAs fan out over four queues (SP/Act/DVE/Pool) to
        # dodge the ~650ns per-DMA dispatch serialization on a single queue.
        # batched multi-tile DMAs keep the dispatch count low (each dispatch
        # costs ~650ns of queue time). The projection runs its main fp8 pass
        # first, so the x8/w8 arrays go first and the residual arrays follow.
        def tri(dram):
            return dram.rearrange("(kp two p) s -> p kp two s", p=128, two=2)

        x8_a, xr_a = tri(x8), tri(xr)
        wp8_a, wpr_a = tri(wpk8), tri(wpkr)
        wv8_a, wvr_a = tri(wv8), tri(wvr)
        xT16_a = xT16.rearrange("(t p) s -> p t s", p=128)
        wpk16_a = wpk16.rearrange("(t p) m -> p t m", p=128)
        wv16_a = wv16.rearrange("(t p) m -> p t m", p=128)
        # fp16 head (first q-tile's projection inputs) loads first and small;
        # the fp8 triples only cover columns QT: onward.
        nc.sync.dma_start(out=wpk16_sb[:, :, 0:128], in_=wpk16_a[:, :, 0:128])
        nc.scalar.dma_start(out=xT16_sb[:, 0:3, :], in_=xT16_a[:, 0:3, :])
        nc.sync.dma_start(out=xT16_sb[:, 3:6, :], in_=xT16_a[:, 3:6, :])
        nc.scalar.dma_start(out=bqk_sb[:], in_=bqk[:, :])
        nc.sync.dma_start(out=wpk16_sb[:, :, 128:WPK], in_=wpk16_a[:, :, 128:WPK])
        nc.scalar.dma_start(out=wv16_sb[:], in_=wv16_a[:])
        nc.gpsimd.dma_start(out=ones_stage[:], in_=vones[:, :])
        nc.gpsimd.dma_start(out=ones64[:], in_=vones[0:1, 0:64])
        nc.vector.tensor_copy(
            out=v_aug[:, :, :, DH],
            in_=ones_stage[:, 0 : NKT * HPC].rearrange("p (t h) -> p t h", t=NKT),
        )
        # fp8 arrives in column-chunk order matching when each q-tile's
        # projection runs: weights + [QT:2QT] first, the rest behind.
        nc.scalar.dma_start(out=wpk8_sb[:], in_=wp8_a[:])
        nc.gpsimd.dma_start(out=wv8_sb[:], in_=wv8_a[:])
        nc.gpsimd.dma_start(out=wvr_sb[:], in_=wvr_a[:])
        nc.sync.dma_start(out=wpkr_sb[:], in_=wpr_a[:])
        nc.scalar.dma_start(
            out=x8_sb[:, :, :, QT : 2 * QT], in_=x8_a[:, :, :, QT : 2 * QT]
        )
        nc.sync.dma_start(
            out=xr_sb[:, :, :, QT : 2 * QT], in_=xr_a[:, :, :, QT : 2 * QT]
        )
        nc.scalar.dma_start(
            out=x8_sb[:, :, :, 2 * QT : S], in_=x8_a[:, :, :, 2 * QT : S]
        )
        nc.sync.dma_start(
            out=xr_sb[:, :, :, 2 * QT : S], in_=xr_a[:, :, :, 2 * QT : S]
        )
        nc.sync.dma_start(out=wo_a[:], in_=woT[0:128, :])
        nc.gpsimd.dma_start(out=wo_b[:], in_=woT[128:GD, :])

        # packed q/k projection m-groups: (col0, evict spec). m2 holds
        # [q2 k2]; its k2 rows evict to kT partitions 0:64 (cross-base copy)
        # so every head's q and k share a partition base for the PE.
        mgroups = [
            (0, [((0, 128), lambda n: qT_sb[0:128, 0, n * QT : (n + 1) * QT], 0)]),
            (128, [((0, 128), lambda n: kT_sb[0:128, 0, n * QT : (n + 1) * QT], 1)]),
            (256, [
                ((0, 64), lambda n: qT_sb[0:64, 1, n * QT : (n + 1) * QT], 2),
                ((64, 128), lambda n: kT_sb[0:64, 1, n * QT : (n + 1) * QT], 2),
            ]),
        ]

        proj_psums = {}
        PROJ_TERMS = [(wpk8_sb, x8_sb), (wpk8_sb, xr_sb), (wpkr_sb, x8_sb)]
        DESCALE = 1.0 / 1024.0  # host pre-scales x and W by 32 for fp8 range

        def proj_unit(mi, n, term):
            """One fp8 DoubleRow pass (3 k-pairs) of group (mi, n): term 0 is
            the w8.x8 main product, terms 1/2 add the x and W residuals."""
            c0, evicts = mgroups[mi]
            key = (mi, n)
            if key not in proj_psums:
                proj_psums[key] = ps.tile([128, QT], F32, tag="ps", name="projp")
            p = proj_psums[key]
            if n == 0:  # fp16 head: the small early arrays, no descale
                for k in (2 * term, 2 * term + 1):
                    nc.tensor.matmul(
                        p[:, :],
                        lhsT=wpk16_sb[:, k, c0 : c0 + 128],
                        rhs=xT16_sb[:, k, :],
                        start=(k == 0),
                        stop=(k == NKD - 1),
                    )
            else:
                wsb, xsb = PROJ_TERMS[term]
                for kp in range(3):
                    nc.tensor.matmul(
                        p[:, :],
                        lhsT=wsb[:, kp, :, c0 : c0 + 128],
                        rhs=xsb[:, kp, :, n * QT : (n + 1) * QT],
                        start=(term == 0 and kp == 0),
                        stop=(term == 2 and kp == 2),
                        perf_mode=DR,
                    )
            if term == 2:
                del proj_psums[key]
                for (r0, r1), dst, bcol in evicts:
                    if n == 0:
                        nc.vector.tensor_scalar_add(
                            out=dst(n),
                            in0=p[r0:r1, :],
                            scalar1=bqk_sb[r0:r1, bcol : bcol + 1],
                        )
                    else:
                        nc.vector.tensor_scalar(
                            out=dst(n),
                            in0=p[r0:r1, :],
                            scalar1=DESCALE,
                            scalar2=bqk_sb[r0:r1, bcol : bcol + 1],
                            op0=mybir.AluOpType.mult,
                            op1=mybir.AluOpType.add,
                        )

        vp_psums = {}
        V_TERMS = [(x8_sb, wv8_sb), (xr_sb, wv8_sb), (x8_sb, wvr_sb)]

        def v_unit(t, term):
            """V projection for keys-tile t (fp8 DoubleRow pass `term`):
            [128 keys, 192] psum; evict into v_aug[:, t, :, 0:64]."""
            if t not in vp_psums:
                vp_psums[t] = ps.tile([128, QT], F32, tag="ps", name="vp")
            p = vp_psums[t]
            if t < 4:  # fp16 head
                for k in (2 * term, 2 * term + 1):
                    nc.tensor.matmul(
                        p[:, 0:GD],
                        lhsT=xT16_sb[:, k, t * 128 : (t + 1) * 128],
                        rhs=wv16_sb[:, k, :],
                        start=(k == 0),
                        stop=(k == NKD - 1),
                    )
            else:
                xsb, wsb = V_TERMS[term]
                for kp in range(3):
                    nc.tensor.matmul(
                        p[:, 0:GD],
                        lhsT=xsb[:, kp, :, t * 128 : (t + 1) * 128],
                        rhs=wsb[:, kp, :, :],
                        start=(term == 0 and kp == 0),
                        stop=(term == 2 and kp == 2),
                        perf_mode=DR,
                    )
            if term == 2:
                del vp_psums[t]
                if t < 4:
                    nc.vector.tensor_copy(
                        out=v_aug[:, t, :, 0:DH],
                        in_=p[:, 0:GD].rearrange("p (h d) -> p h d", h=HPC),
                    )
                else:
                    nc.vector.tensor_scalar_mul(
                        out=v_aug[:, t, :, 0:DH],
                        in0=p[:, 0:GD].rearrange("p (h d) -> p h d", h=HPC),
                        scalar1=DESCALE,
                    )

        out_t = out_p.rearrange("(tp p) d -> tp p d", p=128)
        o_done = {}

        def o_proj_unit(t, n2, evict_eng=None, tail=False):
            key = t
            if key not in o_done:
                o_done[key] = expp.tile([128, D], F16, tag="osb", name="osb", bufs=3)
            ob = o_done[key]
            po = ps.tile([128, QT], F32, tag="ps")
            nc.tensor.matmul(
                po[:, 0:384],
                lhsT=zT01[:, t * 128 : (t + 1) * 128],
                rhs=wo_a[:, n2 * 384 : (n2 + 1) * 384],
                start=True,
                stop=False,
            )
            nc.tensor.matmul(
                po[:, 0:384],
                lhsT=zT2[:, t * 128 : (t + 1) * 128],
                rhs=wo_b[:, n2 * 384 : (n2 + 1) * 384],
                start=False,
                stop=True,
            )
            eng = evict_eng
            if eng is None:
                eng = "dve"
            if eng == "act":
                nc.scalar.activation(
                    out=ob[:, n2 * 384 : (n2 + 1) * 384], in_=po[:, 0:384], func=AF.Copy
                )
            else:
                nc.vector.tensor_copy(
                    out=ob[:, n2 * 384 : (n2 + 1) * 384], in_=po[:, 0:384]
                )
            if n2 == 1:
                del o_done[key]
                (nc.sync if (t % 2 == 0 or tail) else nc.scalar).dma_start(
                    out=out_t[t], in_=ob[:, :]
                )

        # background work queue of (key, fn), drained between attention
        # iterations. Queue order is topological, so force-draining "through
        # the last needed unit" preserves all producer->consumer ordering.
        work = deque()

        def q_proj(n, mis):
            for mi in mis:
                for term in range(3):
                    work.append(
                        (("proj", n, mi), lambda mi=mi, n=n, tm=term: proj_unit(mi, n, tm))
                    )

        def q_v(ts):
            for t in ts:
                for term in range(3):
                    work.append((("v", t), lambda t=t, tm=term: v_unit(t, tm)))

        def drain(k=1):
            for _ in range(k):
                if work:
                    work.popleft()[1]()

        def drain_all():
            while work:
                work.popleft()[1]()

        # head h's scores need these packed q/k groups
        PROJ_GROUPS_FOR_HEAD = {0: (0, 1), 1: (0, 1), 2: (2,)}

        def force_drain_for(h, qt):
            """Emit queued units up to the last one attention(h, qt) depends on."""
            needed = set()
            for n in range(qt + 1):
                for mi in PROJ_GROUPS_FOR_HEAD[h]:
                    needed.add(("proj", n, mi))
            for t in range(4 * qt + 4):
                needed.add(("v", t))
            last = -1
            for i, (key, _) in enumerate(work):
                if key in needed:
                    last = i
            for _ in range(last + 1):
                work.popleft()[1]()

        def qh(h):
            col, off = [(0, 0), (0, 64), (1, 0)][h]
            return qT_sb[off : off + 64, col, :]

        def kh(h):
            col, off = [(0, 0), (0, 64), (1, 0)][h]
            return kT_sb[off : off + 64, col, :]

        zdst = [zT01[0:64, :], zT01[64:128, :], zT2[0:64, :]]

        # PV matmuls are pipelined a few pairs behind their exp across block
        # boundaries, so the in-order PE FIFO never waits on the exp/mask
        # chain, not even at the end of a block.
        pvq = deque()  # (block_serial, pv_closure)
        blk_serial = [0]

        def pv_drain(depth):
            while len(pvq) > depth:
                pvq.popleft()[1]()

        def pv_flush(upto_serial):
            while pvq and pvq[0][0] <= upto_serial:
                pvq.popleft()[1]()

        def attention(h, qt, per_pair, last=False):
            """scores^T -> exp -> causal mask -> PV into zp. Score matmuls for
            kt pairs land in one 2-bank psum tile so a single exp covers both;
            start=True zeroes the whole bank, so the unwritten low columns of
            narrowed diagonal halves exp to 1.0 and are never consumed."""
            zp = psz.tile([DH + 1, QT], F32)
            nkt = 4 * qt + 4
            blk = blk_serial[0]
            blk_serial[0] += 1

            def pv(kt, es_ap, lo):
                nc.tensor.matmul(
                    zp[:, lo:QT],
                    lhsT=v_aug[:, kt, h, :],
                    rhs=es_ap[:, lo:QT],
                    start=(kt == 0),
                    stop=(kt == nkt - 1),
                )

            def pv_q(kt, es_ap, lo):
                # last block: quarter q of zp sees its final write at
                # kt = nkt-4+q, so it can be normalized immediately after.
                # Only that closing kt is split off (multiple start=True
                # writes into one bank would re-zero siblings' columns).
                r = kt - (nkt - 4)
                if r < 0:
                    nc.tensor.matmul(
                        zp[:, 0:QT],
                        lhsT=v_aug[:, kt, h, :],
                        rhs=es_ap[:, 0:QT],
                        start=(kt == 0),
                        stop=False,
                        skip_group_check=True,
                    )
                else:
                    c0 = 128 * r
                    nc.tensor.matmul(
                        zp[:, c0 : c0 + 128],
                        lhsT=v_aug[:, kt, h, :],
                        rhs=es_ap[:, c0 : c0 + 128],
                        start=False,
                        stop=True,
                        skip_group_check=True,
                    )
                    if c0 + 128 < QT:
                        nc.tensor.matmul(
                            zp[:, c0 + 128 : QT],
                            lhsT=v_aug[:, kt, h, :],
                            rhs=es_ap[:, c0 + 128 : QT],
                            start=False,
                            stop=(kt == nkt - 1),
                            skip_group_check=True,
                        )
                    stairs(r)

            st_rec, st_bc = {}, {}

            def qcols(q):
                return slice(128 * q, 128 * (q + 1))

            def stairs(step):
                # stage skew keeps the in-order PE FIFO free of long waits:
                # each quarter's PE pieces (bc, o-proj) are emitted one or two
                # pv-steps after the pv that closed the quarter's psum group.
                if step >= 1:
                    st_bc[step - 1] = norm_bc(st_rec[step - 1], qcols(step - 1), bc_ps2=True)
                st_rec[step] = norm_recip(zp, qcols(step))
                if step >= 1:
                    norm_post(zp, h, qt, st_bc[step - 1], qcols(step - 1), eng="act")
                if step >= 2:
                    t = 4 * qt + step - 2
                    o_proj_unit(t, 0, evict_eng="dve", tail=True)
                    o_proj_unit(t, 1, evict_eng="act", tail=True)

            def stairs_final():
                st_bc[3] = norm_bc(st_rec[3], qcols(3), bc_ps2=True)
                norm_post(zp, h, qt, st_bc[3], qcols(3), eng="act")
                for t in (4 * qt + 2, 4 * qt + 3):
                    o_proj_unit(t, 0, evict_eng="dve", tail=True)
                    o_proj_unit(t, 1, evict_eng="act", tail=True)

            for kp in range(nkt // 2):
                kt0 = 2 * kp
                rr0 = kt0 - 4 * qt
                lo_pair = 128 * rr0 if rr0 > 0 else 0
                sp = ps2.tile([128, 2, QT], F32, tag="ps2")
                los = []
                for i in (0, 1):
                    kt = kt0 + i
                    rr = kt - 4 * qt
                    lo = 128 * rr if rr > 0 else 0
                    los.append(lo)
                    nc.tensor.matmul(
                        sp[:, i, lo:QT],
                        lhsT=kh(h)[:, kt * 128 : (kt + 1) * 128],
                        rhs=qh(h)[:, qt * QT + lo : (qt + 1) * QT],
                        start=True,
                        stop=True,
                    )
                es = expp.tile([128, 2, QT], F16, tag="expp")
                nc.scalar.activation(
                    out=es[:, :, lo_pair:QT], in_=sp[:, :, lo_pair:QT], func=AF.Exp
                )
                for i in (0, 1):
                    kt = kt0 + i
                    rr = kt - 4 * qt
                    lo = los[i]
                    if rr >= 0:  # diagonal square: zero where key > query
                        hi = min(lo + 128, QT)
                        nc.gpsimd.affine_select(
                            out=es[:, i, lo:hi],
                            in_=es[:, i, lo:hi],
                            compare_op=mybir.AluOpType.is_ge,
                            fill=0.0,
                            base=0,
                            channel_multiplier=-1,
                            pattern=[[1, hi - lo]],
                        )
                    fn = pv_q if last else pv
                    pvq.append(
                        (blk, lambda kt=kt, es=es, i=i, lo=lo, fn=fn: fn(kt, es[:, i], lo))
                    )
                drain(per_pair)
                pv_drain(2 if last else 7)
            if last:
                pv_flush(blk)
                stairs_final()
            return zp, blk

        def norm_recip(zp, cols):
            rec = small.tile([1, QT], F16, tag="rec")
            with nc.allow_low_precision(reason="fp16 normalize"):
                nc.vector.reciprocal(rec[:, cols], zp[DH : DH + 1, cols])
            return rec

        def norm_bc(rec, cols, bc_ps2=False):
            if bc_ps2:
                bc2 = ps2.tile([128, 2, QT], F32, tag="ps2", name="bc2")
                bc = bc2[:, 0]
            else:
                bc = ps.tile([128, QT], F32, tag="ps")
            nc.tensor.matmul(
                bc[0:64, cols], lhsT=ones64[:], rhs=rec[:, cols], start=True, stop=True
            )
            return bc

        def norm_post(zp, h, qt, bc, cols, eng="act"):
            bc_sb = small.tile([64, QT], F16, tag="bcsb")
            if eng == "dve":
                nc.vector.tensor_copy(out=bc_sb[:, cols], in_=bc[0:64, cols])
            else:
                nc.scalar.activation(out=bc_sb[:, cols], in_=bc[0:64, cols], func=AF.Copy)
            nc.vector.tensor_mul(
                zdst[h][:, qt * QT : (qt + 1) * QT][:, cols],
                zp[0:DH, cols],
                bc_sb[:, cols],
            )

        def normalize(zp, h, qt, cols=slice(0, QT)):
            rec = norm_recip(zp, cols)
            bc = norm_bc(rec, cols)
            norm_post(zp, h, qt, bc, cols, eng="act")

        # ---- schedule ----
        # prologue: only what attention(h0, qt0) needs; the rest queues up.
        for mi in (0, 1):
            for term in range(3):
                proj_unit(mi, 0, term)
        for t in range(4):
            for term in range(3):
                v_unit(t, term)
        q_proj(0, mis=(2,))
        for n in range(1, NQT):
            q_proj(n, mis=(0, 1))
            q_v(range(4 * n, 4 * n + 2))
            q_proj(n, mis=(2,))
            q_v(range(4 * n + 2, 4 * n + 4))

        pending = None
        for qt in range(NQT):
            per_pair = [4, 1, 1, 2][qt]
            for h in range(HPC):
                force_drain_for(h, qt)
                is_last = qt == NQT - 1 and h == HPC - 1
                if is_last:
                    # everything queued must land before the staircase epilogue
                    pv_flush(pending[3])
                    normalize(*pending[:3])
                    drain_all()
                    pending = None
                zp, blk = attention(h, qt, per_pair, last=is_last)
                if pending is not None:
                    pv_flush(pending[3])  # pending block's PV accumulation done
                    normalize(*pending[:3])
                    ph, pqt = pending[1], pending[2]
                    if ph == HPC - 1:  # whole q-tile normalized -> O-proj ready
                        for t in range(4 * pqt, 4 * pqt + 4):
                            for n2 in range(2):
                                work.append(
                                    (("o", pqt), lambda t=t, n2=n2: o_proj_unit(t, n2))
                                )
                if not is_last:
                    pending = (zp, h, qt, blk)
    nc.finalize()
    return nc


_NC_CACHE = {}


def _f8(a):
    """main fp8 + residual fp8 (inputs pre-scaled x32, so both normal-range)."""
    import ml_dtypes

    f8 = ml_dtypes.float8_e4m3
    a = np.ascontiguousarray(a, np.float32)
    m = a.astype(f8)
    r = (a - m.astype(np.float32)).astype(f8)
    return np.ascontiguousarray(m), np.ascontiguousarray(r)


def make_in_maps(x, W_qkv, b_qkv, W_o):
    in_maps = []
    for c in range(8):
        b, g = divmod(c, 4)
        hs = [HPC * g + i for i in range(HPC)]
        qr = [np.arange(64 * h, 64 * h + 64) for h in hs]
        w_q = [W_qkv[i] * 0.125 for i in qr]
        w_k = [W_qkv[768 + i] for i in qr]
        w_v = [W_qkv[1536 + i] for i in qr]
        b_q = [b_qkv[i] * 0.125 for i in qr]
        b_k = [b_qkv[768 + i] for i in qr]
        # packed rows: m0=[q0 q1] m1=[k0 k1] m2=[q2 k2]
        wpk = np.concatenate(
            [w_q[0], w_q[1], w_k[0], w_k[1], w_q[2], w_k[2]], axis=0
        )
        wv = np.concatenate([w_v[0], w_v[1], w_v[2]], axis=0)
        bqk_col = np.zeros((128, 3), np.float32)
        bqk_col[:, 0] = np.concatenate([b_q[0], b_q[1]])
        bqk_col[:, 1] = np.concatenate([b_k[0], b_k[1]])
        bqk_col[:, 2] = np.concatenate([b_q[2], b_k[2]])
        # fp8 triples (values pre-scaled by 32; 1/1024 folded into evictions)
        xm = _f8(32.0 * x[b].T)
        wpm = _f8(32.0 * wpk.T)
        wvm = _f8(32.0 * wv.T)
        in_maps.append(
            {
                "xT16": np.ascontiguousarray(x[b].T[:, 0:QT].astype(np.float16)),
                "wpk16": np.ascontiguousarray(wpk.T.astype(np.float16)),
                "wv16": np.ascontiguousarray(wv.T.astype(np.float16)),
                "x8": xm[0], "xr": xm[1],
                "wpk8": wpm[0], "wpkr": wpm[1],
                "wv8": wvm[0], "wvr": wvm[1],
                "woT": np.ascontiguousarray(
                    W_o[:, GD * g : GD * (g + 1)].T.astype(np.float16)
                ),
                "bqk": bqk_col,
                "vones": np.ones((128, 64), np.float16),
            }
        )
    return in_maps


def make_in_maps_for_test(inputs):
    return make_in_maps(
        np.asarray(inputs["x"], np.float32),
        np.asarray(inputs["W_qkv"], np.float32),
        np.asarray(inputs["b_qkv"], np.float32),
        np.asarray(inputs["W_o"], np.float32),
    )


def kernel(x, W_qkv, b_qkv, W_o, b_o):
    x = np.asarray(x, np.float32)
    W_qkv = np.asarray(W_qkv, np.float32)
    b_qkv = np.asarray(b_qkv, np.float32)
    W_o = np.asarray(W_o, np.float32)
    b_o = np.asarray(b_o, np.float32)

    if "nc" not in _NC_CACHE:
        _NC_CACHE["nc"] = build_bass()
    nc = _NC_CACHE["nc"]

    in_maps = make_in_maps(x, W_qkv, b_qkv, W_o)

    res = run_bass_kernel_spmd(
        nc,
        in_maps,
        list(range(8)),
        trace=bool(int(os.environ.get("KERNEL_TRACE", "0"))),
    )
    _NC_CACHE["last_results"] = res

    out = np.zeros((B, S, D), np.float32)
    for c in range(8):
        out[c // 4] += res.results[c]["out_p"].astype(np.float32)
    out += b_qkv[1536:] @ W_o.T + b_o
    return out


# revision 24
# speedup vs baseline: 1.2167x; 1.0427x over previous
"""Causal multi-head attention block (B=2, S=2048, D=768, H=12) on 8 trn2 cores.

Sharding: core c -> batch b = c//4 (data parallel), head group g = c%4
(tensor parallel, 3 heads per group). Each core computes its group's QKV
projection, causal attention, and a partial O-projection over its 192
z-columns. Host sums the 4 partials per batch and adds the biases that
commute through the math (v-bias and b_o).

On-core layout: xT [768, 2048] (d on partitions) so q^T/k^T come straight
out of the projection with head dims on partitions; V is projected
separately in [keys, dh] layout (lhsT = xT key-block, rhs = W_v^T columns)
so the PV matmul needs no transposes at all. A ones-column appended to V
yields the softmax denominator for free.

All matmul operands are fp16 (1 col/cycle on the PE, 10-bit mantissa keeps
rel-err ~5e-4); psum stays f32. Score matmuls for kt pairs land in one
2-bank psum tile so a single exp instruction covers both; the causal mask
only touches the 128-wide diagonal square.

The packed q/k projection weight is host-repacked so all three 128-wide
M-groups are full:  m0=[q_h0 q_h1]  m1=[q_h2 k_h0]  m2=[k_h1 k_h2]
(q rows pre-scaled by 1/8; v bias folded into the host-side epilogue).
"""

import os
from collections import deque
from contextlib import ExitStack

import numpy as np

import concourse.tile as tile
from concourse import bacc, mybir
from concourse.bass_utils import run_bass_kernel_spmd

F32 = mybir.dt.float32
F16 = mybir.dt.float16
F8 = mybir.dt.float8e4
DR = mybir.MatmulPerfMode.DoubleRow
AF = mybir.ActivationFunctionType

B, S, D = 2, 2048, 768
NH, DH = 12, 64
HPC = 3            # heads per core
GD = HPC * DH      # 192 z-cols per core
KT, QT = 128, 512  # key tile (partitions), q tile (psum free)
NKT, NQT = S // KT, S // QT   # 16, 4
NKD = D // 128     # 6 contraction tiles for the projections
WPK = 2 * GD       # 384 packed q/k projection rows


def build_bass():
    nc = bacc.Bacc(None)
    xT16 = nc.dram_tensor("xT16", [D, QT], F16, kind="ExternalInput")
    wpk16 = nc.dram_tensor("wpk16", [D, WPK], F16, kind="ExternalInput")
    wv16 = nc.dram_tensor("wv16", [D, GD], F16, kind="ExternalInput")
    x8 = nc.dram_tensor("x8", [D, S], F8, kind="ExternalInput")
    xr = nc.dram_tensor("xr", [D, S], F8, kind="ExternalInput")
    wpk8 = nc.dram_tensor("wpk8", [D, WPK], F8, kind="ExternalInput")
    wpkr = nc.dram_tensor("wpkr", [D, WPK], F8, kind="ExternalInput")
    wv8 = nc.dram_tensor("wv8", [D, GD], F8, kind="ExternalInput")
    wvr = nc.dram_tensor("wvr", [D, GD], F8, kind="ExternalInput")
    woT = nc.dram_tensor("woT", [GD, D], F16, kind="ExternalInput")
    bqk = nc.dram_tensor("bqk", [128, 3], F32, kind="ExternalInput")
    vones = nc.dram_tensor("vones", [128, 64], F16, kind="ExternalInput")
    out_p = nc.dram_tensor("out_p", [S, D], F16, kind="ExternalOutput")

    with tile.TileContext(nc) as tc, ExitStack() as ctx:
        const = ctx.enter_context(tc.tile_pool(name="const", bufs=1))
        ps = ctx.enter_context(tc.tile_pool(name="ps", bufs=2, space="PSUM"))
        ps2 = ctx.enter_context(tc.tile_pool(name="ps2", bufs=2, space="PSUM"))
        psz = ctx.enter_context(tc.tile_pool(name="psz", bufs=2, space="PSUM"))
        expp = ctx.enter_context(tc.tile_pool(name="expp", bufs=9))
        small = ctx.enter_context(tc.tile_pool(name="small", bufs=4))

        xT16_sb = const.tile([128, NKD, QT], F16)
        wpk16_sb = const.tile([128, NKD, WPK], F16)
        wv16_sb = const.tile([128, NKD, GD], F16)
        x8_sb = const.tile([128, 3, 2, S], F8)
        xr_sb = const.tile([128, 3, 2, S], F8)
        wpk8_sb = const.tile([128, 3, 2, WPK], F8)
        wpkr_sb = const.tile([128, 3, 2, WPK], F8)
        wv8_sb = const.tile([128, 3, 2, GD], F8)
        wvr_sb = const.tile([128, 3, 2, GD], F8)
        wo_a = const.tile([128, D], F16)
        wo_b = const.tile([64, D], F16)
        bqk_sb = const.tile([128, 3], F32)
        qT_sb = const.tile([128, 2, S], F16)
        kT_sb = const.tile([128, 2, S], F16)
        v_aug = const.tile([128, NKT, HPC, DH + 1], F16)
        zT01 = const.tile([128, S], F16)
        zT2 = const.tile([64, S], F16)
        ones64 = const.tile([1, 64], F16)
        ones_stage = const.tile([128, 64], F16)

        # ---- loads: k-interleaved so the first projection k-pairs unblock
        # early. Early DMAs fan out over four queues (SP/Act/DVE/Pool) to
        # dodge the ~650ns per-DMA dispatch serialization on a single queue.
        # batched multi-tile DMAs keep the dispatch count low (each dispatch
        # costs ~650ns of queue time). The projection runs its main fp8 pass
        # first, so the x8/w8 arrays go first and the residual arrays follow.
        def tri(dram):
            return dram.rearrange("(kp two p) s -> p kp two s", p=128, two=2)

        x8_a, xr_a = tri(x8), tri(xr)
        wp8_a, wpr_a = tri(wpk8), tri(wpkr)
        wv8_a, wvr_a = tri(wv8), tri(wvr)
        xT16_a = xT16.rearrange("(t p) s -> p t s", p=128)
        wpk16_a = wpk16.rearrange("(t p) m -> p t m", p=128)
        wv16_a = wv16.rearrange("(t p) m -> p t m", p=128)
        # fp16 head (first q-tile's projection inputs) loads first and small;
        # the fp8 triples only cover columns QT: onward.
        nc.sync.dma_start(out=wpk16_sb[:, :, 0:128], in_=wpk16_a[:, :, 0:128])
        nc.scalar.dma_start(out=xT16_sb[:, 0:3, :], in_=xT16_a[:, 0:3, :])
        nc.sync.dma_start(out=xT16_sb[:, 3:6, :], in_=xT16_a[:, 3:6, :])
        nc.scalar.dma_start(out=bqk_sb[:], in_=bqk[:, :])
        nc.sync.dma_start(out=wpk16_sb[:, :, 128:WPK], in_=wpk16_a[:, :, 128:WPK])
        nc.scalar.dma_start(out=wv16_sb[:], in_=wv16_a[:])
        nc.gpsimd.dma_start(out=ones_stage[:], in_=vones[:, :])
        nc.gpsimd.dma_start(out=ones64[:], in_=vones[0:1, 0:64])
        nc.vector.tensor_copy(
            out=v_aug[:, :, :, DH],
            in_=ones_stage[:, 0 : NKT * HPC].rearrange("p (t h) -> p t h", t=NKT),
        )
        # fp8 arrives in column-chunk order matching when each q-tile's
        # projection runs: weights + [QT:2QT] first, the rest behind.
        nc.scalar.dma_start(out=wpk8_sb[:], in_=wp8_a[:])
        nc.sync.dma_start(out=wpkr_sb[:], in_=wpr_a[:])
        nc.gpsimd.dma_start(out=wv8_sb[:], in_=wv8_a[:])
        nc.gpsimd.dma_start(out=wvr_sb[:], in_=wvr_a[:])
        nc.scalar.dma_start(
            out=x8_sb[:, :, :, QT : 2 * QT], in_=x8_a[:, :, :, QT : 2 * QT]
        )
        nc.sync.dma_start(
            out=xr_sb[:, :, :, QT : 2 * QT], in_=xr_a[:, :, :, QT : 2 * QT]
        )
        nc.scalar.dma_start(
            out=x8_sb[:, :, :, 2 * QT : S], in_=x8_a[:, :, :, 2 * QT : S]
        )
        nc.sync.dma_start(
            out=xr_sb[:, :, :, 2 * QT : S], in_=xr_a[:, :, :, 2 * QT : S]
        )
        nc.sync.dma_start(out=wo_a[:], in_=woT[0:128, :])
        nc.gpsimd.dma_start(out=wo_b[:], in_=woT[128:GD, :])

        # packed q/k projection m-groups: (col0, evict spec). m2 holds
        # [q2 k2]; its k2 rows evict to kT partitions 0:64 (cross-base copy)
        # so every head's q and k share a partition base for the PE.
        mgroups = [
            (0, [((0, 128), lambda n: qT_sb[0:128, 0, n * QT : (n + 1) * QT], 0)]),
            (128, [((0, 128), lambda n: kT_sb[0:128, 0, n * QT : (n + 1) * QT], 1)]),
            (256, [
                ((0, 64), lambda n: qT_sb[0:64, 1, n * QT : (n + 1) * QT], 2),
                ((64, 128), lambda n: kT_sb[0:64, 1, n * QT : (n + 1) * QT], 2),
            ]),
        ]

        proj_psums = {}
        PROJ_TERMS = [(wpk8_sb, x8_sb), (wpk8_sb, xr_sb), (wpkr_sb, x8_sb)]
        DESCALE = 1.0 / 1024.0  # host pre-scales x and W by 32 for fp8 range

        def proj_unit(mi, n, term):
            """One fp8 DoubleRow pass (3 k-pairs) of group (mi, n): term 0 is
            the w8.x8 main product, terms 1/2 add the x and W residuals."""
            c0, evicts = mgroups[mi]
            key = (mi, n)
            if key not in proj_psums:
                proj_psums[key] = ps.tile([128, QT], F32, tag="ps", name="projp")
            p = proj_psums[key]
            if n == 0:  # fp16 head: the small early arrays, no descale
                for k in (2 * term, 2 * term + 1):
                    nc.tensor.matmul(
                        p[:, :],
                        lhsT=wpk16_sb[:, k, c0 : c0 + 128],
                        rhs=xT16_sb[:, k, :],
                        start=(k == 0),
                        stop=(k == NKD - 1),
                    )
            else:
                wsb, xsb = PROJ_TERMS[term]
                for kp in range(3):
                    nc.tensor.matmul(
                        p[:, :],
                        lhsT=wsb[:, kp, :, c0 : c0 + 128],
                        rhs=xsb[:, kp, :, n * QT : (n + 1) * QT],
                        start=(term == 0 and kp == 0),
                        stop=(term == 2 and kp == 2),
                        perf_mode=DR,
                    )
            if term == 2:
                del proj_psums[key]
                for (r0, r1), dst, bcol in evicts:
                    if n == 0:
                        nc.vector.tensor_scalar_add(
                            out=dst(n),
                            in0=p[r0:r1, :],
                            scalar1=bqk_sb[r0:r1, bcol : bcol + 1],
                        )
                    else:
                        nc.vector.tensor_scalar(
                            out=dst(n),
                            in0=p[r0:r1, :],
                            scalar1=DESCALE,
                            scalar2=bqk_sb[r0:r1, bcol : bcol + 1],
                            op0=mybir.AluOpType.mult,
                            op1=mybir.AluOpType.add,
                        )

        vp_psums = {}
        V_TERMS = [(x8_sb, wv8_sb), (xr_sb, wv8_sb), (x8_sb, wvr_sb)]

        def v_unit(t, term):
            """V projection for keys-tile t (fp8 DoubleRow pass `term`):
            [128 keys, 192] psum; evict into v_aug[:, t, :, 0:64]."""
            if t not in vp_psums:
                vp_psums[t] = ps.tile([128, QT], F32, tag="ps", name="vp")
            p = vp_psums[t]
            if t < 4:  # fp16 head
                for k in (2 * term, 2 * term + 1):
                    nc.tensor.matmul(
                        p[:, 0:GD],
                        lhsT=xT16_sb[:, k, t * 128 : (t + 1) * 128],
                        rhs=wv16_sb[:, k, :],
                        start=(k == 0),
                        stop=(k == NKD - 1),
                    )
            else:
                xsb, wsb = V_TERMS[term]
                for kp in range(3):
                    nc.tensor.matmul(
                        p[:, 0:GD],
                        lhsT=xsb[:, kp, :, t * 128 : (t + 1) * 128],
                        rhs=wsb[:, kp, :, :],
                        start=(term == 0 and kp == 0),
                        stop=(term == 2 and kp == 2),
                        perf_mode=DR,
                    )
            if term == 2:
                del vp_psums[t]
                if t < 4:
                    nc.vector.tensor_copy(
                        out=v_aug[:, t, :, 0:DH],
                        in_=p[:, 0:GD].rearrange("p (h d) -> p h d", h=HPC),
                    )
                else:
                    nc.vector.tensor_scalar_mul(
                        out=v_aug[:, t, :, 0:DH],
                        in0=p[:, 0:GD].rearrange("p (h d) -> p h d", h=HPC),
                        scalar1=DESCALE,
                    )

        out_t = out_p.rearrange("(tp p) d -> tp p d", p=128)
        o_done = {}

        def o_proj_unit(t, n2, evict_eng=None, tail=False):
            key = t
            if key not in o_done:
                o_done[key] = expp.tile([128, D], F16, tag="osb", name="osb", bufs=3)
            ob = o_done[key]
            po = ps.tile([128, QT], F32, tag="ps")
            nc.tensor.matmul(
                po[:, 0:384],
                lhsT=zT01[:, t * 128 : (t + 1) * 128],
                rhs=wo_a[:, n2 * 384 : (n2 + 1) * 384],
                start=True,
                stop=False,
            )
            nc.tensor.matmul(
                po[:, 0:384],
                lhsT=zT2[:, t * 128 : (t + 1) * 128],
                rhs=wo_b[:, n2 * 384 : (n2 + 1) * 384],
                start=False,
                stop=True,
            )
            eng = evict_eng
            if eng is None:
                eng = "dve"
            if eng == "act":
                nc.scalar.activation(
                    out=ob[:, n2 * 384 : (n2 + 1) * 384], in_=po[:, 0:384], func=AF.Copy
                )
            else:
                nc.vector.tensor_copy(
                    out=ob[:, n2 * 384 : (n2 + 1) * 384], in_=po[:, 0:384]
                )
            if n2 == 1:
                del o_done[key]
                (nc.sync if (t % 2 == 0 or tail) else nc.scalar).dma_start(
                    out=out_t[t], in_=ob[:, :]
                )

        # background work queue of (key, fn), drained between attention
        # iterations. Queue order is topological, so force-draining "through
        # the last needed unit" preserves all producer->consumer ordering.
        work = deque()

        def q_proj(n, mis):
            for mi in mis:
                for term in range(3):
                    work.append(
                        (("proj", n, mi), lambda mi=mi, n=n, tm=term: proj_unit(mi, n, tm))
                    )

        def q_v(ts):
            for t in ts:
                for term in range(3):
                    work.append((("v", t), lambda t=t, tm=term: v_unit(t, tm)))

        def drain(k=1):
            for _ in range(k):
                if work:
                    work.popleft()[1]()

        def drain_all():
            while work:
                work.popleft()[1]()

        # head h's scores need these packed q/k groups
        PROJ_GROUPS_FOR_HEAD = {0: (0, 1), 1: (0, 1), 2: (2,)}

        def force_drain_for(h, qt):
            """Emit queued units up to the last one attention(h, qt) depends on."""
            needed = set()
            for n in range(qt + 1):
                for mi in PROJ_GROUPS_FOR_HEAD[h]:
                    needed.add(("proj", n, mi))
            for t in range(4 * qt + 4):
                needed.add(("v", t))
            last = -1
            for i, (key, _) in enumerate(work):
                if key in needed:
                    last = i
            for _ in range(last + 1):
                work.popleft()[1]()

        def qh(h):
            col, off = [(0, 0), (0, 64), (1, 0)][h]
            return qT_sb[off : off + 64, col, :]

        def kh(h):
            col, off = [(0, 0), (0, 64), (1, 0)][h]
            return kT_sb[off : off + 64, col, :]

        zdst = [zT01[0:64, :], zT01[64:128, :], zT2[0:64, :]]

        # PV matmuls are pipelined a few pairs behind their exp across block
        # boundaries, so the in-order PE FIFO never waits on the exp/mask
        # chain, not even at the end of a block.
        pvq = deque()  # (block_serial, pv_closure)
        blk_serial = [0]

        def pv_drain(depth):
            while len(pvq) > depth:
                pvq.popleft()[1]()

        def pv_flush(upto_serial):
            while pvq and pvq[0][0] <= upto_serial:
                pvq.popleft()[1]()

        def attention(h, qt, per_pair, last=False):
            """scores^T -> exp -> causal mask -> PV into zp. Score matmuls for
            kt pairs land in one 2-bank psum tile so a single exp covers both;
            start=True zeroes the whole bank, so the unwritten low columns of
            narrowed diagonal halves exp to 1.0 and are never consumed."""
            zp = psz.tile([DH + 1, QT], F32)
            nkt = 4 * qt + 4
            blk = blk_serial[0]
            blk_serial[0] += 1

            def pv(kt, es_ap, lo):
                nc.tensor.matmul(
                    zp[:, lo:QT],
                    lhsT=v_aug[:, kt, h, :],
                    rhs=es_ap[:, lo:QT],
                    start=(kt == 0),
                    stop=(kt == nkt - 1),
                )

            def pv_q(kt, es_ap, lo):
                # last block: quarter q of zp sees its final write at
                # kt = nkt-4+q, so it can be normalized immediately after.
                # Only that closing kt is split off (multiple start=True
                # writes into one bank would re-zero siblings' columns).
                r = kt - (nkt - 4)
                if r < 0:
                    nc.tensor.matmul(
                        zp[:, 0:QT],
                        lhsT=v_aug[:, kt, h, :],
                        rhs=es_ap[:, 0:QT],
                        start=(kt == 0),
                        stop=False,
                        skip_group_check=True,
                    )
                else:
                    c0 = 128 * r
                    nc.tensor.matmul(
                        zp[:, c0 : c0 + 128],
                        lhsT=v_aug[:, kt, h, :],
                        rhs=es_ap[:, c0 : c0 + 128],
                        start=False,
                        stop=True,
                        skip_group_check=True,
                    )
                    if c0 + 128 < QT:
                        nc.tensor.matmul(
                            zp[:, c0 + 128 : QT],
                            lhsT=v_aug[:, kt, h, :],
                            rhs=es_ap[:, c0 + 128 : QT],
                            start=False,
                            stop=(kt == nkt - 1),
                            skip_group_check=True,
                        )
                    stairs(r)

            st_rec, st_bc = {}, {}

            def qcols(q):
                return slice(128 * q, 128 * (q + 1))

            def stairs(step):
                # stage skew keeps the in-order PE FIFO free of long waits:
                # each quarter's PE pieces (bc, o-proj) are emitted one or two
                # pv-steps after the pv that closed the quarter's psum group.
                if step >= 1:
                    st_bc[step - 1] = norm_bc(st_rec[step - 1], qcols(step - 1))
                st_rec[step] = norm_recip(zp, qcols(step))
                if step >= 1:
                    norm_post(zp, h, qt, st_bc[step - 1], qcols(step - 1))
                if step >= 2:
                    t = 4 * qt + step - 2
                    o_proj_unit(t, 0, evict_eng="dve", tail=True)
                    o_proj_unit(t, 1, evict_eng="act", tail=True)

            def stairs_final():
                st_bc[3] = norm_bc(st_rec[3], qcols(3))
                norm_post(zp, h, qt, st_bc[3], qcols(3))
                for t in (4 * qt + 2, 4 * qt + 3):
                    o_proj_unit(t, 0, evict_eng="dve", tail=True)
                    o_proj_unit(t, 1, evict_eng="act", tail=True)

            for kp in range(nkt // 2):
                kt0 = 2 * kp
                rr0 = kt0 - 4 * qt
                lo_pair = 128 * rr0 if rr0 > 0 else 0
                sp = ps2.tile([128, 2, QT], F32, tag="ps2")
                los = []
                for i in (0, 1):
                    kt = kt0 + i
                    rr = kt - 4 * qt
                    lo = 128 * rr if rr > 0 else 0
                    los.append(lo)
                    nc.tensor.matmul(
                        sp[:, i, lo:QT],
                        lhsT=kh(h)[:, kt * 128 : (kt + 1) * 128],
                        rhs=qh(h)[:, qt * QT + lo : (qt + 1) * QT],
                        start=True,
                        stop=True,
                    )
                es = expp.tile([128, 2, QT], F16, tag="expp")
                nc.scalar.activation(
                    out=es[:, :, lo_pair:QT], in_=sp[:, :, lo_pair:QT], func=AF.Exp
                )
                for i in (0, 1):
                    kt = kt0 + i
                    rr = kt - 4 * qt
                    lo = los[i]
                    if rr >= 0:  # diagonal square: zero where key > query
                        hi = min(lo + 128, QT)
                        nc.gpsimd.affine_select(
                            out=es[:, i, lo:hi],
                            in_=es[:, i, lo:hi],
                            compare_op=mybir.AluOpType.is_ge,
                            fill=0.0,
                            base=0,
                            channel_multiplier=-1,
                            pattern=[[1, hi - lo]],
                        )
                    fn = pv_q if last else pv
                    pvq.append(
                        (blk, lambda kt=kt, es=es, i=i, lo=lo, fn=fn: fn(kt, es[:, i], lo))
                    )
                drain(per_pair)
                pv_drain(2 if last else 7)
            if last:
                pv_flush(blk)
                stairs_final()
            return zp, blk

        def norm_recip(zp, cols):
            rec = small.tile([1, QT], F16, tag="rec")
            with nc.allow_low_precision(reason="fp16 normalize"):
                nc.vector.reciprocal(rec[:, cols], zp[DH : DH + 1, cols])
            return rec

        def norm_bc(rec, cols):
            bc_sb = small.tile([64, QT], F16, tag="bcsb")
            nc.gpsimd.partition_broadcast(bc_sb[:, cols], rec[0:1, cols])
            return bc_sb

        def norm_post(zp, h, qt, bc_sb, cols):
            nc.vector.tensor_mul(
                zdst[h][:, qt * QT : (qt + 1) * QT][:, cols],
                zp[0:DH, cols],
                bc_sb[:, cols],
            )

        def normalize(zp, h, qt, cols=slice(0, QT)):
            rec = norm_recip(zp, cols)
            bc_sb = norm_bc(rec, cols)
            norm_post(zp, h, qt, bc_sb, cols)

        # ---- schedule ----
        # prologue: only what attention(h0, qt0) needs; the rest queues up.
        for mi in (0, 1):
            for term in range(3):
                proj_unit(mi, 0, term)
        for t in range(4):
            for term in range(3):
                v_unit(t, term)
        q_proj(0, mis=(2,))
        for n in range(1, NQT):
            q_proj(n, mis=(0, 1))
            q_v(range(4 * n, 4 * n + 2))
            q_proj(n, mis=(2,))
            q_v(range(4 * n + 2, 4 * n + 4))

        pending = None
        for qt in range(NQT):
            per_pair = [4, 1, 1, 2][qt]
            for h in range(HPC):
                force_drain_for(h, qt)
                is_last = qt == NQT - 1 and h == HPC - 1
                if is_last:
                    # everything queued must land before the staircase epilogue
                    pv_flush(pending[3])
                    normalize(*pending[:3])
                    drain_all()
                    pending = None
                zp, blk = attention(h, qt, per_pair, last=is_last)
                if pending is not None:
                    pv_flush(pending[3])  # pending block's PV accumulation done
                    normalize(*pending[:3])
                    ph, pqt = pending[1], pending[2]
                    if ph == HPC - 1:  # whole q-tile normalized -> O-proj ready
                        for t in range(4 * pqt, 4 * pqt + 4):
                            for n2 in range(2):
                                work.append(
                                    (("o", pqt), lambda t=t, n2=n2: o_proj_unit(t, n2))
                                )
                if not is_last:
                    pending = (zp, h, qt, blk)
    nc.finalize()
    return nc


_NC_CACHE = {}


def _f8(a):
    """main fp8 + residual fp8 (inputs pre-scaled x32, so both normal-range)."""
    import ml_dtypes

    f8 = ml_dtypes.float8_e4m3
    a = np.ascontiguousarray(a, np.float32)
    m = a.astype(f8)
    r = (a - m.astype(np.float32)).astype(f8)
    return np.ascontiguousarray(m), np.ascontiguousarray(r)


def make_in_maps(x, W_qkv, b_qkv, W_o):
    in_maps = []
    for c in range(8):
        b, g = divmod(c, 4)
        hs = [HPC * g + i for i in range(HPC)]
        qr = [np.arange(64 * h, 64 * h + 64) for h in hs]
        w_q = [W_qkv[i] * 0.125 for i in qr]
        w_k = [W_qkv[768 + i] for i in qr]
        w_v = [W_qkv[1536 + i] for i in qr]
        b_q = [b_qkv[i] * 0.125 for i in qr]
        b_k = [b_qkv[768 + i] for i in qr]
        # packed rows: m0=[q0 q1] m1=[k0 k1] m2=[q2 k2]
        wpk = np.concatenate(
            [w_q[0], w_q[1], w_k[0], w_k[1], w_q[2], w_k[2]], axis=0
        )
        wv = np.concatenate([w_v[0], w_v[1], w_v[2]], axis=0)
        bqk_col = np.zeros((128, 3), np.float32)
        bqk_col[:, 0] = np.concatenate([b_q[0], b_q[1]])
        bqk_col[:, 1] = np.concatenate([b_k[0], b_k[1]])
        bqk_col[:, 2] = np.concatenate([b_q[2], b_k[2]])
        # fp8 triples (values pre-scaled by 32; 1/1024 folded into evictions)
        xm = _f8(32.0 * x[b].T)
        wpm = _f8(32.0 * wpk.T)
        wvm = _f8(32.0 * wv.T)
        in_maps.append(
            {
                "xT16": np.ascontiguousarray(x[b].T[:, 0:QT].astype(np.float16)),
                "wpk16": np.ascontiguousarray(wpk.T.astype(np.float16)),
                "wv16": np.ascontiguousarray(wv.T.astype(np.float16)),
                "x8": xm[0], "xr": xm[1],
                "wpk8": wpm[0], "wpkr": wpm[1],
                "wv8": wvm[0], "wvr": wvm[1],
                "woT": np.ascontiguousarray(
                    W_o[:, GD * g : GD * (g + 1)].T.astype(np.float16)
                ),
                "bqk": bqk_col,
                "vones": np.ones((128, 64), np.float16),
            }
        )
    return in_maps


def make_in_maps_for_test(inputs):
    return make_in_maps(
        np.asarray(inputs["x"], np.float32),
        np.asarray(inputs["W_qkv"], np.float32),
        np.asarray(inputs["b_qkv"], np.float32),
        np.asarray(inputs["W_o"], np.float32),
    )


def kernel(x, W_qkv, b_qkv, W_o, b_o):
    x = np.asarray(x, np.float32)
    W_qkv = np.asarray(W_qkv, np.float32)
    b_qkv = np.asarray(b_qkv, np.float32)
    W_o = np.asarray(W_o, np.float32)
    b_o = np.asarray(b_o, np.float32)

    if "nc" not in _NC_CACHE:
        _NC_CACHE["nc"] = build_bass()
    nc = _NC_CACHE["nc"]

    in_maps = make_in_maps(x, W_qkv, b_qkv, W_o)

    res = run_bass_kernel_spmd(
        nc,
        in_maps,
        list(range(8)),
        trace=bool(int(os.environ.get("KERNEL_TRACE", "0"))),
    )
    _NC_CACHE["last_results"] = res

    out = np.zeros((B, S, D), np.float32)
    for c in range(8):
        out[c // 4] += res.results[c]["out_p"].astype(np.float32)
    out += b_qkv[1536:] @ W_o.T + b_o
    return out


# revision 25
# speedup vs baseline: 1.2351x; 1.0151x over previous
"""Causal multi-head attention block (B=2, S=2048, D=768, H=12) on 8 trn2 cores.

Sharding: core c -> batch b = c//4 (data parallel), head group g = c%4
(tensor parallel, 3 heads per group). Each core computes its group's QKV
projection, causal attention, and a partial O-projection over its 192
z-columns. Host sums the 4 partials per batch and adds the biases that
commute through the math (v-bias and b_o).

On-core layout: xT [768, 2048] (d on partitions) so q^T/k^T come straight
out of the projection with head dims on partitions; V is projected
separately in [keys, dh] layout (lhsT = xT key-block, rhs = W_v^T columns)
so the PV matmul needs no transposes at all. A ones-column appended to V
yields the softmax denominator for free.

All matmul operands are fp16 (1 col/cycle on the PE, 10-bit mantissa keeps
rel-err ~5e-4); psum stays f32. Score matmuls for kt pairs land in one
2-bank psum tile so a single exp instruction covers both; the causal mask
only touches the 128-wide diagonal square.

The packed q/k projection weight is host-repacked so all three 128-wide
M-groups are full:  m0=[q_h0 q_h1]  m1=[q_h2 k_h0]  m2=[k_h1 k_h2]
(q rows pre-scaled by 1/8; v bias folded into the host-side epilogue).
"""

import os
from collections import deque
from contextlib import ExitStack

import numpy as np

import concourse.tile as tile
from concourse import bacc, mybir
from concourse.bass_utils import run_bass_kernel_spmd

F32 = mybir.dt.float32
F16 = mybir.dt.float16
F8 = mybir.dt.float8e4
DR = mybir.MatmulPerfMode.DoubleRow
AF = mybir.ActivationFunctionType

B, S, D = 2, 2048, 768
NH, DH = 12, 64
HPC = 3            # heads per core
GD = HPC * DH      # 192 z-cols per core
KT, QT = 128, 512  # key tile (partitions), q tile (psum free)
NKT, NQT = S // KT, S // QT   # 16, 4
NKD = D // 128     # 6 contraction tiles for the projections
WPK = 2 * GD       # 384 packed q/k projection rows


def build_bass():
    nc = bacc.Bacc(None)
    x8 = nc.dram_tensor("x8", [D, S], F8, kind="ExternalInput")
    xr = nc.dram_tensor("xr", [D, S], F8, kind="ExternalInput")
    wpk8 = nc.dram_tensor("wpk8", [D, WPK], F8, kind="ExternalInput")
    wpkr = nc.dram_tensor("wpkr", [D, WPK], F8, kind="ExternalInput")
    wv8 = nc.dram_tensor("wv8", [D, GD], F8, kind="ExternalInput")
    wvr = nc.dram_tensor("wvr", [D, GD], F8, kind="ExternalInput")
    woT = nc.dram_tensor("woT", [GD, D], F16, kind="ExternalInput")
    bqk = nc.dram_tensor("bqk", [128, 3], F32, kind="ExternalInput")
    vones = nc.dram_tensor("vones", [128, 64], F16, kind="ExternalInput")
    out_p = nc.dram_tensor("out_p", [S, D], F16, kind="ExternalOutput")

    with tile.TileContext(nc) as tc, ExitStack() as ctx:
        const = ctx.enter_context(tc.tile_pool(name="const", bufs=1))
        ps = ctx.enter_context(tc.tile_pool(name="ps", bufs=2, space="PSUM"))
        ps2 = ctx.enter_context(tc.tile_pool(name="ps2", bufs=2, space="PSUM"))
        psz = ctx.enter_context(tc.tile_pool(name="psz", bufs=2, space="PSUM"))
        expp = ctx.enter_context(tc.tile_pool(name="expp", bufs=9))
        small = ctx.enter_context(tc.tile_pool(name="small", bufs=4))

        x8_sb = const.tile([128, 3, 2, S], F8)
        xr_sb = const.tile([128, 3, 2, S], F8)
        wpk8_sb = const.tile([128, 3, 2, WPK], F8)
        wpkr_sb = const.tile([128, 3, 2, WPK], F8)
        wv8_sb = const.tile([128, 3, 2, GD], F8)
        wvr_sb = const.tile([128, 3, 2, GD], F8)
        wo_a = const.tile([128, D], F16)
        wo_b = const.tile([64, D], F16)
        bqk_sb = const.tile([128, 3], F32)
        qT_sb = const.tile([128, 2, S], F16)
        kT_sb = const.tile([128, 2, S], F16)
        v_aug = const.tile([128, NKT, HPC, DH + 1], F16)
        zT01 = const.tile([128, S], F16)
        zT2 = const.tile([64, S], F16)
        ones_stage = const.tile([128, 64], F16)

        # ---- loads: k-interleaved so the first projection k-pairs unblock
        # early. Early DMAs fan out over four queues (SP/Act/DVE/Pool) to
        # dodge the ~650ns per-DMA dispatch serialization on a single queue.
        # batched multi-tile DMAs keep the dispatch count low (each dispatch
        # costs ~650ns of queue time). The projection runs its main fp8 pass
        # first, so the x8/w8 arrays go first and the residual arrays follow.
        def tri(dram):
            return dram.rearrange("(kp two p) s -> p kp two s", p=128, two=2)

        x8_a, xr_a = tri(x8), tri(xr)
        wp8_a, wpr_a = tri(wpk8), tri(wpkr)
        wv8_a, wvr_a = tri(wv8), tri(wvr)
        # the first q-tile's operands stream first: weight slivers for m0,
        # then x8/xr column chunks; residual weights chase the mains.
        nc.sync.dma_start(out=wpk8_sb[:, :, :, 0:128], in_=wp8_a[:, :, :, 0:128])
        nc.scalar.dma_start(out=x8_sb[:, :, :, 0:QT], in_=x8_a[:, :, :, 0:QT])
        nc.sync.dma_start(out=wpkr_sb[:, :, :, 0:128], in_=wpr_a[:, :, :, 0:128])
        nc.scalar.dma_start(out=xr_sb[:, :, :, 0:QT], in_=xr_a[:, :, :, 0:QT])
        nc.sync.dma_start(out=bqk_sb[:], in_=bqk[:, :])
        nc.scalar.dma_start(
            out=wpk8_sb[:, :, :, 128:WPK], in_=wp8_a[:, :, :, 128:WPK]
        )
        nc.sync.dma_start(
            out=wpkr_sb[:, :, :, 128:WPK], in_=wpr_a[:, :, :, 128:WPK]
        )
        nc.gpsimd.dma_start(out=wv8_sb[:], in_=wv8_a[:])
        nc.gpsimd.dma_start(out=wvr_sb[:], in_=wvr_a[:])
        nc.gpsimd.dma_start(out=ones_stage[:], in_=vones[:, :])
        nc.vector.tensor_copy(
            out=v_aug[:, :, :, DH],
            in_=ones_stage[:, 0 : NKT * HPC].rearrange("p (t h) -> p t h", t=NKT),
        )
        nc.scalar.dma_start(
            out=x8_sb[:, :, :, QT : 2 * QT], in_=x8_a[:, :, :, QT : 2 * QT]
        )
        nc.sync.dma_start(
            out=xr_sb[:, :, :, QT : 2 * QT], in_=xr_a[:, :, :, QT : 2 * QT]
        )
        nc.scalar.dma_start(
            out=x8_sb[:, :, :, 2 * QT : S], in_=x8_a[:, :, :, 2 * QT : S]
        )
        nc.sync.dma_start(
            out=xr_sb[:, :, :, 2 * QT : S], in_=xr_a[:, :, :, 2 * QT : S]
        )
        nc.sync.dma_start(out=wo_a[:], in_=woT[0:128, :])
        nc.gpsimd.dma_start(out=wo_b[:], in_=woT[128:GD, :])

        # packed q/k projection m-groups: (col0, evict spec). m2 holds
        # [q2 k2]; its k2 rows evict to kT partitions 0:64 (cross-base copy)
        # so every head's q and k share a partition base for the PE.
        mgroups = [
            (0, [((0, 128), lambda n: qT_sb[0:128, 0, n * QT : (n + 1) * QT], 0)]),
            (128, [((0, 128), lambda n: kT_sb[0:128, 0, n * QT : (n + 1) * QT], 1)]),
            (256, [
                ((0, 64), lambda n: qT_sb[0:64, 1, n * QT : (n + 1) * QT], 2),
                ((64, 128), lambda n: kT_sb[0:64, 1, n * QT : (n + 1) * QT], 2),
            ]),
        ]

        proj_psums = {}
        PROJ_TERMS = [(wpk8_sb, x8_sb), (wpk8_sb, xr_sb), (wpkr_sb, x8_sb)]
        DESCALE = 1.0 / 1024.0  # host pre-scales x and W by 32 for fp8 range

        def proj_unit(mi, n, term):
            """One fp8 DoubleRow pass (3 k-pairs) of group (mi, n): term 0 is
            the w8.x8 main product, terms 1/2 add the x and W residuals."""
            c0, evicts = mgroups[mi]
            key = (mi, n)
            if key not in proj_psums:
                proj_psums[key] = ps.tile([128, QT], F32, tag="ps", name="projp")
            p = proj_psums[key]
            wsb, xsb = PROJ_TERMS[term]
            for kp in range(3):
                nc.tensor.matmul(
                    p[:, :],
                    lhsT=wsb[:, kp, :, c0 : c0 + 128],
                    rhs=xsb[:, kp, :, n * QT : (n + 1) * QT],
                    start=(term == 0 and kp == 0),
                    stop=(term == 2 and kp == 2),
                    perf_mode=DR,
                )
            if term == 2:
                del proj_psums[key]
                for (r0, r1), dst, bcol in evicts:
                    nc.vector.tensor_scalar(
                        out=dst(n),
                        in0=p[r0:r1, :],
                        scalar1=DESCALE,
                        scalar2=bqk_sb[r0:r1, bcol : bcol + 1],
                        op0=mybir.AluOpType.mult,
                        op1=mybir.AluOpType.add,
                    )

        vp_psums = {}
        V_TERMS = [(x8_sb, wv8_sb), (xr_sb, wv8_sb), (x8_sb, wvr_sb)]

        def v_unit(t, term):
            """V projection for keys-tile t (fp8 DoubleRow pass `term`):
            [128 keys, 192] psum; evict into v_aug[:, t, :, 0:64]."""
            if t not in vp_psums:
                vp_psums[t] = ps.tile([128, QT], F32, tag="ps", name="vp")
            p = vp_psums[t]
            xsb, wsb = V_TERMS[term]
            for kp in range(3):
                nc.tensor.matmul(
                    p[:, 0:GD],
                    lhsT=xsb[:, kp, :, t * 128 : (t + 1) * 128],
                    rhs=wsb[:, kp, :, :],
                    start=(term == 0 and kp == 0),
                    stop=(term == 2 and kp == 2),
                    perf_mode=DR,
                )
            if term == 2:
                del vp_psums[t]
                nc.vector.tensor_scalar_mul(
                    out=v_aug[:, t, :, 0:DH],
                    in0=p[:, 0:GD].rearrange("p (h d) -> p h d", h=HPC),
                    scalar1=DESCALE,
                )

        out_t = out_p.rearrange("(tp p) d -> tp p d", p=128)
        o_done = {}

        def o_proj_unit(t, n2, evict_eng=None, tail=False):
            key = t
            if key not in o_done:
                o_done[key] = expp.tile([128, D], F16, tag="osb", name="osb", bufs=3)
            ob = o_done[key]
            po = ps.tile([128, QT], F32, tag="ps")
            nc.tensor.matmul(
                po[:, 0:384],
                lhsT=zT01[:, t * 128 : (t + 1) * 128],
                rhs=wo_a[:, n2 * 384 : (n2 + 1) * 384],
                start=True,
                stop=False,
            )
            nc.tensor.matmul(
                po[:, 0:384],
                lhsT=zT2[:, t * 128 : (t + 1) * 128],
                rhs=wo_b[:, n2 * 384 : (n2 + 1) * 384],
                start=False,
                stop=True,
            )
            eng = evict_eng
            if eng is None:
                eng = "dve"
            if eng == "act":
                nc.scalar.activation(
                    out=ob[:, n2 * 384 : (n2 + 1) * 384], in_=po[:, 0:384], func=AF.Copy
                )
            else:
                nc.vector.tensor_copy(
                    out=ob[:, n2 * 384 : (n2 + 1) * 384], in_=po[:, 0:384]
                )
            if tail:
                (nc.sync if n2 == 0 else nc.scalar).dma_start(
                    out=out_t[t][:, n2 * 384 : (n2 + 1) * 384],
                    in_=ob[:, n2 * 384 : (n2 + 1) * 384],
                )
                if n2 == 1:
                    del o_done[key]
            elif n2 == 1:
                del o_done[key]
                (nc.sync if t % 2 == 0 else nc.scalar).dma_start(
                    out=out_t[t], in_=ob[:, :]
                )

        # background work queue of (key, fn), drained between attention
        # iterations. Queue order is topological, so force-draining "through
        # the last needed unit" preserves all producer->consumer ordering.
        work = deque()

        def q_proj(n, mis):
            for mi in mis:
                for term in range(3):
                    work.append(
                        (("proj", n, mi), lambda mi=mi, n=n, tm=term: proj_unit(mi, n, tm))
                    )

        def q_v(ts):
            for t in ts:
                for term in range(3):
                    work.append((("v", t), lambda t=t, tm=term: v_unit(t, tm)))

        def drain(k=1):
            for _ in range(k):
                if work:
                    work.popleft()[1]()

        def drain_all():
            while work:
                work.popleft()[1]()

        # head h's scores need these packed q/k groups
        PROJ_GROUPS_FOR_HEAD = {0: (0, 1), 1: (0, 1), 2: (2,)}

        def force_drain_for(h, qt):
            """Emit queued units up to the last one attention(h, qt) depends on."""
            needed = set()
            for n in range(qt + 1):
                for mi in PROJ_GROUPS_FOR_HEAD[h]:
                    needed.add(("proj", n, mi))
            for t in range(4 * qt + 4):
                needed.add(("v", t))
            last = -1
            for i, (key, _) in enumerate(work):
                if key in needed:
                    last = i
            for _ in range(last + 1):
                work.popleft()[1]()

        def qh(h):
            col, off = [(0, 0), (0, 64), (1, 0)][h]
            return qT_sb[off : off + 64, col, :]

        def kh(h):
            col, off = [(0, 0), (0, 64), (1, 0)][h]
            return kT_sb[off : off + 64, col, :]

        zdst = [zT01[0:64, :], zT01[64:128, :], zT2[0:64, :]]

        # PV matmuls are pipelined a few pairs behind their exp across block
        # boundaries, so the in-order PE FIFO never waits on the exp/mask
        # chain, not even at the end of a block.
        pvq = deque()  # (block_serial, pv_closure)
        blk_serial = [0]

        def pv_drain(depth):
            while len(pvq) > depth:
                pvq.popleft()[1]()

        def pv_flush(upto_serial):
            while pvq and pvq[0][0] <= upto_serial:
                pvq.popleft()[1]()

        def attention(h, qt, per_pair, last=False):
            """scores^T -> exp -> causal mask -> PV into zp. Score matmuls for
            kt pairs land in one 2-bank psum tile so a single exp covers both;
            start=True zeroes the whole bank, so the unwritten low columns of
            narrowed diagonal halves exp to 1.0 and are never consumed."""
            zp = psz.tile([DH + 1, QT], F32)
            nkt = 4 * qt + 4
            blk = blk_serial[0]
            blk_serial[0] += 1

            def pv(kt, es_ap, lo):
                nc.tensor.matmul(
                    zp[:, lo:QT],
                    lhsT=v_aug[:, kt, h, :],
                    rhs=es_ap[:, lo:QT],
                    start=(kt == 0),
                    stop=(kt == nkt - 1),
                )

            def pv_q(kt, es_ap, lo):
                # last block: quarter q of zp sees its final write at
                # kt = nkt-4+q, so it can be normalized immediately after.
                # Only that closing kt is split off (multiple start=True
                # writes into one bank would re-zero siblings' columns).
                r = kt - (nkt - 4)
                if r < 0:
                    nc.tensor.matmul(
                        zp[:, 0:QT],
                        lhsT=v_aug[:, kt, h, :],
                        rhs=es_ap[:, 0:QT],
                        start=(kt == 0),
                        stop=False,
                        skip_group_check=True,
                    )
                else:
                    c0 = 128 * r
                    nc.tensor.matmul(
                        zp[:, c0 : c0 + 128],
                        lhsT=v_aug[:, kt, h, :],
                        rhs=es_ap[:, c0 : c0 + 128],
                        start=False,
                        stop=True,
                        skip_group_check=True,
                    )
                    if c0 + 128 < QT:
                        nc.tensor.matmul(
                            zp[:, c0 + 128 : QT],
                            lhsT=v_aug[:, kt, h, :],
                            rhs=es_ap[:, c0 + 128 : QT],
                            start=False,
                            stop=(kt == nkt - 1),
                            skip_group_check=True,
                        )
                    stairs(r)

            st_rec, st_bc = {}, {}

            def qcols(q):
                return slice(128 * q, 128 * (q + 1))

            def stairs(step):
                # stage skew keeps the in-order PE FIFO free of long waits:
                # each quarter's PE pieces (bc, o-proj) are emitted one or two
                # pv-steps after the pv that closed the quarter's psum group.
                if step >= 1:
                    st_bc[step - 1] = norm_bc(st_rec[step - 1], qcols(step - 1))
                st_rec[step] = norm_recip(zp, qcols(step))
                if step >= 1:
                    norm_post(zp, h, qt, st_bc[step - 1], qcols(step - 1))
                if step >= 2:
                    t = 4 * qt + step - 2
                    o_proj_unit(t, 0, evict_eng="dve", tail=True)
                    o_proj_unit(t, 1, evict_eng="act", tail=True)

            def stairs_final():
                st_bc[3] = norm_bc(st_rec[3], qcols(3))
                norm_post(zp, h, qt, st_bc[3], qcols(3))
                for t in (4 * qt + 2, 4 * qt + 3):
                    o_proj_unit(t, 0, evict_eng="dve", tail=True)
                    o_proj_unit(t, 1, evict_eng="act", tail=True)

            for kp in range(nkt // 2):
                kt0 = 2 * kp
                rr0 = kt0 - 4 * qt
                lo_pair = 128 * rr0 if rr0 > 0 else 0
                sp = ps2.tile([128, 2, QT], F32, tag="ps2")
                los = []
                for i in (0, 1):
                    kt = kt0 + i
                    rr = kt - 4 * qt
                    lo = 128 * rr if rr > 0 else 0
                    los.append(lo)
                    nc.tensor.matmul(
                        sp[:, i, lo:QT],
                        lhsT=kh(h)[:, kt * 128 : (kt + 1) * 128],
                        rhs=qh(h)[:, qt * QT + lo : (qt + 1) * QT],
                        start=True,
                        stop=True,
                    )
                es = expp.tile([128, 2, QT], F16, tag="expp")
                nc.scalar.activation(
                    out=es[:, :, lo_pair:QT], in_=sp[:, :, lo_pair:QT], func=AF.Exp
                )
                for i in (0, 1):
                    kt = kt0 + i
                    rr = kt - 4 * qt
                    lo = los[i]
                    if rr >= 0:  # diagonal square: zero where key > query
                        hi = min(lo + 128, QT)
                        nc.gpsimd.affine_select(
                            out=es[:, i, lo:hi],
                            in_=es[:, i, lo:hi],
                            compare_op=mybir.AluOpType.is_ge,
                            fill=0.0,
                            base=0,
                            channel_multiplier=-1,
                            pattern=[[1, hi - lo]],
                        )
                    fn = pv_q if last else pv
                    pvq.append(
                        (blk, lambda kt=kt, es=es, i=i, lo=lo, fn=fn: fn(kt, es[:, i], lo))
                    )
                if per_pair >= 1:
                    drain(per_pair)
                elif kp % 3 == 0:
                    drain(1)
                pv_drain(2 if last else 7)
            if last:
                pv_flush(blk)
                stairs_final()
            return zp, blk

        def norm_recip(zp, cols):
            rec = small.tile([1, QT], F16, tag="rec")
            with nc.allow_low_precision(reason="fp16 normalize"):
                nc.vector.reciprocal(rec[:, cols], zp[DH : DH + 1, cols])
            return rec

        def norm_bc(rec, cols):
            bc_sb = small.tile([64, QT], F16, tag="bcsb")
            nc.gpsimd.partition_broadcast(bc_sb[:, cols], rec[0:1, cols])
            return bc_sb

        def norm_post(zp, h, qt, bc_sb, cols):
            nc.vector.tensor_mul(
                zdst[h][:, qt * QT : (qt + 1) * QT][:, cols],
                zp[0:DH, cols],
                bc_sb[:, cols],
            )

        def normalize(zp, h, qt, cols=slice(0, QT)):
            rec = norm_recip(zp, cols)
            bc_sb = norm_bc(rec, cols)
            norm_post(zp, h, qt, bc_sb, cols)

        # ---- schedule ----
        # prologue: only what attention(h0, qt0) needs; the rest queues up.
        for mi in (0, 1):
            for term in range(3):
                proj_unit(mi, 0, term)
        for t in range(4):
            for term in range(3):
                v_unit(t, term)
        q_proj(0, mis=(2,))
        for n in range(1, NQT):
            q_proj(n, mis=(0, 1))
            q_v(range(4 * n, 4 * n + 2))
            q_proj(n, mis=(2,))
            q_v(range(4 * n + 2, 4 * n + 4))

        pending = None
        for qt in range(NQT):
            per_pair = [4, 1, 1, 0][qt]
            for h in range(HPC):
                force_drain_for(h, qt)
                is_last = qt == NQT - 1 and h == HPC - 1
                if is_last:
                    # everything queued must land before the staircase epilogue
                    pv_flush(pending[3])
                    normalize(*pending[:3])
                    drain_all()
                    pending = None
                zp, blk = attention(h, qt, per_pair, last=is_last)
                if pending is not None:
                    pv_flush(pending[3])  # pending block's PV accumulation done
                    normalize(*pending[:3])
                    ph, pqt = pending[1], pending[2]
                    if ph == HPC - 1:  # whole q-tile normalized -> O-proj ready
                        for t in range(4 * pqt, 4 * pqt + 4):
                            for n2 in range(2):
                                work.append(
                                    (("o", pqt), lambda t=t, n2=n2: o_proj_unit(t, n2))
                                )
                if not is_last:
                    pending = (zp, h, qt, blk)
    nc.finalize()
    return nc


_NC_CACHE = {}


def _f8(a):
    """main fp8 + residual fp8 (inputs pre-scaled x32, so both normal-range)."""
    import ml_dtypes

    f8 = ml_dtypes.float8_e4m3
    a = np.ascontiguousarray(a, np.float32)
    m = a.astype(f8)
    r = (a - m.astype(np.float32)).astype(f8)
    return np.ascontiguousarray(m), np.ascontiguousarray(r)


def make_in_maps(x, W_qkv, b_qkv, W_o):
    in_maps = []
    for c in range(8):
        b, g = divmod(c, 4)
        hs = [HPC * g + i for i in range(HPC)]
        qr = [np.arange(64 * h, 64 * h + 64) for h in hs]
        w_q = [W_qkv[i] * 0.125 for i in qr]
        w_k = [W_qkv[768 + i] for i in qr]
        w_v = [W_qkv[1536 + i] for i in qr]
        b_q = [b_qkv[i] * 0.125 for i in qr]
        b_k = [b_qkv[768 + i] for i in qr]
        # packed rows: m0=[q0 q1] m1=[k0 k1] m2=[q2 k2]
        wpk = np.concatenate(
            [w_q[0], w_q[1], w_k[0], w_k[1], w_q[2], w_k[2]], axis=0
        )
        wv = np.concatenate([w_v[0], w_v[1], w_v[2]], axis=0)
        bqk_col = np.zeros((128, 3), np.float32)
        bqk_col[:, 0] = np.concatenate([b_q[0], b_q[1]])
        bqk_col[:, 1] = np.concatenate([b_k[0], b_k[1]])
        bqk_col[:, 2] = np.concatenate([b_q[2], b_k[2]])
        # fp8 triples (values pre-scaled by 32; 1/1024 folded into evictions)
        xm = _f8(32.0 * x[b].T)
        wpm = _f8(32.0 * wpk.T)
        wvm = _f8(32.0 * wv.T)
        in_maps.append(
            {
                "x8": xm[0], "xr": xm[1],
                "wpk8": wpm[0], "wpkr": wpm[1],
                "wv8": wvm[0], "wvr": wvm[1],
                "woT": np.ascontiguousarray(
                    W_o[:, GD * g : GD * (g + 1)].T.astype(np.float16)
                ),
                "bqk": bqk_col,
                "vones": np.ones((128, 64), np.float16),
            }
        )
    return in_maps


def make_in_maps_for_test(inputs):
    return make_in_maps(
        np.asarray(inputs["x"], np.float32),
        np.asarray(inputs["W_qkv"], np.float32),
        np.asarray(inputs["b_qkv"], np.float32),
        np.asarray(inputs["W_o"], np.float32),
    )


def kernel(x, W_qkv, b_qkv, W_o, b_o):
    x = np.asarray(x, np.float32)
    W_qkv = np.asarray(W_qkv, np.float32)
    b_qkv = np.asarray(b_qkv, np.float32)
    W_o = np.asarray(W_o, np.float32)
    b_o = np.asarray(b_o, np.float32)

    if "nc" not in _NC_CACHE:
        _NC_CACHE["nc"] = build_bass()
    nc = _NC_CACHE["nc"]

    in_maps = make_in_maps(x, W_qkv, b_qkv, W_o)

    res = run_bass_kernel_spmd(
        nc,
        in_maps,
        list(range(8)),
        trace=bool(int(os.environ.get("KERNEL_TRACE", "0"))),
    )
    _NC_CACHE["last_results"] = res

    out = np.zeros((B, S, D), np.float32)
    for c in range(8):
        out[c // 4] += res.results[c]["out_p"].astype(np.float32)
    out += b_qkv[1536:] @ W_o.T + b_o
    return out


# revision 34
# speedup vs baseline: 1.2704x; 1.0286x over previous
"""Causal multi-head attention block (B=2, S=2048, D=768, H=12) on 8 trn2 cores.

Sharding: core c -> batch b = c//4 (data parallel), head group g = c%4
(tensor parallel, 3 heads per group). Each core computes its group's QKV
projection, causal attention, and a partial O-projection over its 192
z-columns. Host sums the 4 partials per batch and adds the biases that
commute through the math (v-bias and b_o).

On-core layout: xT [768, 2048] (d on partitions) so q^T/k^T come straight
out of the projection with head dims on partitions; V is projected
separately in [keys, dh] layout (lhsT = xT key-block, rhs = W_v^T columns)
so the PV matmul needs no transposes at all. A ones-column appended to V
yields the softmax denominator for free.

All matmul operands are fp16 (1 col/cycle on the PE, 10-bit mantissa keeps
rel-err ~5e-4); psum stays f32. Score matmuls for kt pairs land in one
2-bank psum tile so a single exp instruction covers both; the causal mask
only touches the 128-wide diagonal square.

The packed q/k projection weight is host-repacked so all three 128-wide
M-groups are full:  m0=[q_h0 q_h1]  m1=[q_h2 k_h0]  m2=[k_h1 k_h2]
(q rows pre-scaled by 1/8; v bias folded into the host-side epilogue).
"""

import os
from collections import deque
from contextlib import ExitStack

import numpy as np

import concourse.tile as tile
from concourse import bacc, mybir
from concourse.bass_utils import run_bass_kernel_spmd

F32 = mybir.dt.float32
F16 = mybir.dt.float16
F8 = mybir.dt.float8e4
DR = mybir.MatmulPerfMode.DoubleRow
AF = mybir.ActivationFunctionType

B, S, D = 2, 2048, 768
NH, DH = 12, 64
HPC = 3            # heads per core
GD = HPC * DH      # 192 z-cols per core
KT, QT = 128, 512  # key tile (partitions), q tile (psum free)
NKT, NQT = S // KT, S // QT   # 16, 4
NKD = D // 128     # 6 contraction tiles for the projections
WPK = 2 * GD       # 384 packed q/k projection rows


def build_bass():
    nc = bacc.Bacc(None)
    x8 = nc.dram_tensor("x8", [D, S], F8, kind="ExternalInput")
    xr = nc.dram_tensor("xr", [D, S], F8, kind="ExternalInput")
    wpk8 = nc.dram_tensor("wpk8", [D, WPK], F8, kind="ExternalInput")
    wpkr = nc.dram_tensor("wpkr", [D, WPK], F8, kind="ExternalInput")
    wv8 = nc.dram_tensor("wv8", [D, GD], F8, kind="ExternalInput")
    wvr = nc.dram_tensor("wvr", [D, GD], F8, kind="ExternalInput")
    woT = nc.dram_tensor("woT", [GD, D], F16, kind="ExternalInput")
    bqk = nc.dram_tensor("bqk", [128, 3], F32, kind="ExternalInput")
    vones = nc.dram_tensor("vones", [128, 64], F16, kind="ExternalInput")
    out_p = nc.dram_tensor("out_p", [S, D], F16, kind="ExternalOutput")

    with tile.TileContext(nc) as tc, ExitStack() as ctx:
        const = ctx.enter_context(tc.tile_pool(name="const", bufs=1))
        ps = ctx.enter_context(tc.tile_pool(name="ps", bufs=2, space="PSUM"))
        ps2 = ctx.enter_context(tc.tile_pool(name="ps2", bufs=2, space="PSUM"))
        psz = ctx.enter_context(tc.tile_pool(name="psz", bufs=2, space="PSUM"))
        expp = ctx.enter_context(tc.tile_pool(name="expp", bufs=9))
        small = ctx.enter_context(tc.tile_pool(name="small", bufs=4))

        x8_sb = const.tile([128, 3, 2, S], F8)
        xr_sb = const.tile([128, 3, 2, S], F8)
        wpk8_sb = const.tile([128, 3, 2, WPK], F8)
        wpkr_sb = const.tile([128, 3, 2, WPK], F8)
        wv8_sb = const.tile([128, 3, 2, GD], F8)
        wvr_sb = const.tile([128, 3, 2, GD], F8)
        wo_a = const.tile([128, D], F16)
        wo_b = const.tile([64, D], F16)
        bqk_sb = const.tile([128, 3], F32)
        qT_sb = const.tile([128, 2, S], F16)
        kT_sb = const.tile([128, 2, S], F16)
        v_aug = const.tile([128, NKT, HPC, DH + 1], F16)
        zT01 = const.tile([128, S], F16)
        zT2 = const.tile([64, S], F16)
        ones_stage = const.tile([128, 64], F16)

        # ---- loads: k-interleaved so the first projection k-pairs unblock
        # early. Early DMAs fan out over four queues (SP/Act/DVE/Pool) to
        # dodge the ~650ns per-DMA dispatch serialization on a single queue.
        # batched multi-tile DMAs keep the dispatch count low (each dispatch
        # costs ~650ns of queue time). The projection runs its main fp8 pass
        # first, so the x8/w8 arrays go first and the residual arrays follow.
        def tri(dram):
            return dram.rearrange("(kp two p) s -> p kp two s", p=128, two=2)

        x8_a, xr_a = tri(x8), tri(xr)
        wp8_a, wpr_a = tri(wpk8), tri(wpkr)
        wv8_a, wvr_a = tri(wv8), tri(wvr)
        # the first q-tile's operands stream first: weight slivers for m0,
        # then x8/xr column chunks; residual weights chase the mains.
        nc.sync.dma_start(out=wpk8_sb[:, :, :, 0:256], in_=wp8_a[:, :, :, 0:256])
        nc.scalar.dma_start(out=x8_sb[:, :, :, 0:QT], in_=x8_a[:, :, :, 0:QT])
        nc.sync.dma_start(out=wpkr_sb[:, :, :, 0:256], in_=wpr_a[:, :, :, 0:256])
        nc.scalar.dma_start(out=xr_sb[:, :, :, 0:QT], in_=xr_a[:, :, :, 0:QT])
        nc.sync.dma_start(out=bqk_sb[:], in_=bqk[:, :])
        nc.scalar.dma_start(
            out=wpk8_sb[:, :, :, 256:WPK], in_=wp8_a[:, :, :, 256:WPK]
        )
        nc.sync.dma_start(
            out=wpkr_sb[:, :, :, 256:WPK], in_=wpr_a[:, :, :, 256:WPK]
        )
        nc.gpsimd.dma_start(out=wv8_sb[:], in_=wv8_a[:])
        nc.gpsimd.dma_start(out=wvr_sb[:], in_=wvr_a[:])
        nc.gpsimd.dma_start(out=ones_stage[:], in_=vones[:, :])
        nc.vector.tensor_copy(
            out=v_aug[:, :, :, DH],
            in_=ones_stage[:, 0 : NKT * HPC].rearrange("p (t h) -> p t h", t=NKT),
        )
        nc.scalar.dma_start(
            out=x8_sb[:, :, :, QT : 2 * QT], in_=x8_a[:, :, :, QT : 2 * QT]
        )
        nc.sync.dma_start(
            out=xr_sb[:, :, :, QT : 2 * QT], in_=xr_a[:, :, :, QT : 2 * QT]
        )
        nc.scalar.dma_start(
            out=x8_sb[:, :, :, 2 * QT : S], in_=x8_a[:, :, :, 2 * QT : S]
        )
        nc.sync.dma_start(
            out=xr_sb[:, :, :, 2 * QT : S], in_=xr_a[:, :, :, 2 * QT : S]
        )
        nc.sync.dma_start(out=wo_a[:], in_=woT[0:128, :])
        nc.gpsimd.dma_start(out=wo_b[:], in_=woT[128:GD, :])

        # packed q/k projection m-groups: (col0, evict spec). m2 holds
        # [q2 k2]; its k2 rows evict to kT partitions 0:64 (cross-base copy)
        # so every head's q and k share a partition base for the PE.
        mgroups = [
            (0, [((0, 128), lambda n: qT_sb[0:128, 0, n * QT : (n + 1) * QT], 0)]),
            (128, [((0, 128), lambda n: kT_sb[0:128, 0, n * QT : (n + 1) * QT], 1)]),
            (256, [
                ((0, 64), lambda n: qT_sb[0:64, 1, n * QT : (n + 1) * QT], 2),
                ((64, 128), lambda n: kT_sb[0:64, 1, n * QT : (n + 1) * QT], 2),
            ]),
        ]

        proj_psums = {}
        PROJ_TERMS = [(wpk8_sb, x8_sb), (wpk8_sb, xr_sb), (wpkr_sb, x8_sb)]
        DESCALE = 1.0 / 1024.0  # host pre-scales x and W by 32 for fp8 range

        def proj_unit(mi, n, term):
            """One fp8 DoubleRow pass (3 k-pairs) of group (mi, n): term 0 is
            the w8.x8 main product, terms 1/2 add the x and W residuals."""
            c0, evicts = mgroups[mi]
            key = (mi, n)
            if key not in proj_psums:
                proj_psums[key] = ps.tile([128, QT], F32, tag="ps", name="projp")
            p = proj_psums[key]
            wsb, xsb = PROJ_TERMS[term]
            for kp in range(3):
                nc.tensor.matmul(
                    p[:, :],
                    lhsT=wsb[:, kp, :, c0 : c0 + 128],
                    rhs=xsb[:, kp, :, n * QT : (n + 1) * QT],
                    start=(term == 0 and kp == 0),
                    stop=(term == 2 and kp == 2),
                    perf_mode=DR,
                )
            if term == 2:
                del proj_psums[key]
                for (r0, r1), dst, bcol in evicts:
                    nc.vector.tensor_scalar(
                        out=dst(n),
                        in0=p[r0:r1, :],
                        scalar1=DESCALE,
                        scalar2=bqk_sb[r0:r1, bcol : bcol + 1],
                        op0=mybir.AluOpType.mult,
                        op1=mybir.AluOpType.add,
                    )

        vp_psums = {}
        V_TERMS = [(x8_sb, wv8_sb), (xr_sb, wv8_sb), (x8_sb, wvr_sb)]

        def v_unit(t, term):
            """V projection for keys-tile t (fp8 DoubleRow pass `term`):
            [128 keys, 192] psum; evict into v_aug[:, t, :, 0:64]."""
            if t not in vp_psums:
                vp_psums[t] = ps.tile([128, QT], F32, tag="ps", name="vp")
            p = vp_psums[t]
            xsb, wsb = V_TERMS[term]
            for kp in range(3):
                nc.tensor.matmul(
                    p[:, 0:GD],
                    lhsT=xsb[:, kp, :, t * 128 : (t + 1) * 128],
                    rhs=wsb[:, kp, :, :],
                    start=(term == 0 and kp == 0),
                    stop=(term == 2 and kp == 2),
                    perf_mode=DR,
                )
            if term == 2:
                del vp_psums[t]
                nc.vector.tensor_scalar_mul(
                    out=v_aug[:, t, :, 0:DH],
                    in0=p[:, 0:GD].rearrange("p (h d) -> p h d", h=HPC),
                    scalar1=DESCALE,
                )

        out_t = out_p.rearrange("(tp p) d -> tp p d", p=128)
        o_done = {}

        def o_proj_unit(t, n2, evict_eng=None, tail=False):
            key = t
            if key not in o_done:
                o_done[key] = expp.tile([128, D], F16, tag="osb", name="osb", bufs=3)
            ob = o_done[key]
            po = ps.tile([128, QT], F32, tag="ps")
            nc.tensor.matmul(
                po[:, 0:384],
                lhsT=zT01[:, t * 128 : (t + 1) * 128],
                rhs=wo_a[:, n2 * 384 : (n2 + 1) * 384],
                start=True,
                stop=False,
            )
            nc.tensor.matmul(
                po[:, 0:384],
                lhsT=zT2[:, t * 128 : (t + 1) * 128],
                rhs=wo_b[:, n2 * 384 : (n2 + 1) * 384],
                start=False,
                stop=True,
            )
            eng = evict_eng
            if eng is None:
                eng = "dve"
            if eng == "act":
                nc.scalar.activation(
                    out=ob[:, n2 * 384 : (n2 + 1) * 384], in_=po[:, 0:384], func=AF.Copy
                )
            else:
                nc.vector.tensor_copy(
                    out=ob[:, n2 * 384 : (n2 + 1) * 384], in_=po[:, 0:384]
                )
            if tail:
                (nc.sync if n2 == 0 else nc.scalar).dma_start(
                    out=out_t[t][:, n2 * 384 : (n2 + 1) * 384],
                    in_=ob[:, n2 * 384 : (n2 + 1) * 384],
                )
                if n2 == 1:
                    del o_done[key]
            elif n2 == 1:
                del o_done[key]
                (nc.sync if t % 2 == 0 else nc.scalar).dma_start(
                    out=out_t[t], in_=ob[:, :]
                )

        # background work queue of (key, fn), drained between attention
        # iterations. Queue order is topological, so force-draining "through
        # the last needed unit" preserves all producer->consumer ordering.
        work = deque()

        def q_proj(n, mis):
            for mi in mis:
                for term in range(3):
                    work.append(
                        (("proj", n, mi), lambda mi=mi, n=n, tm=term: proj_unit(mi, n, tm))
                    )

        def q_v(ts):
            for t in ts:
                for term in range(3):
                    work.append((("v", t), lambda t=t, tm=term: v_unit(t, tm)))

        def drain(k=1):
            for _ in range(k):
                if work:
                    work.popleft()[1]()

        def drain_all():
            while work:
                work.popleft()[1]()

        # head h's scores need these packed q/k groups
        PROJ_GROUPS_FOR_HEAD = {0: (0, 1), 1: (0, 1), 2: (2,)}

        def force_drain_for(h, qt):
            """Emit queued units up to the last one attention(h, qt) depends on."""
            needed = set()
            for n in range(qt + 1):
                for mi in PROJ_GROUPS_FOR_HEAD[h]:
                    needed.add(("proj", n, mi))
            for t in range(4 * qt + 4):
                needed.add(("v", t))
            last = -1
            for i, (key, _) in enumerate(work):
                if key in needed:
                    last = i
            for _ in range(last + 1):
                work.popleft()[1]()

        def qh(h):
            col, off = [(0, 0), (0, 64), (1, 0)][h]
            return qT_sb[off : off + 64, col, :]

        def kh(h):
            col, off = [(0, 0), (0, 64), (1, 0)][h]
            return kT_sb[off : off + 64, col, :]

        zdst = [zT01[0:64, :], zT01[64:128, :], zT2[0:64, :]]

        # PV matmuls are pipelined a few pairs behind their exp across block
        # boundaries, so the in-order PE FIFO never waits on the exp/mask
        # chain, not even at the end of a block.
        pvq = deque()  # (block_serial, pv_closure)
        blk_serial = [0]

        def pv_drain(depth):
            while len(pvq) > depth:
                pvq.popleft()[1]()

        def pv_flush(upto_serial):
            while pvq and pvq[0][0] <= upto_serial:
                pvq.popleft()[1]()

        def attention(h, qt, per_pair, last=False):
            """scores^T -> exp -> causal mask -> PV into zp. Score matmuls for
            kt pairs land in one 2-bank psum tile so a single exp covers both;
            start=True zeroes the whole bank, so the unwritten low columns of
            narrowed diagonal halves exp to 1.0 and are never consumed."""
            zp = psz.tile([DH + 1, QT], F32)
            nkt = 4 * qt + 4
            blk = blk_serial[0]
            blk_serial[0] += 1

            def pv(kt, es_ap, lo):
                nc.tensor.matmul(
                    zp[:, lo:QT],
                    lhsT=v_aug[:, kt, h, :],
                    rhs=es_ap[:, lo:QT],
                    start=(kt == 0),
                    stop=(kt == nkt - 1),
                )

            def pv_q(kt, es_ap, lo):
                # last block: quarter q of zp sees its final write at
                # kt = nkt-4+q, so it can be normalized immediately after.
                # Only that closing kt is split off (multiple start=True
                # writes into one bank would re-zero siblings' columns).
                r = kt - (nkt - 4)
                if r < 0:
                    nc.tensor.matmul(
                        zp[:, 0:QT],
                        lhsT=v_aug[:, kt, h, :],
                        rhs=es_ap[:, 0:QT],
                        start=(kt == 0),
                        stop=False,
                        skip_group_check=True,
                    )
                else:
                    c0 = 128 * r
                    nc.tensor.matmul(
                        zp[:, c0 : c0 + 128],
                        lhsT=v_aug[:, kt, h, :],
                        rhs=es_ap[:, c0 : c0 + 128],
                        start=False,
                        stop=True,
                        skip_group_check=True,
                    )
                    if c0 + 128 < QT:
                        nc.tensor.matmul(
                            zp[:, c0 + 128 : QT],
                            lhsT=v_aug[:, kt, h, :],
                            rhs=es_ap[:, c0 + 128 : QT],
                            start=False,
                            stop=(kt == nkt - 1),
                            skip_group_check=True,
                        )
                    stairs(r)

            st_rec, st_bc = {}, {}

            def qcols(q):
                return slice(128 * q, 128 * (q + 1))

            def stairs(step):
                # stage skew keeps the in-order PE FIFO free of long waits:
                # each quarter's PE pieces (bc, o-proj) are emitted one or two
                # pv-steps after the pv that closed the quarter's psum group.
                if step >= 1:
                    st_bc[step - 1] = norm_bc(st_rec[step - 1], qcols(step - 1))
                st_rec[step] = norm_recip(zp, qcols(step))
                if step >= 1:
                    norm_post(zp, h, qt, st_bc[step - 1], qcols(step - 1))
                if step >= 2:
                    t = 4 * qt + step - 2
                    o_proj_unit(t, 0, evict_eng="dve", tail=True)
                    o_proj_unit(t, 1, evict_eng="act", tail=True)
                drain(2)

            def stairs_final():
                st_bc[3] = norm_bc(st_rec[3], qcols(3))
                norm_post(zp, h, qt, st_bc[3], qcols(3))
                for t in (4 * qt + 2, 4 * qt + 3):
                    o_proj_unit(t, 0, evict_eng="dve", tail=True)
                    o_proj_unit(t, 1, evict_eng="act", tail=True)

            for kp in range(nkt // 2):
                kt0 = 2 * kp
                rr0 = kt0 - 4 * qt
                lo_pair = 128 * rr0 if rr0 > 0 else 0
                sp = ps2.tile([128, 2, QT], F32, tag="ps2")
                los = []
                for i in (0, 1):
                    kt = kt0 + i
                    rr = kt - 4 * qt
                    lo = 128 * rr if rr > 0 else 0
                    los.append(lo)
                    nc.tensor.matmul(
                        sp[:, i, lo:QT],
                        lhsT=kh(h)[:, kt * 128 : (kt + 1) * 128],
                        rhs=qh(h)[:, qt * QT + lo : (qt + 1) * QT],
                        start=True,
                        stop=True,
                    )
                es = expp.tile([128, 2, QT], F16, tag="expp")
                nc.scalar.activation(
                    out=es[:, :, lo_pair:QT], in_=sp[:, :, lo_pair:QT], func=AF.Exp
                )
                for i in (0, 1):
                    kt = kt0 + i
                    rr = kt - 4 * qt
                    lo = los[i]
                    if rr >= 0:  # diagonal square: zero where key > query
                        hi = min(lo + 128, QT)
                        nc.gpsimd.affine_select(
                            out=es[:, i, lo:hi],
                            in_=es[:, i, lo:hi],
                            compare_op=mybir.AluOpType.is_ge,
                            fill=0.0,
                            base=0,
                            channel_multiplier=-1,
                            pattern=[[1, hi - lo]],
                        )
                    fn = pv_q if last else pv
                    pvq.append(
                        (blk, lambda kt=kt, es=es, i=i, lo=lo, fn=fn: fn(kt, es[:, i], lo))
                    )
                drain(per_pair)
                pv_drain(2 if last else int(os.environ.get("KPVD", "8")))
            if last:
                pv_flush(blk)
                stairs_final()
            return zp, blk

        def norm_recip(zp, cols):
            rec = small.tile([1, QT], F16, tag="rec")
            with nc.allow_low_precision(reason="fp16 normalize"):
                nc.vector.reciprocal(rec[:, cols], zp[DH : DH + 1, cols])
            return rec

        def norm_bc(rec, cols):
            bc_sb = small.tile([64, QT], F16, tag="bcsb")
            nc.gpsimd.partition_broadcast(bc_sb[:, cols], rec[0:1, cols])
            return bc_sb

        def norm_post(zp, h, qt, bc_sb, cols):
            nc.vector.tensor_mul(
                zdst[h][:, qt * QT : (qt + 1) * QT][:, cols],
                zp[0:DH, cols],
                bc_sb[:, cols],
            )

        def normalize(zp, h, qt, cols=slice(0, QT)):
            rec = norm_recip(zp, cols)
            bc_sb = norm_bc(rec, cols)
            norm_post(zp, h, qt, bc_sb, cols)

        # ---- schedule ----
        # prologue: only what attention(h0, qt0) needs, ordered by when each
        # term's operands land; the rest queues up.
        for term in range(3):
            proj_unit(0, 0, term=term)
            proj_unit(1, 0, term=term)
            for t in range(4):
                v_unit(t, term)
        q_proj(0, mis=(2,))
        for n in range(1, NQT):
            q_proj(n, mis=(0, 1))
            q_v(range(4 * n, 4 * n + 2))
            q_proj(n, mis=(2,))
            q_v(range(4 * n + 2, 4 * n + 4))

        # qt2/qt3 blocks interleave so the exp-dense last q-tile spreads
        # over the whole back half instead of saturating Act at the end.
        BLOCKS = [(qt, h) for qt in range(NQT) for h in range(HPC)]

        deferred = []  # O-proj units held back to feed the exp-dense end

        def flush_pending(pending):
            pv_flush(pending[3])  # pending block's PV accumulation done
            normalize(*pending[:3])
            ph, pqt = pending[1], pending[2]
            if ph == HPC - 1:  # whole q-tile normalized -> O-proj ready
                for t in range(4 * pqt, 4 * pqt + 4):
                    for n2 in range(2):
                        unit = (("o", pqt), lambda t=t, n2=n2: o_proj_unit(t, n2))
                        (deferred if pqt <= 1 else work).append(unit)

        pending = None
        for qt, h in BLOCKS:
            per_pair = [int(c) for c in os.environ.get("KCAD", "4111")][qt]
            force_drain_for(h, qt)
            if (qt, h) == (3, 0):
                work.extend(deferred)
                deferred.clear()
            is_last = (qt, h) == BLOCKS[-1]
            if is_last:
                flush_pending(pending)
                pending = None
            zp, blk = attention(h, qt, per_pair, last=is_last)
            if pending is not None:
                flush_pending(pending)
            if not is_last:
                pending = (zp, h, qt, blk)
        drain_all()
    nc.finalize()
    return nc


_NC_CACHE = {}


def _f8(a):
    """main fp8 + residual fp8 (inputs pre-scaled x32, so both normal-range)."""
    import ml_dtypes

    f8 = ml_dtypes.float8_e4m3
    a = np.ascontiguousarray(a, np.float32)
    m = a.astype(f8)
    r = (a - m.astype(np.float32)).astype(f8)
    return np.ascontiguousarray(m), np.ascontiguousarray(r)


def make_in_maps(x, W_qkv, b_qkv, W_o):
    in_maps = []
    for c in range(8):
        b, g = divmod(c, 4)
        hs = [HPC * g + i for i in range(HPC)]
        qr = [np.arange(64 * h, 64 * h + 64) for h in hs]
        w_q = [W_qkv[i] * 0.125 for i in qr]
        w_k = [W_qkv[768 + i] for i in qr]
        w_v = [W_qkv[1536 + i] for i in qr]
        b_q = [b_qkv[i] * 0.125 for i in qr]
        b_k = [b_qkv[768 + i] for i in qr]
        # packed rows: m0=[q0 q1] m1=[k0 k1] m2=[q2 k2]
        wpk = np.concatenate(
            [w_q[0], w_q[1], w_k[0], w_k[1], w_q[2], w_k[2]], axis=0
        )
        wv = np.concatenate([w_v[0], w_v[1], w_v[2]], axis=0)
        bqk_col = np.zeros((128, 3), np.float32)
        bqk_col[:, 0] = np.concatenate([b_q[0], b_q[1]])
        bqk_col[:, 1] = np.concatenate([b_k[0], b_k[1]])
        bqk_col[:, 2] = np.concatenate([b_q[2], b_k[2]])
        # fp8 triples (values pre-scaled by 32; 1/1024 folded into evictions)
        xm = _f8(32.0 * x[b].T)
        wpm = _f8(32.0 * wpk.T)
        wvm = _f8(32.0 * wv.T)
        in_maps.append(
            {
                "x8": xm[0], "xr": xm[1],
                "wpk8": wpm[0], "wpkr": wpm[1],
                "wv8": wvm[0], "wvr": wvm[1],
                "woT": np.ascontiguousarray(
                    W_o[:, GD * g : GD * (g + 1)].T.astype(np.float16)
                ),
                "bqk": bqk_col,
                "vones": np.ones((128, 64), np.float16),
            }
        )
    return in_maps


def make_in_maps_for_test(inputs):
    return make_in_maps(
        np.asarray(inputs["x"], np.float32),
        np.asarray(inputs["W_qkv"], np.float32),
        np.asarray(inputs["b_qkv"], np.float32),
        np.asarray(inputs["W_o"], np.float32),
    )


def kernel(x, W_qkv, b_qkv, W_o, b_o):
    x = np.asarray(x, np.float32)
    W_qkv = np.asarray(W_qkv, np.float32)
    b_qkv = np.asarray(b_qkv, np.float32)
    W_o = np.asarray(W_o, np.float32)
    b_o = np.asarray(b_o, np.float32)

    if "nc" not in _NC_CACHE:
        _NC_CACHE["nc"] = build_bass()
    nc = _NC_CACHE["nc"]

    in_maps = make_in_maps(x, W_qkv, b_qkv, W_o)

    res = run_bass_kernel_spmd(
        nc,
        in_maps,
        list(range(8)),
        trace=bool(int(os.environ.get("KERNEL_TRACE", "0"))),
    )
    _NC_CACHE["last_results"] = res

    out = np.zeros((B, S, D), np.float32)
    for c in range(8):
        out[c // 4] += res.results[c]["out_p"].astype(np.float32)
    out += b_qkv[1536:] @ W_o.T + b_o
    return out
